# revision 1
# baseline (speedup 1.0000x reference)
"""Trainium2 Bass kernel for nn_BertCNN (3x BERT-small encoder + CNN maxpool head).

Strategy: data-parallel over batch across 8 NeuronCores. Each core gets 4
samples x 3 branches (q/a/b) = 12 sequences of 256 tokens and runs the full
4-layer BERT encoder + conv/maxpool head + fc on-device, emitting a [4, 4]
logits tile. Host concatenates core outputs into the final [32, 4].

Dataflow per core (all matmuls bf16 operands, fp32 PSUM accumulation):
  - token embeddings gathered on-device via indirect DMA from word_emb
  - residual stream kept token-major fp32 in SBUF; a feature-major bf16
    transposed copy (via PE transpose) feeds the QKV / FFN matmuls
  - sequences processed in pairs so projection / FFN1 matmuls run at the
    full 512-wide moving-operand limit
  - attention computed in S^T = [key, query] layout so the ragged-length mask
    folds into the Exp activation as a per-partition bias, and exp(S^T) is
    directly the lhsT of the context matmul; softmax denominators come from a
    ones-column appended to V (row-sums land in PSUM next to the context)
  - conv head runs feature-major ([filters, windows]) so global maxpool is a
    per-partition free-axis reduce_max; ragged window validity is a -1e30
    penalty added before the max
"""

import numpy as np
import ml_dtypes

V, D, H, DH, NL, FF = 30522, 512, 8, 64, 4, 2048
NF, NCLS, B, S = 256, 4, 32, 256
NCORES, SPC = 8, 4
NSEQ = 3 * SPC          # 12 sequences per core
NPAIR = NSEQ // 2       # 6 sequence pairs
NT = S // 128           # 2 token tiles per sequence
ND = D // 128           # 4 feature tiles
NFT = FF // 128         # 16 FFN hidden tiles
NCH = 2 * 3 * 3         # 18 fc chunks of 128 (3 branches x 3 kernels x 2 ftiles)
SW = S + 1              # 257 padded positions for conv

BF = ml_dtypes.bfloat16
_CACHE = {}


def _flags(inputs):
    z = lambda a: bool(np.all(np.asarray(a) == 0))
    o = lambda a: bool(np.all(np.asarray(a) == 1))
    return {
        "bqk": z(inputs["bq"]) and z(inputs["bk"]),
        "bv": z(inputs["bv"]),
        "bo": z(inputs["bo"]),
        "bi": z(inputs["bi"]),
        "bo2": z(inputs["bo2"]),
        "ln": all(o(inputs[k]) for k in ("emb_ln_s", "ln1_s", "ln2_s"))
        and all(z(inputs[k]) for k in ("emb_ln_b", "ln1_b", "ln2_b")),
        "cb": z(inputs["conv_b1"]) and z(inputs["conv_b2"]) and z(inputs["conv_b3"]),
        "fcb": z(inputs["fc_b"]),
    }


def _build_program(fl, debug=False):
    import contextlib
    import concourse.bass as bass
    import concourse.mybir as mybir
    import concourse.tile as tile
    from concourse import bacc
    from concourse.masks import make_identity

    F32, BF16, I32 = mybir.dt.float32, mybir.dt.bfloat16, mybir.dt.int32
    AL, AF = mybir.AluOpType, mybir.ActivationFunctionType

    nc = bacc.Bacc("TRN2", target_bir_lowering=False, debug=False,
                   num_devices=NCORES)

    di = lambda n, s, d: nc.dram_tensor(n, s, d, kind="ExternalInput").ap()
    word = di("word_emb", [V, D], F32)
    ids_d = di("ids", [NSEQ, S], I32)
    mb_d = di("maskbias", [NSEQ, NT, 128], F32)
    posty_d = di("posty", [NT, 128, D], F32)
    cmask_d = di("convmask", [NSEQ, SW], BF16)
    cpen_d = di("convpen", [NSEQ, 3, SW], F32)
    wq_d = [di(f"wq{l}", [ND, 128, D], BF16) for l in range(NL)]
    wk_d = [di(f"wk{l}", [ND, 128, D], BF16) for l in range(NL)]
    wv_d = [di(f"wv{l}", [ND, 128, D], BF16) for l in range(NL)]
    wo_d = [di(f"wo{l}", [ND, 128, D], BF16) for l in range(NL)]
    wi_d = [di(f"wi{l}", [ND, 128, FF], BF16) for l in range(NL)]
    wo2_d = [di(f"wo2{l}", [NFT, 128, D], BF16) for l in range(NL)]
    cw_d = [di(f"cw{k}", [k, ND, 128, NF], BF16) for k in (1, 2, 3)]
    fcw_d = di("fcw", [NCH, 128, NCLS], F32)
    if not fl["bqk"]:
        bq_d = [di(f"bq{l}", [ND, 128], F32) for l in range(NL)]
        bk_d = [di(f"bk{l}", [ND, 128], F32) for l in range(NL)]
    if not fl["bv"]:
        bv_d = [di(f"bv{l}", [D], F32) for l in range(NL)]
    if not fl["bo"]:
        bo_d = [di(f"bo{l}", [D], F32) for l in range(NL)]
    if not fl["bi"]:
        bi_d = [di(f"bi{l}", [NFT, 128], F32) for l in range(NL)]
    if not fl["bo2"]:
        bo2_d = [di(f"bo2{l}", [D], F32) for l in range(NL)]
    if not fl["ln"]:
        elns_d = di("lnes", [D], F32)
        elnb_d = di("lneb", [D], F32)
        ln1s_d = [di(f"ln1s{l}", [D], F32) for l in range(NL)]
        ln1b_d = [di(f"ln1b{l}", [D], F32) for l in range(NL)]
        ln2s_d = [di(f"ln2s{l}", [D], F32) for l in range(NL)]
        ln2b_d = [di(f"ln2b{l}", [D], F32) for l in range(NL)]
    if not fl["cb"]:
        cb_d = di("convb", [3, 2, 128], F32)
    if not fl["fcb"]:
        fcb_d = di("fcb", [NCLS], F32)

    out_d = nc.dram_tensor("out", [SPC, NCLS], F32, kind="ExternalOutput").ap()
    if debug:
        dbgx_d = nc.dram_tensor("dbgx", [NSEQ, NT, 128, D], BF16,
                                kind="ExternalOutput").ap()
        dbgr_d = nc.dram_tensor("dbgr", [128, NCH, SPC], F32,
                                kind="ExternalOutput").ap()

    with tile.TileContext(nc) as tc, contextlib.ExitStack() as ctx:
        consts = ctx.enter_context(tc.tile_pool(name="consts", bufs=1))
        state = ctx.enter_context(tc.tile_pool(name="state", bufs=1))
        wts = ctx.enter_context(tc.tile_pool(name="wts", bufs=1))
        big = ctx.enter_context(tc.tile_pool(name="big", bufs=1))
        work = ctx.enter_context(tc.tile_pool(name="work", bufs=2))
        small = ctx.enter_context(tc.tile_pool(name="small", bufs=4))
        ps_mm = ctx.enter_context(tc.tile_pool(name="ps_mm", bufs=4, space="PSUM"))
        ps_ctx = ctx.enter_context(tc.tile_pool(name="ps_ctx", bufs=3, space="PSUM"))
        ps_tp = ctx.enter_context(tc.tile_pool(name="ps_tp", bufs=1, space="PSUM"))

        # ---- constants ----
        ident = consts.tile([128, 128], BF16, tag="ident")
        make_identity(nc, ident[:])
        eps_t = consts.tile([128, 1], F32, tag="eps")
        nc.vector.memset(eps_t[:], 1e-12)
        ids_sb = consts.tile([128, NSEQ * NT], I32, tag="ids")
        nc.sync.dma_start(out=ids_sb[:],
                          in_=ids_d.rearrange("s (t p) -> p (s t)", p=128))
        mb_sb = consts.tile([128, NSEQ * NT], F32, tag="mb")
        nc.sync.dma_start(out=mb_sb[:], in_=mb_d.rearrange("s t p -> p (s t)"))
        posty = consts.tile([128, NT, D], F32, tag="posty")
        nc.sync.dma_start(out=posty[:], in_=posty_d.rearrange("t p d -> p t d"))

        bcast = lambda ap, n: ap[None, :].to_broadcast([128, n])
        if not fl["ln"]:
            elns = consts.tile([128, D], F32, tag="elns")
            nc.sync.dma_start(out=elns[:], in_=bcast(elns_d, D))
            elnb = consts.tile([128, D], F32, tag="elnb")
            nc.sync.dma_start(out=elnb[:], in_=bcast(elnb_d, D))

        # persistent per-sequence / per-pair state
        x_tm = [state.tile([128, NT, D], BF16, tag=f"x{q}", name=f"x{q}")
                for q in range(NSEQ)]
        xT = [state.tile([128, 2, NT, ND, 128], BF16, tag=f"xT{q}",
                         name=f"xT{q}") for q in range(NPAIR)]
        rep = state.tile([128, NCH, SPC], F32, tag="rep")

        def ln_stats(src_ap):
            """Start LN of src [128, D] f32; returns handle for ln_apply."""
            st = small.tile([128, 6], F32, tag="st")
            mv = small.tile([128, 2], F32, tag="mv")
            nc.vector.bn_stats(out=st[:], in_=src_ap)
            nc.vector.bn_aggr(out=mv[:], in_=st[:])
            sd = small.tile([128, 1], F32, tag="sd")
            nc.scalar.activation(out=sd[:], in_=mv[:, 1:2], func=AF.Sqrt,
                                 bias=eps_t[:], scale=1.0)
            nmr = small.tile([128, 2], F32, tag="nmr")
            nc.vector.reciprocal(out=nmr[:, 1:2], in_=sd[:])
            # nmr[:,0] = -mean * rstd  (bias for the fused ACT Identity apply)
            nc.vector.tensor_scalar(out=nmr[:, 0:1], in0=mv[:, 0:1],
                                    scalar1=nmr[:, 1:2], scalar2=-1.0,
                                    op0=AL.mult, op1=AL.mult)
            return nmr

        def ln_apply(src_ap, dst_ap, nmr, s_tile, b_tile):
            """dst = (src - m) * rstd [* s + b] via one ACT pass."""
            nc.scalar.activation(out=dst_ap, in_=src_ap, func=AF.Identity,
                                 bias=nmr[:, 0:1], scale=nmr[:, 1:2])
            if s_tile is not None:
                nc.vector.tensor_tensor(out=dst_ap, in0=dst_ap, in1=s_tile[:],
                                        op=AL.mult)
            if b_tile is not None:
                nc.vector.tensor_tensor(out=dst_ap, in0=dst_ap, in1=b_tile[:],
                                        op=AL.add)

        def layernorm(src_ap, dst_ap, s_tile, b_tile):
            ln_apply(src_ap, dst_ap, ln_stats(src_ap), s_tile, b_tile)

        def ln_stats_batch(rs):
            mvs = []
            for i, r in enumerate(rs):
                st = small.tile([128, 6], F32, tag="st", name=f"st{i}")
                mv = small.tile([128, 2], F32, tag="mv", name=f"mv{i}")
                nc.vector.bn_stats(out=st[:], in_=r[:])
                nc.vector.bn_aggr(out=mv[:], in_=st[:])
                mvs.append(mv)
            nmrs = []
            for i, mv in enumerate(mvs):
                sd = small.tile([128, 1], F32, tag="sd", name=f"sd{i}")
                nc.scalar.activation(out=sd[:], in_=mv[:, 1:2], func=AF.Sqrt,
                                     bias=eps_t[:], scale=1.0)
                nmr = small.tile([128, 2], F32, tag="nmr", name=f"nmr{i}")
                nc.vector.reciprocal(out=nmr[:, 1:2], in_=sd[:])
                nc.vector.tensor_scalar(out=nmr[:, 0:1], in0=mv[:, 0:1],
                                        scalar1=nmr[:, 1:2], scalar2=-1.0,
                                        op0=AL.mult, op1=AL.mult)
                nmrs.append(nmr)
            return nmrs

        def cast_transpose(dst_tile, seq, tt, src_ap):
            """src [128tok, D] bf16 -> dst_tile[:, seq%2, tt, :, :] via DMA
            transpose (runs on the otherwise-idle DMA queues)."""
            nc.sync.dma_start_transpose(dst_tile[:, seq % 2, tt, :, :], src_ap)

        def embed(seq):
            for tt in range(NT):
                g = work.tile([128, D], F32, tag="r")
                ti = seq * NT + tt
                nc.gpsimd.indirect_dma_start(
                    out=g[:], out_offset=None, in_=word[:],
                    in_offset=bass.IndirectOffsetOnAxis(
                        ap=ids_sb[:, ti:ti + 1], axis=0))
                nc.vector.tensor_tensor(out=g[:], in0=g[:], in1=posty[:, tt, :],
                                        op=AL.add)
                dst = x_tm[seq][:, tt, :]
                layernorm(g[:], dst,
                          None if fl["ln"] else elns,
                          None if fl["ln"] else elnb)
                cast_transpose(xT[seq // 2], seq, tt, dst)

        # ---- encoder layers (sequences processed in pairs, software-
        # pipelined: the next pair's QKV / V projections are emitted inside
        # this pair's LN phases so the PE never idles on the LN chains) ----
        def load_layer_weights(l):
            w = {}
            w["wq"] = wts.tile([128, ND, D], BF16, tag="wq", name=f"wq_{l}")
            for dt in range(ND):
                nc.sync.dma_start(out=w["wq"][:, dt, :], in_=wq_d[l][dt])
            w["wk"] = wts.tile([128, ND, D], BF16, tag="wk", name=f"wk_{l}")
            for dt in range(ND):
                nc.sync.dma_start(out=w["wk"][:, dt, :], in_=wk_d[l][dt])
            w["wv"] = wts.tile([128, ND, D], BF16, tag="wv", name=f"wv_{l}")
            for dt in range(ND):
                nc.sync.dma_start(out=w["wv"][:, dt, :], in_=wv_d[l][dt])
            w["wo"] = wts.tile([128, ND, D], BF16, tag="wo", name=f"wo_{l}")
            for dt in range(ND):
                nc.sync.dma_start(out=w["wo"][:, dt, :], in_=wo_d[l][dt])
            w["wi"] = wts.tile([128, ND, FF], BF16, tag="wi", name=f"wi_{l}")
            for dt in range(ND):
                nc.sync.dma_start(out=w["wi"][:, dt, :], in_=wi_d[l][dt])
            w["wo2"] = wts.tile([128, NFT, D], BF16, tag="wo2", name=f"wo2_{l}")
            for ft in range(0, NFT, 4):
                nc.sync.dma_start(
                    out=w["wo2"][:, ft:ft + 4, :],
                    in_=wo2_d[l][ft:ft + 4].rearrange("t p o -> p t o"))
            if not fl["bqk"]:
                w["bq"] = consts.tile([128, ND], F32, tag="bq", name=f"bq_{l}")
                nc.sync.dma_start(out=w["bq"][:], in_=bq_d[l].rearrange("t p -> p t"))
                w["bk"] = consts.tile([128, ND], F32, tag="bk", name=f"bk_{l}")
                nc.sync.dma_start(out=w["bk"][:], in_=bk_d[l].rearrange("t p -> p t"))
            if not fl["bv"]:
                w["bv"] = consts.tile([128, D], F32, tag="bv", name=f"bv_{l}")
                nc.sync.dma_start(out=w["bv"][:], in_=bcast(bv_d[l], D))
            if not fl["bo"]:
                w["bo"] = consts.tile([128, D], F32, tag="bo", name=f"bo_{l}")
                nc.sync.dma_start(out=w["bo"][:], in_=bcast(bo_d[l], D))
            if not fl["bi"]:
                w["bi"] = consts.tile([128, NFT], F32, tag="bi", name=f"bi_{l}")
                nc.sync.dma_start(out=w["bi"][:], in_=bi_d[l].rearrange("t p -> p t"))
            if not fl["bo2"]:
                w["bo2"] = consts.tile([128, D], F32, tag="bo2", name=f"bo2_{l}")
                nc.sync.dma_start(out=w["bo2"][:], in_=bcast(bo2_d[l], D))
            if not fl["ln"]:
                for nm, dd in (("ln1s", ln1s_d), ("ln1b", ln1b_d),
                               ("ln2s", ln2s_d), ("ln2b", ln2b_d)):
                    w[nm] = consts.tile([128, D], F32, tag=nm, name=f"{nm}_{l}")
                    nc.sync.dma_start(out=w[nm][:], in_=bcast(dd[l], D))
            return w

        def qkv_pair(pr, w):
            """qT/kT feature-major bf16 for both seqs (Wq host-scaled 1/8)."""
            xts = xT[pr]
            qT = work.tile([128, ND, 2 * S], BF16, tag="qT", name=f"qT{pr}")
            kT = work.tile([128, ND, 2 * S], BF16, tag="kT", name=f"kT{pr}")
            for dst_t, wt, which in ((qT, w["wq"], "q"), (kT, w["wk"], "k")):
                for ot in range(ND):
                    ps = ps_mm.tile([128, 2 * S], F32, tag="mm")
                    for dt in range(ND):
                        nc.tensor.matmul(
                            ps[:], wt[:, dt, ot * 128:(ot + 1) * 128],
                            xts[:, :, :, dt, :], start=dt == 0,
                            stop=dt == ND - 1)
                    if fl["bqk"]:
                        nc.scalar.copy(out=dst_t[:, ot, :], in_=ps[:])
                    else:
                        bt = w["bq"] if which == "q" else w["bk"]
                        nc.scalar.activation(
                            out=dst_t[:, ot, :], in_=ps[:], func=AF.Identity,
                            bias=bt[:, ot:ot + 1], scale=1.0)
            return qT, kT

        def v_pair(pr, w):
            """V token-major bf16 with a ones column per head, both seqs."""
            xts = xT[pr]
            vAs = []
            for si in range(2):
                so = si * S
                vA = work.tile([128, NT, H, DH + 1], BF16, tag="vA",
                               name=f"vA{pr}_{si}")
                nc.vector.memset(vA[:, :, :, DH:DH + 1], 1.0)
                for tt in range(NT):
                    ps = ps_mm.tile([128, D], F32, tag="mm")
                    for dt in range(ND):
                        nc.tensor.matmul(
                            ps[:], xts[:, si, tt, dt, :],
                            w["wv"][:, dt, :], start=dt == 0, stop=dt == ND - 1)
                    if fl["bv"]:
                        nc.vector.tensor_copy(
                            out=vA[:, tt, :, 0:DH],
                            in_=ps.rearrange("p (h d) -> p h d", h=H))
                    else:
                        nc.vector.tensor_tensor(
                            out=vA[:, tt, :, 0:DH],
                            in0=ps.rearrange("p (h d) -> p h d", h=H),
                            in1=w["bv"].rearrange("p (h d) -> p h d", h=H),
                            op=AL.add)
                vAs.append(vA)
            return vAs

        att_state = {}

        def att_scores(pr, g_, qk):
            """S^T + exp for head-group g_ of both seqs of pair pr."""
            qT, kT = qk
            for si in range(2):
                seq = 2 * pr + si
                so = si * S
                eT = work.tile([128, 4, NT, S], BF16, tag="eT",
                               name=f"eT{pr}_{g_}_{si}")
                att_state[(pr, g_, si)] = eT
                for hi in range(4):
                    h = g_ * 4 + hi
                    ot, hh = h // 2, (h % 2) * DH
                    for kt in range(NT):
                        ps = ps_mm.tile([128, S], F32, tag="mm")
                        nc.tensor.matmul(
                            ps[:],
                            kT[hh:hh + DH, ot, so + kt * 128:so + (kt + 1) * 128],
                            qT[hh:hh + DH, ot, so:so + S],
                            start=True, stop=True)
                        nc.scalar.activation(
                            out=eT[:, hi, kt, :], in_=ps[:], func=AF.Exp,
                            bias=mb_sb[:, seq * NT + kt:seq * NT + kt + 1],
                            scale=1.0)

        def att_ctx(pr, g_, vAs, ctxbs):
            """ctx (+denominator) matmuls and DVE normalization for group g_."""
            for si in range(2):
                eT, vA, ctxb = att_state.pop((pr, g_, si)), vAs[si], ctxbs[si]
                for qt in range(NT):
                    cps = ps_ctx.tile([128, 4 * (DH + 1)], F32, tag="ctx",
                                      name=f"ctx{si}_{qt}_{g_}")
                    for hi in range(4):
                        h = g_ * 4 + hi
                        sl = slice(hi * (DH + 1), (hi + 1) * (DH + 1))
                        for kt in range(NT):
                            nc.tensor.matmul(
                                cps[:, sl],
                                eT[:, hi, kt, qt * 128:(qt + 1) * 128],
                                vA[:, kt, h, :], start=kt == 0,
                                stop=kt == NT - 1)
                    rcp = small.tile([128, 4], F32, tag="rcp")
                    nc.vector.reciprocal(
                        out=rcp[:],
                        in_=cps.rearrange("p (h c) -> p h c", c=DH + 1)[:, :, DH])
                    for hi in range(4):
                        h = g_ * 4 + hi
                        base = hi * (DH + 1)
                        nc.vector.tensor_scalar_mul(
                            out=ctxb[:, qt, h * DH:(h + 1) * DH],
                            in0=cps[:, base:base + DH], scalar1=rcp[:, hi:hi + 1])

        def residual_ln(rs, dsts, s_tile, b_tile):
            """Pipelined LN of the four r tiles into dsts (bf16 x_tm slices)."""
            nmrs = ln_stats_batch(rs)
            for i in range(4):
                # out = r * rstd + (-mean * rstd)
                nc.vector.tensor_scalar(
                    out=dsts[i], in0=rs[i][:], scalar1=nmrs[i][:, 1:2],
                    scalar2=nmrs[i][:, 0:1], op0=AL.mult, op1=AL.add)
                if s_tile is not None:
                    nc.vector.tensor_tensor(out=dsts[i], in0=dsts[i],
                                            in1=s_tile[:], op=AL.mult)
                if b_tile is not None:
                    nc.vector.tensor_tensor(out=dsts[i], in0=dsts[i],
                                            in1=b_tile[:], op=AL.add)

        # ---- conv + fc weights (own tags so conv can overlap layer 3) ----
        cw = {}
        ti = 0
        for ki, k in enumerate((1, 2, 3)):
            for j in range(k):
                t = wts.tile([128, ND, NF], BF16, tag=f"cw{ti}",
                             name=f"cwt{k}_{j}")
                nc.sync.dma_start(out=t[:],
                                  in_=cw_d[ki][j].rearrange("t p f -> p t f"))
                cw[(k, j)] = t
                ti += 1
        fcw = consts.tile([128, NCH, NCLS], F32, tag="fcw")
        nc.sync.dma_start(out=fcw[:], in_=fcw_d.rearrange("c p n -> p c n"))
        if not fl["cb"]:
            cb = consts.tile([128, 3, 2], F32, tag="cb")
            nc.sync.dma_start(out=cb[:], in_=cb_d.rearrange("k t p -> p k t"))
        if not fl["fcb"]:
            fcb = consts.tile([4, NCLS], F32, tag="fcb")
            nc.sync.dma_start(out=fcb[:],
                              in_=fcb_d[None, :].to_broadcast([4, NCLS]))

        border = {0: 0, 1: 2, 2: 1}  # branch q/a/b -> fc chunk order q,b,a

        def conv_seq(seq):
            br, sample = seq // SPC, seq % SPC
            xcv = work.tile([128, ND, SW], BF16, tag="xcv", name=f"xcv{seq}")
            nc.vector.memset(xcv[:], 0.0)
            cm = work.tile([128, SW], BF16, tag="cm", name=f"cm{seq}")
            nc.sync.dma_start(out=cm[:],
                              in_=cmask_d[seq][None, :].to_broadcast([128, SW]))
            for dt in range(ND):
                nc.vector.tensor_tensor(
                    out=xcv[:, dt, 0:S].rearrange("p (t c) -> p t c", t=NT),
                    in0=xT[seq // 2][:, seq % 2, :, dt, :],
                    in1=cm[:, 0:S].rearrange("p (t c) -> p t c", t=NT),
                    op=AL.mult)
            for ki, k in enumerate((1, 2, 3)):
                pen = work.tile([128, SW], F32, tag="pen", name=f"pen{seq}_{k}")
                nc.sync.dma_start(
                    out=pen[:],
                    in_=cpen_d[seq, ki][None, :].to_broadcast([128, SW]))
                nw = SW - k + 1
                for ft in range(2):
                    ps = ps_mm.tile([128, SW], F32, tag="mm")
                    idx = 0
                    for dt in range(ND):
                        for j in range(k):
                            nc.tensor.matmul(
                                ps[:, 0:nw],
                                cw[(k, j)][:, dt, ft * 128:(ft + 1) * 128],
                                xcv[:, dt, j:j + nw],
                                start=idx == 0, stop=idx == ND * k - 1)
                            idx += 1
                    cvt = work.tile([128, SW], F32, tag="cvt", name=f"cv{seq}_{k}_{ft}")
                    nc.vector.tensor_tensor(out=cvt[:, 0:nw], in0=ps[:, 0:nw],
                                            in1=pen[:, 0:nw], op=AL.add)
                    co = border[br] * 6 + ki * 2 + ft
                    nc.vector.tensor_reduce(
                        out=rep[:, co, sample:sample + 1], in_=cvt[:, 0:nw],
                        axis=mybir.AxisListType.X, op=AL.max)


        # prologue: pair 0 of layer 0
        w_cur = load_layer_weights(0)
        embed(0)
        embed(1)
        qk_cur = qkv_pair(0, w_cur)
        v_cur = v_pair(0, w_cur)
        embed(2)
        embed(3)
        att_scores(0, 0, qk_cur)
        for l in range(NL):
            for pr in range(NPAIR):
                w = w_cur
                if pr + 1 < NPAIR:
                    nxt_l, nxt_pr = l, pr + 1
                elif l + 1 < NL:
                    nxt_l, nxt_pr = l + 1, 0
                else:
                    nxt_l = nxt_pr = None

                att_scores(pr, 1, qk_cur)
                ctxbs = [work.tile([128, NT, D], BF16, tag="ctxb",
                                   name=f"cb{l}_{pr}_{si}") for si in range(2)]
                att_ctx(pr, 0, v_cur, ctxbs)
                att_ctx(pr, 1, v_cur, ctxbs)
                # ctx^T via DMA transpose
                cts = [work.tile([128, NT, ND, 128], BF16, tag="cT",
                                 name=f"cT{l}_{pr}_{si}") for si in range(2)]
                for si in range(2):
                    for qt in range(NT):
                        nc.sync.dma_start_transpose(cts[si][:, qt, :, :],
                                                    ctxbs[si][:, qt, :])
                # attention out projection + residual
                y1T = work.tile([128, 2, NT, ND, 128], BF16, tag="y1T",
                                name=f"y1T{pr}")
                rs = []
                for i in range(4):
                    si, tt = i // 2, i % 2
                    seq = 2 * pr + si
                    ps = ps_mm.tile([128, D], F32, tag="mm")
                    for dt in range(ND):
                        nc.tensor.matmul(
                            ps[:], cts[si][:, tt, dt, :],
                            w["wo"][:, dt, :], start=dt == 0, stop=dt == ND - 1)
                    r = work.tile([128, D], F32, tag="rln", name=f"r{i}", bufs=4)
                    nc.vector.tensor_tensor(out=r[:], in0=ps[:],
                                            in1=x_tm[seq][:, tt, :], op=AL.add)
                    if not fl["bo"]:
                        nc.vector.tensor_tensor(out=r[:], in0=r[:],
                                                in1=w["bo"][:], op=AL.add)
                    rs.append(r)
                # PE backfill while the LN1 chain runs: next pair's QKV
                if nxt_pr is not None:
                    w_nxt = w if nxt_l == l else load_layer_weights(nxt_l)
                    if l == 0 and pr + 2 < NPAIR:
                        embed(2 * (pr + 2))
                        embed(2 * (pr + 2) + 1)
                    qk_nxt = qkv_pair(nxt_pr, w_nxt)
                residual_ln(rs, [x_tm[2 * pr + i // 2][:, i % 2, :]
                                 for i in range(4)],
                            None if fl["ln"] else w["ln1s"],
                            None if fl["ln"] else w["ln1b"])
                for i in range(4):
                    si, tt = i // 2, i % 2
                    cast_transpose(y1T, 2 * pr + si, tt,
                                   x_tm[2 * pr + si][:, tt, :])
                # FFN1: hidden feature-major, gelu fused with bias
                hT = big.tile([128, NFT, 2 * S], BF16, tag="hT")
                for ft in range(NFT):
                    ps = ps_mm.tile([128, 2 * S], F32, tag="mm")
                    for dt in range(ND):
                        nc.tensor.matmul(
                            ps[:], w["wi"][:, dt, ft * 128:(ft + 1) * 128],
                            y1T[:, :, :, dt, :], start=dt == 0,
                            stop=dt == ND - 1)
                    nc.scalar.activation(
                        out=hT[:, ft, :], in_=ps[:], func=AF.Gelu,
                        bias=0.0 if fl["bi"] else w["bi"][:, ft:ft + 1],
                        scale=1.0)
                # FFN2 + residual
                rs = []
                for i in range(4):
                    si, tt = i // 2, i % 2
                    seq = 2 * pr + si
                    so = si * S
                    ps = ps_mm.tile([128, D], F32, tag="mm")
                    for ft in range(NFT):
                        nc.tensor.matmul(
                            ps[:], hT[:, ft, so + tt * 128:so + (tt + 1) * 128],
                            w["wo2"][:, ft, :], start=ft == 0, stop=ft == NFT - 1)
                    r = work.tile([128, D], F32, tag="rln", name=f"r2{i}", bufs=4)
                    nc.vector.tensor_tensor(out=r[:], in0=ps[:],
                                            in1=x_tm[seq][:, tt, :], op=AL.add)
                    if not fl["bo2"]:
                        nc.vector.tensor_tensor(out=r[:], in0=r[:],
                                                in1=w["bo2"][:], op=AL.add)
                    rs.append(r)
                # PE backfill while the LN2 chain runs: next pair's V, then
                # its first score group
                if nxt_pr is not None:
                    v_nxt = v_pair(nxt_pr, w_nxt)
                residual_ln(rs, [x_tm[2 * pr + i // 2][:, i % 2, :]
                                 for i in range(4)],
                            None if fl["ln"] else w["ln2s"],
                            None if fl["ln"] else w["ln2b"])
                for i in range(4):
                    si, tt = i // 2, i % 2
                    cast_transpose(xT[pr], 2 * pr + si, tt,
                                   x_tm[2 * pr + si][:, tt, :])
                if nxt_pr is not None:
                    att_scores(nxt_pr, 0, qk_nxt)
                    qk_cur, v_cur, w_cur = qk_nxt, v_nxt, w_nxt
                if l == NL - 1:
                    conv_seq(2 * pr)
                    conv_seq(2 * pr + 1)


        if not fl["cb"]:
            for bo_ in range(3):
                for ki in range(3):
                    for ft in range(2):
                        co = bo_ * 6 + ki * 2 + ft
                        nc.vector.tensor_scalar_add(
                            out=rep[:, co, :], in0=rep[:, co, :],
                            scalar1=cb[:, ki, ft:ft + 1])
        nc.scalar.activation(out=rep[:], in_=rep[:], func=AF.Relu)
        if debug:
            nc.sync.dma_start(out=dbgr_d[:], in_=rep[:])

        fps = ps_tp.tile([128, NCLS], F32, tag="tp")
        for co in range(NCH):
            nc.tensor.matmul(fps[:SPC, :], rep[:, co, :], fcw[:, co, :],
                             start=co == 0, stop=co == NCH - 1)
        ob = small.tile([SPC, NCLS], F32, tag="ob")
        nc.scalar.copy(out=ob[:], in_=fps[:SPC, :])
        if not fl["fcb"]:
            nc.vector.tensor_tensor(out=ob[:], in0=ob[:], in1=fcb[:SPC, :],
                                    op=AL.add)
        nc.sync.dma_start(out=out_d[:], in_=ob[:])

    nc.compile()
    return nc


def _core_inputs(inputs, fl):
    """Build the 8 per-core input maps from the full problem inputs."""
    f32 = lambda a: np.ascontiguousarray(np.asarray(a, dtype=np.float32))
    tile_w = lambda w: np.ascontiguousarray(
        f32(w).reshape(w.shape[0] // 128, 128, w.shape[1]).astype(BF))

    shared = {}
    shared["posty"] = np.ascontiguousarray(
        (f32(inputs["pos_emb"][:S]) + f32(inputs["type_emb"][0])).reshape(
            NT, 128, D))
    for l in range(NL):
        shared[f"wq{l}"] = tile_w(f32(inputs["Wq"][l]) / 8.0)
        shared[f"wk{l}"] = tile_w(inputs["Wk"][l])
        shared[f"wv{l}"] = tile_w(inputs["Wv"][l])
        shared[f"wo{l}"] = tile_w(inputs["Wo"][l])
        shared[f"wi{l}"] = tile_w(inputs["Wi"][l])
        shared[f"wo2{l}"] = tile_w(inputs["Wo2"][l])
        if not fl["bqk"]:
            shared[f"bq{l}"] = f32(inputs["bq"][l]).reshape(ND, 128) / 8.0
            shared[f"bk{l}"] = f32(inputs["bk"][l]).reshape(ND, 128)
        if not fl["bv"]:
            shared[f"bv{l}"] = f32(inputs["bv"][l])
        if not fl["bo"]:
            shared[f"bo{l}"] = f32(inputs["bo"][l])
        if not fl["bi"]:
            shared[f"bi{l}"] = f32(inputs["bi"][l]).reshape(NFT, 128)
        if not fl["bo2"]:
            shared[f"bo2{l}"] = f32(inputs["bo2"][l])
        if not fl["ln"]:
            shared[f"ln1s{l}"] = f32(inputs["ln1_s"][l])
            shared[f"ln1b{l}"] = f32(inputs["ln1_b"][l])
            shared[f"ln2s{l}"] = f32(inputs["ln2_s"][l])
            shared[f"ln2b{l}"] = f32(inputs["ln2_b"][l])
    if not fl["ln"]:
        shared["lnes"] = f32(inputs["emb_ln_s"])
        shared["lneb"] = f32(inputs["emb_ln_b"])
    for ki, k in enumerate((1, 2, 3)):
        w = f32(inputs[f"conv_w{k}"])          # [NF, k, D]
        wt = np.ascontiguousarray(w.transpose(1, 2, 0))  # [k, D, NF]
        shared[f"cw{k}"] = np.ascontiguousarray(
            wt.reshape(k, ND, 128, NF).astype(BF))
    shared["fcw"] = np.ascontiguousarray(
        f32(inputs["fc_w"]).reshape(NCH, 128, NCLS))
    if not fl["cb"]:
        shared["convb"] = np.stack(
            [f32(inputs[f"conv_b{k}"]).reshape(2, 128) for k in (1, 2, 3)])
    if not fl["fcb"]:
        shared["fcb"] = f32(inputs["fc_b"])
    shared["word_emb"] = f32(inputs["word_emb"])

    in_maps = []
    for c in range(NCORES):
        sl = slice(c * SPC, (c + 1) * SPC)
        ids = np.concatenate([np.asarray(inputs[p + "_input_ids"][sl])
                              for p in ("q", "a", "b")]).astype(np.int32)
        masks = np.concatenate([np.asarray(inputs[p + "_attention_mask"][sl])
                                for p in ("q", "a", "b")]).astype(np.int32)
        lens = masks.sum(1)                        # [12]
        m = dict(shared)
        m["ids"] = np.ascontiguousarray(ids)
        m["maskbias"] = np.ascontiguousarray(
            ((masks - 1) * 10000.0).astype(np.float32).reshape(NSEQ, NT, 128))
        cmask = np.zeros((NSEQ, SW), dtype=np.float32)
        cmask[:, :S] = masks
        m["convmask"] = cmask.astype(BF)
        w_idx = np.arange(SW)[None, :]
        pen = np.zeros((NSEQ, 3, SW), dtype=np.float32)
        for ki, k in enumerate((1, 2, 3)):
            valid = (w_idx + k - 1) <= lens[:, None]
            valid[:, SW - k + 1:] = False
            pen[:, ki] = np.where(valid, 0.0, -1e30)
        m["convpen"] = pen
        in_maps.append(m)
    return in_maps


def _get_program(fl, debug=False):
    key = (tuple(sorted(fl.items())), debug)
    if key not in _CACHE:
        _CACHE[key] = _build_program(fl, debug=debug)
    return _CACHE[key]


def run_sharded(inputs, debug=False, **run_kwargs):
    """Shard, run on 8 cores, gather. Returns (output, BassKernelResults)."""
    from concourse.bass_utils import run_bass_kernel_spmd
    fl = _flags(inputs)
    nc = _get_program(fl, debug=debug)
    in_maps = _core_inputs(inputs, fl)
    res = run_bass_kernel_spmd(nc, in_maps, core_ids=list(range(NCORES)),
                               **run_kwargs)
    out = np.concatenate([res.results[c]["out"] for c in range(NCORES)], axis=0)
    return out.astype(np.float32), res


def kernel(**inputs):
    out, _ = run_sharded(inputs)
    return out



# revision 13
# speedup vs baseline: 1.3682x; 1.3682x over previous
"""Trainium2 Bass kernel for nn_BertCNN (3x BERT-small encoder + CNN maxpool head).

Ragged-packed data-parallel strategy. The 96 sequences (3 branches x 32
samples) are sorted by actual length (from the attention mask), dealt
round-robin into 8 cores x 12 slots, and each core packs its 12 sequences
into one ~1900-token stream (slot budgets = max length in each rank group,
32-aligned starts). All encoder linear ops (QKV/O/FFN/conv) run over the
packed stream; attention runs per-slot with exact budget widths; key
validity is folded multiplicatively into V (invalid keys get zero V rows
and a zero softmax-denominator contribution), so no attention bias is
needed. The conv head runs over a separately packed layout with 2-token
gaps; per-slot maxpool ranges and window-validity penalties come from the
host. The fc output is computed for all 3 branch hypotheses per slot
([12, 12] per core) and the host scatter-adds the right 4 columns into the
final [32, 4].

The Bass program depends only on the slot-budget layout (not on per-core
data); it is built once per layout signature and cached.
"""

import numpy as np
import ml_dtypes

V, D, H, DH, NL, FF = 30522, 512, 8, 64, 4, 2048
NF, NCLS, B, S = 256, 4, 32, 256
NCORES = 8
NSL = 12                 # slots (sequences) per core
NSEQ = NSL               # test.py compat
SPC = 4                  # test.py compat
ND = D // 128
NFT = FF // 128
DH1 = DH + 1

BF = ml_dtypes.bfloat16
_CACHE = {}


def _flags(inputs):
    z = lambda a: bool(np.all(np.asarray(a) == 0))
    o = lambda a: bool(np.all(np.asarray(a) == 1))
    return {
        "bqk": z(inputs["bq"]) and z(inputs["bk"]),
        "bv": z(inputs["bv"]),
        "bo": z(inputs["bo"]),
        "bi": z(inputs["bi"]),
        "bo2": z(inputs["bo2"]),
        "ln": all(o(inputs[k]) for k in ("emb_ln_s", "ln1_s", "ln2_s"))
        and all(z(inputs[k]) for k in ("emb_ln_b", "ln1_b", "ln2_b")),
        "cb": z(inputs["conv_b1"]) and z(inputs["conv_b2"]) and z(inputs["conv_b3"]),
    }


def _layout(lens96):
    """Pack layout shared by all cores (program-shaping constants)."""
    lens96 = np.asarray(lens96, dtype=np.int64)
    order = np.argsort(-lens96, kind="stable")
    assign = order.reshape(NSL, NCORES)           # [slot, core] -> seq idx
    lam = lens96[assign].max(1).astype(int)       # slot budgets (max len)
    # 64-aligned slot starts (PE col/row tiling only supports base 0/64 for
    # >32-wide tiles); bump a start to the next 128 boundary when the slot
    # would otherwise span 3 token tiles (score/eT tiles hold 2 chunks)
    lam32 = ((lam + 63) // 64) * 64
    off = np.zeros(NSL + 1, np.int64)
    for j in range(NSL):
        o = off[j]
        if (o % 128) + lam[j] > 256:
            o = ((o + 127) // 128) * 128
            off[j] = o
        off[j + 1] = o + lam32[j]
    T32 = int(off[-1])
    NTT = -(-T32 // 128)
    TP = NTT * 128
    coff = np.zeros(NSL + 1, np.int64)
    coff[1:] = np.cumsum(lam + 2)
    CW = int(coff[-1])

    def greedy(offs, cap):
        groups, cur = [], [0]
        for j in range(1, NSL):
            if offs[j + 1] - offs[cur[0]] <= cap:
                cur.append(j)
            else:
                groups.append(cur)
                cur = [j]
        groups.append(cur)
        return groups

    qk_groups = greedy(off, 512)
    cv_groups = greedy(coff, 512)

    # per-slot key/query chunks: intersections with the global 128 grid
    chunks = []
    for j in range(NSL):
        lo, hi = int(off[j]), int(off[j] + lam[j])
        ch = []
        p = lo
        while p < hi:
            nt = p // 128
            e = min(hi, (nt + 1) * 128)
            ch.append((nt, p - nt * 128, e - p, p - lo))  # (tile, base, width, rel)
            p = e
        chunks.append(ch)
    return dict(
        assign=assign, lam=[int(x) for x in lam], lam32=[int(x) for x in lam32],
        off=[int(x) for x in off], coff=[int(x) for x in coff],
        T32=T32, NTT=NTT, TP=TP, CW=CW, CWP=CW + 2,
        qk_groups=qk_groups, cv_groups=cv_groups, chunks=chunks,
        key=(tuple(int(x) for x in lam), tuple(int(x) for x in lam32)),
    )


def _build_program(fl, lay, debug=False):
    import contextlib
    import concourse.bass as bass
    import concourse.mybir as mybir
    import concourse.tile as tile
    from concourse import bacc

    F32, BF16, I32 = mybir.dt.float32, mybir.dt.bfloat16, mybir.dt.int32
    AL, AF = mybir.AluOpType, mybir.ActivationFunctionType

    NTT, TP, CWP = lay["NTT"], lay["TP"], lay["CWP"]
    lam, lam32, off, coff = lay["lam"], lay["lam32"], lay["off"], lay["coff"]
    chunks = lay["chunks"]

    nc = bacc.Bacc("TRN2", target_bir_lowering=False, debug=False,
                   num_devices=NCORES)

    di = lambda n, s, d: nc.dram_tensor(n, s, d, kind="ExternalInput").ap()
    word = di("word_emb", [V, D], F32)
    ids_d = di("ids", [NTT, 128], I32)
    vm_d = di("vmask", [NTT, 128], F32)
    posty_d = di("posty", [NTT, 128, D], F32)
    cm_d = di("convmask", [CWP], BF16)
    cpen_d = di("convpen", [3, CWP], F32)
    wq_d = [di(f"wq{l}", [ND, 128, D], BF16) for l in range(NL)]
    wk_d = [di(f"wk{l}", [ND, 128, D], BF16) for l in range(NL)]
    wv_d = [di(f"wv{l}", [ND, 128, D], BF16) for l in range(NL)]
    wo_d = [di(f"wo{l}", [ND, 128, D], BF16) for l in range(NL)]
    wi_d = [di(f"wi{l}", [ND, 128, FF], BF16) for l in range(NL)]
    wo2_d = [di(f"wo2{l}", [NFT, 128, D], BF16) for l in range(NL)]
    cw_d = [di(f"cw{k}", [k, ND, 128, NF], BF16) for k in (1, 2, 3)]
    fcw_d = di("fcw", [6, 128, 3 * NCLS], F32)
    if not fl["bqk"]:
        bq_d = [di(f"bq{l}", [ND, 128], F32) for l in range(NL)]
        bk_d = [di(f"bk{l}", [ND, 128], F32) for l in range(NL)]
    if not fl["bv"]:
        bv_d = [di(f"bv{l}", [D], F32) for l in range(NL)]
    if not fl["bo"]:
        bo_d = [di(f"bo{l}", [D], F32) for l in range(NL)]
    if not fl["bi"]:
        bi_d = [di(f"bi{l}", [NFT, 128], F32) for l in range(NL)]
    if not fl["bo2"]:
        bo2_d = [di(f"bo2{l}", [D], F32) for l in range(NL)]
    if not fl["ln"]:
        elns_d = di("lnes", [D], F32)
        elnb_d = di("lneb", [D], F32)
        ln1s_d = [di(f"ln1s{l}", [D], F32) for l in range(NL)]
        ln1b_d = [di(f"ln1b{l}", [D], F32) for l in range(NL)]
        ln2s_d = [di(f"ln2s{l}", [D], F32) for l in range(NL)]
        ln2b_d = [di(f"ln2b{l}", [D], F32) for l in range(NL)]
    if not fl["cb"]:
        cb_d = di("convb", [3, 2, 128], F32)

    out_d = nc.dram_tensor("out", [NSL, 3 * NCLS], F32, kind="ExternalOutput").ap()
    if debug:
        dbgx_d = nc.dram_tensor("dbgx", [NTT, 128, D], F32,
                                kind="ExternalOutput").ap()

    with tile.TileContext(nc) as tc, contextlib.ExitStack() as ctx:
        consts = ctx.enter_context(tc.tile_pool(name="consts", bufs=1))
        state = ctx.enter_context(tc.tile_pool(name="state", bufs=1))
        wts = ctx.enter_context(tc.tile_pool(name="wts", bufs=1))
        qkp = ctx.enter_context(tc.tile_pool(name="qkp", bufs=2))
        etp = ctx.enter_context(tc.tile_pool(name="etp", bufs=2))
        htp = ctx.enter_context(tc.tile_pool(name="htp", bufs=1))
        work = ctx.enter_context(tc.tile_pool(name="work", bufs=2))
        cxp = ctx.enter_context(tc.tile_pool(name="cxp", bufs=4))
        small = ctx.enter_context(tc.tile_pool(name="small", bufs=4))
        ps_mm = ctx.enter_context(tc.tile_pool(name="ps_mm", bufs=4, space="PSUM"))
        ps_s = ctx.enter_context(tc.tile_pool(name="ps_s", bufs=2, space="PSUM"))
        ps_c = ctx.enter_context(tc.tile_pool(name="ps_c", bufs=2, space="PSUM"))

        # ---- constants ----
        eps_t = consts.tile([128, 1], F32, tag="eps")
        nc.vector.memset(eps_t[:], 1e-12)
        ones8 = consts.tile([128, H], BF16, tag="ones8")
        nc.vector.memset(ones8[:], 1.0)
        ids_sb = consts.tile([128, NTT], I32, tag="ids")
        nc.sync.dma_start(out=ids_sb[:], in_=ids_d.rearrange("t p -> p t"))
        vm_sb = consts.tile([128, NTT], F32, tag="vm")
        nc.sync.dma_start(out=vm_sb[:], in_=vm_d.rearrange("t p -> p t"))
        cm_sb = consts.tile([128, CWP], BF16, tag="cm")
        nc.sync.dma_start(out=cm_sb[:],
                          in_=cm_d[None, :].to_broadcast([128, CWP]))
        fcw_sb = consts.tile([128, 6, 3 * NCLS], F32, tag="fcw")
        nc.sync.dma_start(out=fcw_sb[:], in_=fcw_d.rearrange("c p n -> p c n"))
        bcast = lambda ap, n: ap[None, :].to_broadcast([128, n])
        if not fl["ln"]:
            elns = consts.tile([128, D], F32, tag="elns")
            nc.sync.dma_start(out=elns[:], in_=bcast(elns_d, D))
            elnb = consts.tile([128, D], F32, tag="elnb")
            nc.sync.dma_start(out=elnb[:], in_=bcast(elnb_d, D))
        if not fl["cb"]:
            cb = consts.tile([128, 3, 2], F32, tag="cb")
            nc.sync.dma_start(out=cb[:], in_=cb_d.rearrange("k t p -> p k t"))

        cw = {}
        for ki, k in enumerate((1, 2, 3)):
            for j in range(k):
                t = wts.tile([128, ND, NF], BF16, tag=f"cw{ki}_{j}")
                nc.sync.dma_start(out=t[:],
                                  in_=cw_d[ki][j].rearrange("t p f -> p t f"))
                cw[(k, j)] = t

        # ---- persistent state ----
        x_tm = state.tile([128, NTT, D], BF16, tag="x_tm")
        xTa = state.tile([128, ND, TP], BF16, tag="xTa")
        ctxT = state.tile([128, ND, TP], BF16, tag="ctxT")
        vA = state.tile([128, NTT, H, DH1], BF16, tag="vA")
        rep = state.tile([128, 6, NSL], F32, tag="rep")

        def load_layer_weights(l):
            w = {}
            for nm, dd, nfree in (("wq", wq_d, D), ("wk", wk_d, D),
                                  ("wv", wv_d, D), ("wo", wo_d, D),
                                  ("wi", wi_d, FF)):
                w[nm] = wts.tile([128, ND, nfree], BF16, tag=nm, name=f"{nm}_{l}")
                for dt in range(ND):
                    nc.sync.dma_start(out=w[nm][:, dt, :], in_=dd[l][dt])
            w["wo2"] = wts.tile([128, NFT, D], BF16, tag="wo2", name=f"wo2_{l}")
            for ft in range(0, NFT, 4):
                nc.sync.dma_start(
                    out=w["wo2"][:, ft:ft + 4, :],
                    in_=wo2_d[l][ft:ft + 4].rearrange("t p o -> p t o"))
            if not fl["bqk"]:
                w["bq"] = consts.tile([128, ND], F32, tag="bq", name=f"bq_{l}")
                nc.sync.dma_start(out=w["bq"][:], in_=bq_d[l].rearrange("t p -> p t"))
                w["bk"] = consts.tile([128, ND], F32, tag="bk", name=f"bk_{l}")
                nc.sync.dma_start(out=w["bk"][:], in_=bk_d[l].rearrange("t p -> p t"))
            if not fl["bv"]:
                w["bv"] = consts.tile([128, D], F32, tag="bv", name=f"bv_{l}")
                nc.sync.dma_start(out=w["bv"][:], in_=bcast(bv_d[l], D))
            if not fl["bo"]:
                w["bo"] = consts.tile([128, D], F32, tag="bo", name=f"bo_{l}")
                nc.sync.dma_start(out=w["bo"][:], in_=bcast(bo_d[l], D))
            if not fl["bi"]:
                w["bi"] = consts.tile([128, NFT], F32, tag="bi", name=f"bi_{l}")
                nc.sync.dma_start(out=w["bi"][:], in_=bi_d[l].rearrange("t p -> p t"))
            if not fl["bo2"]:
                w["bo2"] = consts.tile([128, D], F32, tag="bo2", name=f"bo2_{l}")
                nc.sync.dma_start(out=w["bo2"][:], in_=bcast(bo2_d[l], D))
            if not fl["ln"]:
                for nm, dd in (("ln1s", ln1s_d), ("ln1b", ln1b_d),
                               ("ln2s", ln2s_d), ("ln2b", ln2b_d)):
                    w[nm] = consts.tile([128, D], F32, tag=nm, name=f"{nm}_{l}")
                    nc.sync.dma_start(out=w[nm][:], in_=bcast(dd[l], D))
            return w

        def ln_start(src_ap, i=0):
            st = small.tile([128, 6], F32, tag="st", name=f"st{i}")
            mv = small.tile([128, 2], F32, tag="mv", name=f"mv{i}")
            nc.vector.bn_stats(out=st[:], in_=src_ap)
            nc.vector.bn_aggr(out=mv[:], in_=st[:])
            sd = small.tile([128, 1], F32, tag="sd", name=f"sd{i}")
            nc.scalar.activation(out=sd[:], in_=mv[:, 1:2], func=AF.Sqrt,
                                 bias=eps_t[:], scale=1.0)
            nmr = small.tile([128, 2], F32, tag="nmr", name=f"nmr{i}")
            nc.vector.reciprocal(out=nmr[:, 1:2], in_=sd[:])
            nc.vector.tensor_scalar(out=nmr[:, 0:1], in0=mv[:, 0:1],
                                    scalar1=nmr[:, 1:2], scalar2=-1.0,
                                    op0=AL.mult, op1=AL.mult)
            return nmr

        def ln_apply(src_ap, dst_ap, nmr, s_tile, b_tile):
            nc.vector.tensor_scalar(out=dst_ap, in0=src_ap,
                                    scalar1=nmr[:, 1:2], scalar2=nmr[:, 0:1],
                                    op0=AL.mult, op1=AL.add)
            if s_tile is not None:
                nc.vector.tensor_tensor(out=dst_ap, in0=dst_ap, in1=s_tile[:],
                                        op=AL.mult)
            if b_tile is not None:
                nc.vector.tensor_tensor(out=dst_ap, in0=dst_ap, in1=b_tile[:],
                                        op=AL.add)

        def embed(nt):
            g = work.tile([128, D], F32, tag="g", name=f"g{nt}")
            nc.gpsimd.indirect_dma_start(
                out=g[:], out_offset=None, in_=word[:],
                in_offset=bass.IndirectOffsetOnAxis(
                    ap=ids_sb[:, nt:nt + 1], axis=0))
            pt = work.tile([128, D], F32, tag="pt", name=f"pt{nt}")
            nc.sync.dma_start(out=pt[:], in_=posty_d[nt])
            nc.vector.tensor_tensor(out=g[:], in0=g[:], in1=pt[:], op=AL.add)
            dst = x_tm[:, nt, :]
            ln_apply(g[:], dst, ln_start(g[:], i=nt % 4),
                     None if fl["ln"] else elns, None if fl["ln"] else elnb)
            nc.sync.dma_start_transpose(xTa[:, :, nt * 128:(nt + 1) * 128], dst)

        def qk_group(gi, grp, w):
            g0, g1 = off[grp[0]], off[grp[-1] + 1]
            W = g1 - g0
            qkT = qkp.tile([128, 2, ND, 512], BF16, tag="qkT", name=f"qkT{gi}")
            for qi, wt, bt in ((0, w["wq"], "bq"), (1, w["wk"], "bk")):
                for ot in range(ND):
                    ps = ps_mm.tile([128, 512], F32, tag="mm")
                    for dt in range(ND):
                        nc.tensor.matmul(
                            ps[:, 0:W], wt[:, dt, ot * 128:(ot + 1) * 128],
                            xTa[:, dt, g0:g1], start=dt == 0, stop=dt == ND - 1)
                    if fl["bqk"]:
                        nc.vector.tensor_copy(out=qkT[:, qi, ot, 0:W],
                                              in_=ps[:, 0:W])
                    else:
                        nc.scalar.activation(
                            out=qkT[:, qi, ot, 0:W], in_=ps[:, 0:W],
                            func=AF.Identity, bias=w[bt][:, ot:ot + 1], scale=1.0)
            return qkT

        def v_tile(nt, w):
            ps = ps_mm.tile([128, 512], F32, tag="mm")
            for dt in range(ND):
                nc.tensor.matmul(ps[:], xTa[:, dt, nt * 128:(nt + 1) * 128],
                                 w["wv"][:, dt, :], start=dt == 0,
                                 stop=dt == ND - 1)
            if fl["bv"]:
                nc.vector.tensor_scalar_mul(
                    out=vA[:, nt, :, 0:DH],
                    in0=ps.rearrange("p (h d) -> p h d", h=H),
                    scalar1=vm_sb[:, nt:nt + 1])
            else:
                nc.vector.tensor_tensor(
                    out=vA[:, nt, :, 0:DH],
                    in0=ps.rearrange("p (h d) -> p h d", h=H),
                    in1=w["bv"].rearrange("p (h d) -> p h d", h=H), op=AL.add)
                nc.vector.tensor_scalar_mul(
                    out=vA[:, nt, :, 0:DH], in0=vA[:, nt, :, 0:DH],
                    scalar1=vm_sb[:, nt:nt + 1])
            nc.vector.tensor_scalar_mul(
                out=vA[:, nt, :, DH], in0=ones8[:],
                scalar1=vm_sb[:, nt:nt + 1])

        def attn_slot(l, j, qkT, g0, ctxb_map, last_slot_of):
            L = lam[j]
            ch = chunks[j]
            q0 = off[j] - g0
            eT = etp.tile([128, H, 512], BF16, tag="eT", name=f"eT{l}_{j}")
            for h in range(H):
                hh, dtH = (h % 2) * DH, h // 2
                pss = ps_s.tile([128, 512], F32, tag="s")
                for ci, (nt, b, kw, rel) in enumerate(ch):
                    nc.tensor.matmul(
                        pss[b:b + kw, ci * L:ci * L + L],
                        qkT[hh:hh + DH, 1, dtH, q0 + rel:q0 + rel + kw],
                        qkT[hh:hh + DH, 0, dtH, q0:q0 + L],
                        start=True, stop=True)
                nc.scalar.activation(out=eT[:, h, 0:len(ch) * L],
                                     in_=pss[:, 0:len(ch) * L], func=AF.Exp,
                                     bias=0.0, scale=1.0)
            # ctx per query chunk
            for (qnt, qb, qw, qrel) in ch:
                for hg in range(2):
                    cps = ps_c.tile([128, 4 * DH1], F32, tag="c")
                    for hi in range(4):
                        h = hg * 4 + hi
                        sl = slice(hi * DH1, hi * DH1 + DH1)
                        for ci, (nt, b, kw, rel) in enumerate(ch):
                            nc.tensor.matmul(
                                cps[qb:qb + qw, sl],
                                eT[b:b + kw, h, ci * L + qrel:ci * L + qrel + qw],
                                vA[b:b + kw, nt, h, :],
                                start=ci == 0, stop=ci == len(ch) - 1)
                    if qnt not in ctxb_map:
                        ctxb_map[qnt] = cxp.tile([128, D], BF16, tag="ctxb",
                                                 name=f"cb{l}_{qnt}")
                    ctxb = ctxb_map[qnt]
                    rcp = small.tile([128, 4], F32, tag="rcp")
                    nc.vector.reciprocal(
                        out=rcp[qb:qb + qw, :],
                        in_=cps.rearrange("p (h c) -> p h c", c=DH1)[qb:qb + qw, :, DH])
                    for hi in range(4):
                        h = hg * 4 + hi
                        nc.vector.tensor_scalar_mul(
                            out=ctxb[qb:qb + qw, h * DH:(h + 1) * DH],
                            in0=cps[qb:qb + qw, hi * DH1:hi * DH1 + DH],
                            scalar1=rcp[qb:qb + qw, hi:hi + 1])
            # flush finished ctxb tiles
            for (qnt, qb, qw, qrel) in ch:
                if last_slot_of.get(qnt) == j:
                    nc.sync.dma_start_transpose(
                        ctxT[:, :, qnt * 128:(qnt + 1) * 128], ctxb_map[qnt][:])

        def wo_ln1(nt, w):
            ps = ps_mm.tile([128, 512], F32, tag="mm")
            for dt in range(ND):
                nc.tensor.matmul(ps[:], ctxT[:, dt, nt * 128:(nt + 1) * 128],
                                 w["wo"][:, dt, :], start=dt == 0,
                                 stop=dt == ND - 1)
            r = work.tile([128, D], F32, tag="rln", name=f"r1_{nt}", bufs=4)
            nc.vector.tensor_tensor(out=r[:], in0=ps[:], in1=x_tm[:, nt, :],
                                    op=AL.add)
            if not fl["bo"]:
                nc.vector.tensor_tensor(out=r[:], in0=r[:], in1=w["bo"][:],
                                        op=AL.add)
            ln_apply(r[:], x_tm[:, nt, :], ln_start(r[:], i=nt % 4),
                     None if fl["ln"] else w["ln1s"],
                     None if fl["ln"] else w["ln1b"])
            nc.sync.dma_start_transpose(xTa[:, :, nt * 128:(nt + 1) * 128],
                                        x_tm[:, nt, :])

        def ffn_chunk(l, c0, c1, w):
            Wc = c1 - c0
            hT = htp.tile([128, NFT, 512], BF16, tag="hT", name=f"hT{l}_{c0}")
            for ft in range(NFT):
                ps = ps_mm.tile([128, 512], F32, tag="mm")
                for dt in range(ND):
                    nc.tensor.matmul(
                        ps[:, 0:Wc], w["wi"][:, dt, ft * 128:(ft + 1) * 128],
                        xTa[:, dt, c0:c1], start=dt == 0, stop=dt == ND - 1)
                nc.scalar.activation(
                    out=hT[:, ft, 0:Wc], in_=ps[:, 0:Wc], func=AF.Gelu,
                    bias=0.0 if fl["bi"] else w["bi"][:, ft:ft + 1], scale=1.0)
            for nt in range(c0 // 128, c1 // 128):
                toff = nt * 128 - c0
                ps = ps_mm.tile([128, 512], F32, tag="mm")
                for ft in range(NFT):
                    nc.tensor.matmul(ps[:], hT[:, ft, toff:toff + 128],
                                     w["wo2"][:, ft, :], start=ft == 0,
                                     stop=ft == NFT - 1)
                r = work.tile([128, D], F32, tag="rln", name=f"r2_{l}_{nt}", bufs=4)
                nc.vector.tensor_tensor(out=r[:], in0=ps[:], in1=x_tm[:, nt, :],
                                        op=AL.add)
                if not fl["bo2"]:
                    nc.vector.tensor_tensor(out=r[:], in0=r[:], in1=w["bo2"][:],
                                            op=AL.add)
                ln_apply(r[:], x_tm[:, nt, :], ln_start(r[:], i=nt % 4),
                         None if fl["ln"] else w["ln2s"],
                         None if fl["ln"] else w["ln2b"])
                nc.sync.dma_start_transpose(xTa[:, :, nt * 128:(nt + 1) * 128],
                                            x_tm[:, nt, :])

        # last slot writing each ctx tile (for flush scheduling)
        last_slot_of = {}
        for j in range(NSL):
            for (nt, b, kw, rel) in chunks[j]:
                last_slot_of[nt] = j

        NCH = TP // 512 + (1 if TP % 512 else 0)
        chunk_rng = [(ci * 512, min((ci + 1) * 512, TP)) for ci in range(NCH)]

        # tiles first touched by each qk group (for embed/V scheduling)
        emb_done = set()

        def new_tiles(grp):
            g0, g1 = off[grp[0]], off[grp[-1] + 1]
            ts = [t for t in range(g0 // 128, -(-g1 // 128)) if t not in emb_done]
            emb_done.update(ts)
            return ts

        nc.vector.memset(ctxT[:], 0.0)

        # ---- program ----
        w_cur = load_layer_weights(0)
        for l in range(NL):
            ctxb_map = {}
            v_done = set()
            wo_done = set()
            ffn_done = set()

            def flush(j):
                # Wo + LN1 for tiles whose attention is complete, then any
                # FFN chunk whose 4 tiles are all LN1'd — keeps dense PE
                # work interleaved with the ACT-paced softmax chain.
                for nt in range(NTT):
                    if nt in wo_done or last_slot_of.get(nt, -1) > j:
                        continue
                    if nt not in ctxb_map and nt in last_slot_of:
                        continue  # not yet computed this pass
                    wo_done.add(nt)
                    wo_ln1(nt, w_cur)
                for ci, (c0, c1) in enumerate(chunk_rng):
                    if ci in ffn_done:
                        continue
                    if all(t in wo_done for t in range(c0 // 128, c1 // 128)):
                        ffn_done.add(ci)
                        ffn_chunk(l, c0, c1, w_cur)

            for gi, grp in enumerate(lay["qk_groups"]):
                if l == 0:
                    for nt in new_tiles(grp):
                        embed(nt)
                    if gi == len(lay["qk_groups"]) - 1:
                        for nt in range(TP // 128):
                            if nt not in emb_done:
                                emb_done.add(nt)
                                embed(nt)
                qkT = qk_group(gi, grp, w_cur)
                for j in grp:
                    for (nt, b, kw, rel) in chunks[j]:
                        if nt not in v_done:
                            v_done.add(nt)
                            v_tile(nt, w_cur)
                    attn_slot(l, j, qkT, off[grp[0]], ctxb_map, last_slot_of)
                    if j >= 1:
                        flush(j - 1)
            if l + 1 < NL:
                w_nxt = load_layer_weights(l + 1)
            flush(NSL)
            if l + 1 < NL:
                w_cur = w_nxt

        if debug:
            for nt in range(NTT):
                dx = work.tile([128, D], F32, tag="dbg", name=f"dbg{nt}")
                nc.vector.tensor_copy(out=dx[:], in_=x_tm[:, nt, :])
                nc.sync.dma_start(out=dbgx_d[nt], in_=dx[:])

        # ---- conv head ----
        # xcv reuses ctxT's slot (attention is done), pen/cvt reuse the
        # embed-phase work slots — keeps peak SBUF under the cap
        xcv = state.tile([128, ND, CWP], BF16, tag="ctxT", name="xcv")
        nc.vector.memset(xcv[:], 0.0)
        for j in range(NSL):
            o0, c0 = off[j], coff[j]
            for dt in range(ND):
                nc.vector.tensor_tensor(
                    out=xcv[:, dt, c0:c0 + lam[j]],
                    in0=xTa[:, dt, o0:o0 + lam[j]],
                    in1=cm_sb[:, c0:c0 + lam[j]], op=AL.mult)
        for cgi, cg in enumerate(lay["cv_groups"]):
            cs, ce = coff[cg[0]], coff[cg[-1] + 1]
            Wg = ce - cs
            for ki, k in enumerate((1, 2, 3)):
                pen = work.tile([128, 512], F32, tag="g", name=f"pn{cgi}_{ki}")
                nc.sync.dma_start(
                    out=pen[:, 0:Wg],
                    in_=cpen_d[ki, cs:ce][None, :].to_broadcast([128, Wg]))
                for ft in range(2):
                    ps = ps_mm.tile([128, 512], F32, tag="mm")
                    idx = 0
                    for dt in range(ND):
                        for jj in range(k):
                            nc.tensor.matmul(
                                ps[:, 0:Wg],
                                cw[(k, jj)][:, dt, ft * 128:(ft + 1) * 128],
                                xcv[:, dt, cs + jj:cs + jj + Wg],
                                start=idx == 0, stop=idx == ND * k - 1)
                            idx += 1
                    cvt = work.tile([128, 512], F32, tag="pt",
                                    name=f"cv{cgi}_{ki}_{ft}")
                    nc.vector.tensor_tensor(out=cvt[:, 0:Wg], in0=ps[:, 0:Wg],
                                            in1=pen[:, 0:Wg], op=AL.add)
                    for j in cg:
                        rs = coff[j] - cs
                        re = rs + lam[j] - k + 2
                        nc.vector.tensor_reduce(
                            out=rep[:, ki * 2 + ft, j:j + 1], in_=cvt[:, rs:re],
                            axis=mybir.AxisListType.X, op=AL.max)

        if not fl["cb"]:
            for ki in range(3):
                for ft in range(2):
                    nc.vector.tensor_scalar_add(
                        out=rep[:, ki * 2 + ft, :], in0=rep[:, ki * 2 + ft, :],
                        scalar1=cb[:, ki, ft:ft + 1])
        nc.scalar.activation(out=rep[:], in_=rep[:], func=AF.Relu)

        fps = ps_c.tile([128, 3 * NCLS], F32, tag="c")
        for c in range(6):
            nc.tensor.matmul(fps[:NSL, :], rep[:, c, :], fcw_sb[:, c, :],
                             start=c == 0, stop=c == 5)
        ob = small.tile([NSL, 3 * NCLS], F32, tag="ob")
        nc.scalar.copy(out=ob[:], in_=fps[:NSL, :])
        nc.sync.dma_start(out=out_d[:], in_=ob[:])

    nc.compile()
    return nc


def _core_inputs(inputs, fl, lay):
    """Build the 8 per-core input maps from the full problem inputs."""
    f32 = lambda a: np.ascontiguousarray(np.asarray(a, dtype=np.float32))
    tile_w = lambda w: np.ascontiguousarray(
        f32(w).reshape(w.shape[0] // 128, 128, w.shape[1]).astype(BF))

    NTT, TP, CWP = lay["NTT"], lay["TP"], lay["CWP"]
    lam, lam32, off, coff = lay["lam"], lay["lam32"], lay["off"], lay["coff"]
    assign = lay["assign"]

    shared = {}
    # packed position+type embedding
    posv = np.zeros((TP, D), np.float32)
    pe = f32(inputs["pos_emb"])
    for j in range(NSL):
        posv[off[j]:off[j] + lam32[j]] = pe[:lam32[j]]
    posv += f32(inputs["type_emb"][0])[None, :]
    shared["posty"] = np.ascontiguousarray(posv.reshape(NTT, 128, D))
    for l in range(NL):
        shared[f"wq{l}"] = tile_w(f32(inputs["Wq"][l]) / 8.0)
        shared[f"wk{l}"] = tile_w(inputs["Wk"][l])
        shared[f"wv{l}"] = tile_w(inputs["Wv"][l])
        shared[f"wo{l}"] = tile_w(inputs["Wo"][l])
        shared[f"wi{l}"] = tile_w(inputs["Wi"][l])
        shared[f"wo2{l}"] = tile_w(inputs["Wo2"][l])
        if not fl["bqk"]:
            shared[f"bq{l}"] = f32(inputs["bq"][l]).reshape(ND, 128) / 8.0
            shared[f"bk{l}"] = f32(inputs["bk"][l]).reshape(ND, 128)
        if not fl["bv"]:
            shared[f"bv{l}"] = f32(inputs["bv"][l])
        if not fl["bo"]:
            shared[f"bo{l}"] = f32(inputs["bo"][l])
        if not fl["bi"]:
            shared[f"bi{l}"] = f32(inputs["bi"][l]).reshape(NFT, 128)
        if not fl["bo2"]:
            shared[f"bo2{l}"] = f32(inputs["bo2"][l])
        if not fl["ln"]:
            shared[f"ln1s{l}"] = f32(inputs["ln1_s"][l])
            shared[f"ln1b{l}"] = f32(inputs["ln1_b"][l])
            shared[f"ln2s{l}"] = f32(inputs["ln2_s"][l])
            shared[f"ln2b{l}"] = f32(inputs["ln2_b"][l])
    if not fl["ln"]:
        shared["lnes"] = f32(inputs["emb_ln_s"])
        shared["lneb"] = f32(inputs["emb_ln_b"])
    for ki, k in enumerate((1, 2, 3)):
        w = f32(inputs[f"conv_w{k}"])                    # [NF, k, D]
        wt = np.ascontiguousarray(w.transpose(1, 2, 0))  # [k, D, NF]
        shared[f"cw{k}"] = np.ascontiguousarray(
            wt.reshape(k, ND, 128, NF).astype(BF))
    if not fl["cb"]:
        shared["convb"] = np.stack(
            [f32(inputs[f"conv_b{k}"]).reshape(2, 128) for k in (1, 2, 3)])
    # fc weights for all 3 branch-block hypotheses: [6, 128, 3*NCLS]
    fcw = f32(inputs["fc_w"])                            # [2304, NCLS]
    fcw3 = np.zeros((6, 128, 3 * NCLS), np.float32)
    for bb in range(3):
        for ki in range(3):
            for ft in range(2):
                c = ki * 2 + ft
                rows = 768 * bb + 256 * ki + 128 * ft
                fcw3[c, :, bb * NCLS:(bb + 1) * NCLS] = fcw[rows:rows + 128]
    shared["fcw"] = fcw3
    shared["word_emb"] = f32(inputs["word_emb"])

    ids_all = np.stack([np.asarray(inputs[p + "_input_ids"])
                        for p in ("q", "a", "b")]).reshape(96, S)
    mask_all = np.stack([np.asarray(inputs[p + "_attention_mask"])
                         for p in ("q", "a", "b")]).reshape(96, S)
    lens_all = mask_all.sum(1).astype(int)

    in_maps = []
    for c in range(NCORES):
        m = dict(shared)
        idv = np.zeros(TP, np.int32)
        vmv = np.zeros(TP, np.float32)
        cmv = np.zeros(CWP, np.float32)
        pen = np.full((3, CWP), -1e30, np.float32)
        for j in range(NSL):
            sq = int(assign[j, c])
            l = int(lens_all[sq])
            idv[off[j]:off[j] + lam32[j]] = ids_all[sq][:lam32[j]]
            vmv[off[j]:off[j] + l] = 1.0
            cmv[coff[j]:coff[j] + l] = 1.0
            for ki, k in enumerate((1, 2, 3)):
                nw = l - k + 2
                pen[ki, coff[j]:coff[j] + nw] = 0.0
        m["ids"] = np.ascontiguousarray(idv.reshape(NTT, 128))
        m["vmask"] = np.ascontiguousarray(vmv.reshape(NTT, 128))
        m["convmask"] = np.ascontiguousarray(cmv.astype(BF))
        m["convpen"] = np.ascontiguousarray(pen)
        in_maps.append(m)
    return in_maps


def _get_program(fl, lay, debug=False):
    key = (tuple(sorted(fl.items())), lay["key"], debug)
    if key not in _CACHE:
        _CACHE[key] = _build_program(fl, lay, debug=debug)
    return _CACHE[key]


def run_sharded(inputs, debug=False, **run_kwargs):
    """Shard, run on 8 cores, gather. Returns (output, BassKernelResults)."""
    from concourse.bass_utils import run_bass_kernel_spmd
    fl = _flags(inputs)
    lens96 = np.concatenate([
        np.asarray(inputs[p + "_attention_mask"]).sum(1) for p in ("q", "a", "b")])
    lay = _layout(lens96)
    nc = _get_program(fl, lay, debug=debug)
    in_maps = _core_inputs(inputs, fl, lay)
    res = run_bass_kernel_spmd(nc, in_maps, core_ids=list(range(NCORES)),
                               **run_kwargs)
    border = {0: 0, 1: 2, 2: 1}   # branch q/a/b -> fc block q,b,a
    out = np.zeros((B, NCLS), np.float32)
    for c in range(NCORES):
        o3 = np.asarray(res.results[c]["out"], np.float32)   # [NSL, 12]
        for j in range(NSL):
            sq = int(lay["assign"][j, c])
            br, sample = sq // B, sq % B
            out[sample] += o3[j, border[br] * NCLS:(border[br] + 1) * NCLS]
    out += np.asarray(inputs["fc_b"], np.float32)[None, :]
    return out, res


def kernel(**inputs):
    out, _ = run_sharded(inputs)
    return out


# revision 22
# speedup vs baseline: 1.6270x; 1.1891x over previous
"""Trainium2 Bass kernel for nn_BertCNN (3x BERT-small encoder + CNN maxpool head).

Ragged-packed data-parallel strategy. The 96 sequences (3 branches x 32
samples) are sorted by actual length (from the attention mask), dealt
round-robin into 8 cores x 12 slots, and each core packs its 12 sequences
into one ~1900-token stream (slot budgets = max length in each rank group,
32-aligned starts). All encoder linear ops (QKV/O/FFN/conv) run over the
packed stream; attention runs per-slot with exact budget widths; key
validity is folded multiplicatively into V (invalid keys get zero V rows
and a zero softmax-denominator contribution), so no attention bias is
needed. The conv head runs over a separately packed layout with 2-token
gaps; per-slot maxpool ranges and window-validity penalties come from the
host. The fc output is computed for all 3 branch hypotheses per slot
([12, 12] per core) and the host scatter-adds the right 4 columns into the
final [32, 4].

The Bass program depends only on the slot-budget layout (not on per-core
data); it is built once per layout signature and cached.
"""

import numpy as np
import ml_dtypes

V, D, H, DH, NL, FF = 30522, 512, 8, 64, 4, 2048
NF, NCLS, B, S = 256, 4, 32, 256
NCORES = 8
NSL = 12                 # slots (sequences) per core
NSEQ = NSL               # test.py compat
SPC = 4                  # test.py compat
ND = D // 128
NFT = FF // 128
DH1 = DH + 1

BF = ml_dtypes.bfloat16
_CACHE = {}


def _flags(inputs):
    z = lambda a: bool(np.all(np.asarray(a) == 0))
    o = lambda a: bool(np.all(np.asarray(a) == 1))
    return {
        "bqk": z(inputs["bq"]) and z(inputs["bk"]),
        "bv": z(inputs["bv"]),
        "bo": z(inputs["bo"]),
        "bi": z(inputs["bi"]),
        "bo2": z(inputs["bo2"]),
        "ln": all(o(inputs[k]) for k in ("emb_ln_s", "ln1_s", "ln2_s"))
        and all(z(inputs[k]) for k in ("emb_ln_b", "ln1_b", "ln2_b")),
        "cb": z(inputs["conv_b1"]) and z(inputs["conv_b2"]) and z(inputs["conv_b3"]),
    }


def _layout(lens96):
    """Pack layout shared by all cores (program-shaping constants)."""
    lens96 = np.asarray(lens96, dtype=np.int64)
    order = np.argsort(-lens96, kind="stable")
    assign = order.reshape(NSL, NCORES)           # [slot, core] -> seq idx
    lam = lens96[assign].max(1).astype(int)       # slot budgets (max len)
    # 64-aligned slot starts (PE col/row tiling only supports base 0/64 for
    # >32-wide tiles); bump a start to the next 128 boundary when the slot
    # would otherwise span 3 token tiles (score/eT tiles hold 2 chunks)
    lam32 = ((lam + 63) // 64) * 64
    off = np.zeros(NSL + 1, np.int64)
    for j in range(NSL):
        o = off[j]
        if (o % 128) + lam[j] > 256:
            o = ((o + 127) // 128) * 128
            off[j] = o
        off[j + 1] = o + lam32[j]
    T32 = int(off[-1])
    NTT = -(-T32 // 128)
    TP = NTT * 128
    coff = np.zeros(NSL + 1, np.int64)
    coff[1:] = np.cumsum(lam + 2)
    CW = int(coff[-1])

    def greedy(offs, cap):
        groups, cur = [], [0]
        for j in range(1, NSL):
            if offs[j + 1] - offs[cur[0]] <= cap:
                cur.append(j)
            else:
                groups.append(cur)
                cur = [j]
        groups.append(cur)
        return groups

    qk_groups = greedy(off, 512)
    cv_groups = greedy(coff, 512)

    # per-slot key/query chunks: intersections with the global 128 grid
    chunks = []
    for j in range(NSL):
        lo, hi = int(off[j]), int(off[j] + lam[j])
        ch = []
        p = lo
        while p < hi:
            nt = p // 128
            e = min(hi, (nt + 1) * 128)
            ch.append((nt, p - nt * 128, e - p, p - lo))  # (tile, base, width, rel)
            p = e
        chunks.append(ch)
    return dict(
        assign=assign, lam=[int(x) for x in lam], lam32=[int(x) for x in lam32],
        off=[int(x) for x in off], coff=[int(x) for x in coff],
        T32=T32, NTT=NTT, TP=TP, CW=CW, CWP=CW + 2,
        qk_groups=qk_groups, cv_groups=cv_groups, chunks=chunks,
        key=(tuple(int(x) for x in lam), tuple(int(x) for x in lam32)),
    )


def _build_program(fl, lay, debug=False):
    import contextlib
    import concourse.bass as bass
    import concourse.mybir as mybir
    import concourse.tile as tile
    from concourse import bacc

    F32, BF16, I32 = mybir.dt.float32, mybir.dt.bfloat16, mybir.dt.int32
    AL, AF = mybir.AluOpType, mybir.ActivationFunctionType

    NTT, TP, CWP = lay["NTT"], lay["TP"], lay["CWP"]
    lam, lam32, off, coff = lay["lam"], lay["lam32"], lay["off"], lay["coff"]
    chunks = lay["chunks"]

    nc = bacc.Bacc("TRN2", target_bir_lowering=False, debug=False,
                   num_devices=NCORES)

    di = lambda n, s, d: nc.dram_tensor(n, s, d, kind="ExternalInput").ap()
    word = di("word_emb", [V, D], F32)
    ids_d = di("ids", [NTT, 128], I32)
    vm_d = di("vmask", [NTT, 128], F32)
    posty_d = di("posty", [NTT, 128, D], F32)
    cm_d = di("convmask", [CWP], BF16)
    cpen_d = di("convpen", [3, CWP], F32)
    wq_d = [di(f"wq{l}", [ND, 128, D], BF16) for l in range(NL)]
    wk_d = [di(f"wk{l}", [ND, 128, D], BF16) for l in range(NL)]
    wv_d = [di(f"wv{l}", [ND, 128, D], BF16) for l in range(NL)]
    wo_d = [di(f"wo{l}", [ND, 128, D], BF16) for l in range(NL)]
    wi_d = [di(f"wi{l}", [ND, 128, FF], BF16) for l in range(NL)]
    wo2_d = [di(f"wo2{l}", [NFT, 128, D], BF16) for l in range(NL)]
    cw_d = [di(f"cw{k}", [k, ND, 128, NF], BF16) for k in (1, 2, 3)]
    fcw_d = di("fcw", [6, 128, 3 * NCLS], F32)
    if not fl["bqk"]:
        bq_d = [di(f"bq{l}", [ND, 128], F32) for l in range(NL)]
        bk_d = [di(f"bk{l}", [ND, 128], F32) for l in range(NL)]
    if not fl["bv"]:
        bv_d = [di(f"bv{l}", [D], F32) for l in range(NL)]
    if not fl["bo"]:
        bo_d = [di(f"bo{l}", [D], F32) for l in range(NL)]
    if not fl["bi"]:
        bi_d = [di(f"bi{l}", [NFT, 128], F32) for l in range(NL)]
    if not fl["bo2"]:
        bo2_d = [di(f"bo2{l}", [D], F32) for l in range(NL)]
    if not fl["ln"]:
        elns_d = di("lnes", [D], F32)
        elnb_d = di("lneb", [D], F32)
        ln1s_d = [di(f"ln1s{l}", [D], F32) for l in range(NL)]
        ln1b_d = [di(f"ln1b{l}", [D], F32) for l in range(NL)]
        ln2s_d = [di(f"ln2s{l}", [D], F32) for l in range(NL)]
        ln2b_d = [di(f"ln2b{l}", [D], F32) for l in range(NL)]
    if not fl["cb"]:
        cb_d = di("convb", [3, 2, 128], F32)

    out_d = nc.dram_tensor("out", [NSL, 3 * NCLS], F32, kind="ExternalOutput").ap()
    if debug:
        dbgx_d = nc.dram_tensor("dbgx", [NTT, 128, D], F32,
                                kind="ExternalOutput").ap()

    with tile.TileContext(nc) as tc, contextlib.ExitStack() as ctx:
        consts = ctx.enter_context(tc.tile_pool(name="consts", bufs=1))
        state = ctx.enter_context(tc.tile_pool(name="state", bufs=1))
        wts = ctx.enter_context(tc.tile_pool(name="wts", bufs=1))
        qkp = ctx.enter_context(tc.tile_pool(name="qkp", bufs=2))
        etp = ctx.enter_context(tc.tile_pool(name="etp", bufs=2))
        htp = ctx.enter_context(tc.tile_pool(name="htp", bufs=1))
        work = ctx.enter_context(tc.tile_pool(name="work", bufs=2))
        cxp = ctx.enter_context(tc.tile_pool(name="cxp", bufs=3))
        small = ctx.enter_context(tc.tile_pool(name="small", bufs=4))
        ps_mm = ctx.enter_context(tc.tile_pool(name="ps_mm", bufs=3, space="PSUM"))
        ps_s = ctx.enter_context(tc.tile_pool(name="ps_s", bufs=3, space="PSUM"))
        ps_c = ctx.enter_context(tc.tile_pool(name="ps_c", bufs=2, space="PSUM"))

        # ---- constants ----
        eps_t = consts.tile([128, 1], F32, tag="eps")
        nc.vector.memset(eps_t[:], 1e-12)
        ones8 = consts.tile([128, H], BF16, tag="ones8")
        nc.vector.memset(ones8[:], 1.0)
        ids_sb = consts.tile([128, NTT], I32, tag="ids")
        nc.sync.dma_start(out=ids_sb[:], in_=ids_d.rearrange("t p -> p t"))
        vm_sb = consts.tile([128, NTT], F32, tag="vm")
        nc.sync.dma_start(out=vm_sb[:], in_=vm_d.rearrange("t p -> p t"))
        cm_sb = consts.tile([128, CWP], BF16, tag="cm")
        nc.sync.dma_start(out=cm_sb[:],
                          in_=cm_d[None, :].to_broadcast([128, CWP]))
        fcw_sb = consts.tile([128, 6, 3 * NCLS], F32, tag="fcw")
        nc.sync.dma_start(out=fcw_sb[:], in_=fcw_d.rearrange("c p n -> p c n"))
        bcast = lambda ap, n: ap[None, :].to_broadcast([128, n])
        if not fl["ln"]:
            elns = consts.tile([128, D], F32, tag="elns")
            nc.sync.dma_start(out=elns[:], in_=bcast(elns_d, D))
            elnb = consts.tile([128, D], F32, tag="elnb")
            nc.sync.dma_start(out=elnb[:], in_=bcast(elnb_d, D))
        if not fl["cb"]:
            cb = consts.tile([128, 3, 2], F32, tag="cb")
            nc.sync.dma_start(out=cb[:], in_=cb_d.rearrange("k t p -> p k t"))

        cw = {}
        for ki, k in enumerate((1, 2, 3)):
            for j in range(k):
                t = wts.tile([128, ND, NF], BF16, tag=f"cw{ki}_{j}")
                nc.sync.dma_start(out=t[:],
                                  in_=cw_d[ki][j].rearrange("t p f -> p t f"))
                cw[(k, j)] = t

        # ---- persistent state ----
        x_tm = state.tile([128, NTT, D], BF16, tag="x_tm")
        xTa = state.tile([128, ND, TP], BF16, tag="xTa")
        ctxT = state.tile([128, ND, TP], BF16, tag="ctxT")
        vA = state.tile([128, NTT, H, DH1], BF16, tag="vA")
        rep = state.tile([128, 6, NSL], F32, tag="rep")

        def load_layer_weights(l, w=None, part="all"):
            if w is None:
                w = {}
            names = {"qk": ("wq", "wk"), "rest": ("wv", "wo", "wi"),
                     "all": ("wq", "wk", "wv", "wo", "wi")}[part]
            for nm, dd, nfree in (("wq", wq_d, D), ("wk", wk_d, D),
                                  ("wv", wv_d, D), ("wo", wo_d, D),
                                  ("wi", wi_d, FF)):
                if nm not in names:
                    continue
                w[nm] = wts.tile([128, ND, nfree], BF16, tag=nm, name=f"{nm}_{l}",
                                 bufs=2 if nm in ("wq", "wk") else 1)
                for dt in range(ND):
                    nc.sync.dma_start(out=w[nm][:, dt, :], in_=dd[l][dt])
            if part == "qk":
                return w
            w["wo2"] = wts.tile([128, NFT, D], BF16, tag="wo2", name=f"wo2_{l}")
            for ft in range(0, NFT, 4):
                nc.sync.dma_start(
                    out=w["wo2"][:, ft:ft + 4, :],
                    in_=wo2_d[l][ft:ft + 4].rearrange("t p o -> p t o"))
            if not fl["bqk"]:
                w["bq"] = consts.tile([128, ND], F32, tag="bq", name=f"bq_{l}")
                nc.sync.dma_start(out=w["bq"][:], in_=bq_d[l].rearrange("t p -> p t"))
                w["bk"] = consts.tile([128, ND], F32, tag="bk", name=f"bk_{l}")
                nc.sync.dma_start(out=w["bk"][:], in_=bk_d[l].rearrange("t p -> p t"))
            if not fl["bv"]:
                w["bv"] = consts.tile([128, D], F32, tag="bv", name=f"bv_{l}")
                nc.sync.dma_start(out=w["bv"][:], in_=bcast(bv_d[l], D))
            if not fl["bo"]:
                w["bo"] = consts.tile([128, D], F32, tag="bo", name=f"bo_{l}")
                nc.sync.dma_start(out=w["bo"][:], in_=bcast(bo_d[l], D))
            if not fl["bi"]:
                w["bi"] = consts.tile([128, NFT], F32, tag="bi", name=f"bi_{l}")
                nc.sync.dma_start(out=w["bi"][:], in_=bi_d[l].rearrange("t p -> p t"))
            if not fl["bo2"]:
                w["bo2"] = consts.tile([128, D], F32, tag="bo2", name=f"bo2_{l}")
                nc.sync.dma_start(out=w["bo2"][:], in_=bcast(bo2_d[l], D))
            if not fl["ln"]:
                for nm, dd in (("ln1s", ln1s_d), ("ln1b", ln1b_d),
                               ("ln2s", ln2s_d), ("ln2b", ln2b_d)):
                    w[nm] = consts.tile([128, D], F32, tag=nm, name=f"{nm}_{l}")
                    nc.sync.dma_start(out=w[nm][:], in_=bcast(dd[l], D))
            return w

        def ln_start(src_ap, i=0):
            st = small.tile([128, 6], F32, tag="st", name=f"st{i}")
            mv = small.tile([128, 2], F32, tag="mv", name=f"mv{i}")
            nc.vector.bn_stats(out=st[:], in_=src_ap)
            nc.vector.bn_aggr(out=mv[:], in_=st[:])
            sd = small.tile([128, 1], F32, tag="sd", name=f"sd{i}")
            nc.scalar.activation(out=sd[:], in_=mv[:, 1:2], func=AF.Sqrt,
                                 bias=eps_t[:], scale=1.0)
            nmr = small.tile([128, 2], F32, tag="nmr", name=f"nmr{i}")
            nc.vector.reciprocal(out=nmr[:, 1:2], in_=sd[:])
            nc.vector.tensor_scalar(out=nmr[:, 0:1], in0=mv[:, 0:1],
                                    scalar1=nmr[:, 1:2], scalar2=-1.0,
                                    op0=AL.mult, op1=AL.mult)
            return nmr

        def ln_apply(src_ap, dst_ap, nmr, s_tile, b_tile):
            nc.vector.tensor_scalar(out=dst_ap, in0=src_ap,
                                    scalar1=nmr[:, 1:2], scalar2=nmr[:, 0:1],
                                    op0=AL.mult, op1=AL.add)
            if s_tile is not None:
                nc.vector.tensor_tensor(out=dst_ap, in0=dst_ap, in1=s_tile[:],
                                        op=AL.mult)
            if b_tile is not None:
                nc.vector.tensor_tensor(out=dst_ap, in0=dst_ap, in1=b_tile[:],
                                        op=AL.add)

        def embed(nt):
            g = work.tile([128, D], F32, tag="g", name=f"g{nt}")
            nc.gpsimd.indirect_dma_start(
                out=g[:], out_offset=None, in_=word[:],
                in_offset=bass.IndirectOffsetOnAxis(
                    ap=ids_sb[:, nt:nt + 1], axis=0))
            pt = work.tile([128, D], F32, tag="pt", name=f"pt{nt}")
            nc.sync.dma_start(out=pt[:], in_=posty_d[nt])
            nc.vector.tensor_tensor(out=g[:], in0=g[:], in1=pt[:], op=AL.add)
            dst = x_tm[:, nt, :]
            ln_apply(g[:], dst, ln_start(g[:], i=nt % 4),
                     None if fl["ln"] else elns, None if fl["ln"] else elnb)
            nc.sync.dma_start_transpose(xTa[:, :, nt * 128:(nt + 1) * 128], dst)

        def qk_group(gi, grp, w):
            g0, g1 = off[grp[0]], off[grp[-1] + 1]
            W = g1 - g0
            qkT = qkp.tile([128, 2, ND, 512], BF16, tag="qkT", name=f"qkT{gi}")
            for qi, wt, bt in ((0, w["wq"], "bq"), (1, w["wk"], "bk")):
                for ot in range(ND):
                    ps = ps_mm.tile([128, 512], F32, tag="mm")
                    for dt in range(ND):
                        nc.tensor.matmul(
                            ps[:, 0:W], wt[:, dt, ot * 128:(ot + 1) * 128],
                            xTa[:, dt, g0:g1], start=dt == 0, stop=dt == ND - 1)
                    if fl["bqk"]:
                        nc.vector.tensor_copy(out=qkT[:, qi, ot, 0:W],
                                              in_=ps[:, 0:W])
                    else:
                        nc.scalar.activation(
                            out=qkT[:, qi, ot, 0:W], in_=ps[:, 0:W],
                            func=AF.Identity, bias=w[bt][:, ot:ot + 1], scale=1.0)
            return qkT

        def v_tile(nt, w):
            ps = ps_mm.tile([128, 512], F32, tag="mm")
            for dt in range(ND):
                nc.tensor.matmul(ps[:], xTa[:, dt, nt * 128:(nt + 1) * 128],
                                 w["wv"][:, dt, :], start=dt == 0,
                                 stop=dt == ND - 1)
            if fl["bv"]:
                nc.vector.tensor_scalar_mul(
                    out=vA[:, nt, :, 0:DH],
                    in0=ps.rearrange("p (h d) -> p h d", h=H),
                    scalar1=vm_sb[:, nt:nt + 1])
            else:
                nc.vector.tensor_tensor(
                    out=vA[:, nt, :, 0:DH],
                    in0=ps.rearrange("p (h d) -> p h d", h=H),
                    in1=w["bv"].rearrange("p (h d) -> p h d", h=H), op=AL.add)
                nc.vector.tensor_scalar_mul(
                    out=vA[:, nt, :, 0:DH], in0=vA[:, nt, :, 0:DH],
                    scalar1=vm_sb[:, nt:nt + 1])
            nc.vector.tensor_scalar_mul(
                out=vA[:, nt, :, DH], in0=ones8[:],
                scalar1=vm_sb[:, nt:nt + 1])

        def attn_slot(l, j, qkT, g0, ctxb_map, last_slot_of):
            L = lam[j]
            ch = chunks[j]
            q0 = off[j] - g0
            eT = etp.tile([128, H, 512], BF16, tag="eT", name=f"eT{l}_{j}")
            for h in range(H):
                hh, dtH = (h % 2) * DH, h // 2
                pss = ps_s.tile([128, 512], F32, tag="s")
                for ci, (nt, b, kw, rel) in enumerate(ch):
                    nc.tensor.matmul(
                        pss[b:b + kw, ci * L:ci * L + L],
                        qkT[hh:hh + DH, 1, dtH, q0 + rel:q0 + rel + kw],
                        qkT[hh:hh + DH, 0, dtH, q0:q0 + L],
                        start=True, stop=True)
                nc.scalar.activation(out=eT[:, h, 0:len(ch) * L],
                                     in_=pss[:, 0:len(ch) * L], func=AF.Exp,
                                     bias=0.0, scale=1.0)
            # ctx per query chunk
            for (qnt, qb, qw, qrel) in ch:
                for hg in range(2):
                    cps = ps_c.tile([128, 4 * DH1], F32, tag="c")
                    for hi in range(4):
                        h = hg * 4 + hi
                        sl = slice(hi * DH1, hi * DH1 + DH1)
                        for ci, (nt, b, kw, rel) in enumerate(ch):
                            nc.tensor.matmul(
                                cps[qb:qb + qw, sl],
                                eT[b:b + kw, h, ci * L + qrel:ci * L + qrel + qw],
                                vA[b:b + kw, nt, h, :],
                                start=ci == 0, stop=ci == len(ch) - 1)
                    if qnt not in ctxb_map:
                        ctxb_map[qnt] = cxp.tile([128, D], BF16, tag="ctxb",
                                                 name=f"cb{l}_{qnt}")
                    ctxb = ctxb_map[qnt]
                    rcp = small.tile([128, 4], F32, tag="rcp")
                    nc.vector.reciprocal(
                        out=rcp[qb:qb + qw, :],
                        in_=cps.rearrange("p (h c) -> p h c", c=DH1)[qb:qb + qw, :, DH])
                    nc.vector.tensor_tensor(
                        out=ctxb.rearrange("p (h d) -> p h d", d=DH)[
                            qb:qb + qw, hg * 4:hg * 4 + 4, :],
                        in0=cps.rearrange("p (h c) -> p h c", c=DH1)[qb:qb + qw, :, 0:DH],
                        in1=rcp[qb:qb + qw, :, None].to_broadcast([qw, 4, DH]),
                        op=AL.mult)
            # flush finished ctxb tiles
            for (qnt, qb, qw, qrel) in ch:
                if last_slot_of.get(qnt) == j:
                    nc.sync.dma_start_transpose(
                        ctxT[:, :, qnt * 128:(qnt + 1) * 128], ctxb_map[qnt][:])

        def wo_ln1(nt, w):
            ps = ps_mm.tile([128, 512], F32, tag="mm")
            for dt in range(ND):
                nc.tensor.matmul(ps[:], ctxT[:, dt, nt * 128:(nt + 1) * 128],
                                 w["wo"][:, dt, :], start=dt == 0,
                                 stop=dt == ND - 1)
            r = work.tile([128, D], F32, tag="rln", name=f"r1_{nt}", bufs=3)
            nc.vector.tensor_tensor(out=r[:], in0=ps[:], in1=x_tm[:, nt, :],
                                    op=AL.add)
            if not fl["bo"]:
                nc.vector.tensor_tensor(out=r[:], in0=r[:], in1=w["bo"][:],
                                        op=AL.add)
            ln_apply(r[:], x_tm[:, nt, :], ln_start(r[:], i=nt % 4),
                     None if fl["ln"] else w["ln1s"],
                     None if fl["ln"] else w["ln1b"])
            nc.sync.dma_start_transpose(xTa[:, :, nt * 128:(nt + 1) * 128],
                                        x_tm[:, nt, :])

        def ffn_chunk(l, c0, c1, w):
            Wc = c1 - c0
            hT = htp.tile([128, NFT, 512], BF16, tag="hT", name=f"hT{l}_{c0}")
            for ft in range(NFT):
                ps = ps_mm.tile([128, 512], F32, tag="mm")
                for dt in range(ND):
                    nc.tensor.matmul(
                        ps[:, 0:Wc], w["wi"][:, dt, ft * 128:(ft + 1) * 128],
                        xTa[:, dt, c0:c1], start=dt == 0, stop=dt == ND - 1)
                nc.scalar.activation(
                    out=hT[:, ft, 0:Wc], in_=ps[:, 0:Wc], func=AF.Gelu,
                    bias=0.0 if fl["bi"] else w["bi"][:, ft:ft + 1], scale=1.0)
            for nt in range(c0 // 128, c1 // 128):
                toff = nt * 128 - c0
                ps = ps_mm.tile([128, 512], F32, tag="mm")
                for ft in range(NFT):
                    nc.tensor.matmul(ps[:], hT[:, ft, toff:toff + 128],
                                     w["wo2"][:, ft, :], start=ft == 0,
                                     stop=ft == NFT - 1)
                r = work.tile([128, D], F32, tag="rln", name=f"r2_{l}_{nt}", bufs=3)
                nc.vector.tensor_tensor(out=r[:], in0=ps[:], in1=x_tm[:, nt, :],
                                        op=AL.add)
                if not fl["bo2"]:
                    nc.vector.tensor_tensor(out=r[:], in0=r[:], in1=w["bo2"][:],
                                            op=AL.add)
                ln_apply(r[:], x_tm[:, nt, :], ln_start(r[:], i=nt % 4),
                         None if fl["ln"] else w["ln2s"],
                         None if fl["ln"] else w["ln2b"])
                nc.sync.dma_start_transpose(xTa[:, :, nt * 128:(nt + 1) * 128],
                                            x_tm[:, nt, :])

        # last slot writing each ctx tile (for flush scheduling)
        last_slot_of = {}
        for j in range(NSL):
            for (nt, b, kw, rel) in chunks[j]:
                last_slot_of[nt] = j

        NCH = TP // 512 + (1 if TP % 512 else 0)
        chunk_rng = [(ci * 512, min((ci + 1) * 512, TP)) for ci in range(NCH)]

        # tiles first touched by each qk group (for embed/V scheduling)
        emb_done = set()

        def new_tiles(grp):
            g0, g1 = off[grp[0]], off[grp[-1] + 1]
            ts = [t for t in range(g0 // 128, -(-g1 // 128)) if t not in emb_done]
            emb_done.update(ts)
            return ts

        # ---- program ----
        # All embeds first (gathers + posty DMAs ahead of the big weight
        # DMAs in the queues), in group order so group 0 finishes first.
        for grp in lay["qk_groups"]:
            for nt in new_tiles(grp):
                embed(nt)
        for nt in range(TP // 128):
            if nt not in emb_done:
                emb_done.add(nt)
                embed(nt)
        w_cur = load_layer_weights(0, part="qk")
        load_layer_weights(0, w=w_cur, part="rest")
        nc.vector.memset(ctxT[:], 0.0)
        for l in range(NL):
            ctxb_map = {}
            v_done = set()
            wo_done = set()
            ffn_done = set()

            def flush(j):
                # Wo + LN1 for tiles whose attention is complete, then any
                # FFN chunk whose 4 tiles are all LN1'd — keeps dense PE
                # work interleaved with the ACT-paced softmax chain.
                for nt in range(NTT):
                    if nt in wo_done or last_slot_of.get(nt, -1) > j:
                        continue
                    if nt not in ctxb_map and nt in last_slot_of:
                        continue  # not yet computed this pass
                    wo_done.add(nt)
                    wo_ln1(nt, w_cur)
                for ci, (c0, c1) in enumerate(chunk_rng):
                    if ci in ffn_done:
                        continue
                    if all(t in wo_done for t in range(c0 // 128, c1 // 128)):
                        ffn_done.add(ci)
                        ffn_chunk(l, c0, c1, w_cur)

            for gi, grp in enumerate(lay["qk_groups"]):
                qkT = qk_group(gi, grp, w_cur)
                for j in grp:
                    for (nt, b, kw, rel) in chunks[j]:
                        if nt not in v_done:
                            v_done.add(nt)
                            v_tile(nt, w_cur)
                    attn_slot(l, j, qkT, off[grp[0]], ctxb_map, last_slot_of)
                    if j >= 1:
                        flush(j - 1)
                if gi == 0 and l + 1 < NL:
                    w_nxt = load_layer_weights(l + 1, part="qk")
            if l + 1 < NL:
                load_layer_weights(l + 1, w=w_nxt, part="rest")
            flush(NSL)
            if l + 1 < NL:
                w_cur = w_nxt

        if debug:
            for nt in range(NTT):
                dx = work.tile([128, D], F32, tag="dbg", name=f"dbg{nt}")
                nc.vector.tensor_copy(out=dx[:], in_=x_tm[:, nt, :])
                nc.sync.dma_start(out=dbgx_d[nt], in_=dx[:])

        # ---- conv head ----
        # xcv reuses ctxT's slot (attention is done), pen/cvt reuse the
        # embed-phase work slots — keeps peak SBUF under the cap
        xcv = state.tile([128, ND, CWP], BF16, tag="ctxT", name="xcv")
        nc.vector.memset(xcv[:], 0.0)
        for j in range(NSL):
            o0, c0 = off[j], coff[j]
            for dt in range(ND):
                nc.vector.tensor_tensor(
                    out=xcv[:, dt, c0:c0 + lam[j]],
                    in0=xTa[:, dt, o0:o0 + lam[j]],
                    in1=cm_sb[:, c0:c0 + lam[j]], op=AL.mult)
        for cgi, cg in enumerate(lay["cv_groups"]):
            cs, ce = coff[cg[0]], coff[cg[-1] + 1]
            Wg = ce - cs
            for ki, k in enumerate((1, 2, 3)):
                pen = work.tile([128, 512], F32, tag="g", name=f"pn{cgi}_{ki}")
                nc.sync.dma_start(
                    out=pen[:, 0:Wg],
                    in_=cpen_d[ki, cs:ce][None, :].to_broadcast([128, Wg]))
                for ft in range(2):
                    ps = ps_mm.tile([128, 512], F32, tag="mm")
                    idx = 0
                    for dt in range(ND):
                        for jj in range(k):
                            nc.tensor.matmul(
                                ps[:, 0:Wg],
                                cw[(k, jj)][:, dt, ft * 128:(ft + 1) * 128],
                                xcv[:, dt, cs + jj:cs + jj + Wg],
                                start=idx == 0, stop=idx == ND * k - 1)
                            idx += 1
                    cvt = work.tile([128, 512], F32, tag="pt",
                                    name=f"cv{cgi}_{ki}_{ft}")
                    nc.vector.tensor_tensor(out=cvt[:, 0:Wg], in0=ps[:, 0:Wg],
                                            in1=pen[:, 0:Wg], op=AL.add)
                    for j in cg:
                        rs = coff[j] - cs
                        re = rs + lam[j] - k + 2
                        nc.vector.tensor_reduce(
                            out=rep[:, ki * 2 + ft, j:j + 1], in_=cvt[:, rs:re],
                            axis=mybir.AxisListType.X, op=AL.max)

        if not fl["cb"]:
            for ki in range(3):
                for ft in range(2):
                    nc.vector.tensor_scalar_add(
                        out=rep[:, ki * 2 + ft, :], in0=rep[:, ki * 2 + ft, :],
                        scalar1=cb[:, ki, ft:ft + 1])
        nc.scalar.activation(out=rep[:], in_=rep[:], func=AF.Relu)

        fps = ps_c.tile([128, 3 * NCLS], F32, tag="c")
        for c in range(6):
            nc.tensor.matmul(fps[:NSL, :], rep[:, c, :], fcw_sb[:, c, :],
                             start=c == 0, stop=c == 5)
        ob = small.tile([NSL, 3 * NCLS], F32, tag="ob")
        nc.scalar.copy(out=ob[:], in_=fps[:NSL, :])
        nc.sync.dma_start(out=out_d[:], in_=ob[:])

    nc.compile()
    return nc


def _core_inputs(inputs, fl, lay):
    """Build the 8 per-core input maps from the full problem inputs."""
    f32 = lambda a: np.ascontiguousarray(np.asarray(a, dtype=np.float32))
    tile_w = lambda w: np.ascontiguousarray(
        f32(w).reshape(w.shape[0] // 128, 128, w.shape[1]).astype(BF))

    NTT, TP, CWP = lay["NTT"], lay["TP"], lay["CWP"]
    lam, lam32, off, coff = lay["lam"], lay["lam32"], lay["off"], lay["coff"]
    assign = lay["assign"]

    shared = {}
    # packed position+type embedding
    posv = np.zeros((TP, D), np.float32)
    pe = f32(inputs["pos_emb"])
    for j in range(NSL):
        posv[off[j]:off[j] + lam32[j]] = pe[:lam32[j]]
    posv += f32(inputs["type_emb"][0])[None, :]
    shared["posty"] = np.ascontiguousarray(posv.reshape(NTT, 128, D))
    for l in range(NL):
        shared[f"wq{l}"] = tile_w(f32(inputs["Wq"][l]) / 8.0)
        shared[f"wk{l}"] = tile_w(inputs["Wk"][l])
        shared[f"wv{l}"] = tile_w(inputs["Wv"][l])
        shared[f"wo{l}"] = tile_w(inputs["Wo"][l])
        shared[f"wi{l}"] = tile_w(inputs["Wi"][l])
        shared[f"wo2{l}"] = tile_w(inputs["Wo2"][l])
        if not fl["bqk"]:
            shared[f"bq{l}"] = f32(inputs["bq"][l]).reshape(ND, 128) / 8.0
            shared[f"bk{l}"] = f32(inputs["bk"][l]).reshape(ND, 128)
        if not fl["bv"]:
            shared[f"bv{l}"] = f32(inputs["bv"][l])
        if not fl["bo"]:
            shared[f"bo{l}"] = f32(inputs["bo"][l])
        if not fl["bi"]:
            shared[f"bi{l}"] = f32(inputs["bi"][l]).reshape(NFT, 128)
        if not fl["bo2"]:
            shared[f"bo2{l}"] = f32(inputs["bo2"][l])
        if not fl["ln"]:
            shared[f"ln1s{l}"] = f32(inputs["ln1_s"][l])
            shared[f"ln1b{l}"] = f32(inputs["ln1_b"][l])
            shared[f"ln2s{l}"] = f32(inputs["ln2_s"][l])
            shared[f"ln2b{l}"] = f32(inputs["ln2_b"][l])
    if not fl["ln"]:
        shared["lnes"] = f32(inputs["emb_ln_s"])
        shared["lneb"] = f32(inputs["emb_ln_b"])
    for ki, k in enumerate((1, 2, 3)):
        w = f32(inputs[f"conv_w{k}"])                    # [NF, k, D]
        wt = np.ascontiguousarray(w.transpose(1, 2, 0))  # [k, D, NF]
        shared[f"cw{k}"] = np.ascontiguousarray(
            wt.reshape(k, ND, 128, NF).astype(BF))
    if not fl["cb"]:
        shared["convb"] = np.stack(
            [f32(inputs[f"conv_b{k}"]).reshape(2, 128) for k in (1, 2, 3)])
    # fc weights for all 3 branch-block hypotheses: [6, 128, 3*NCLS]
    fcw = f32(inputs["fc_w"])                            # [2304, NCLS]
    fcw3 = np.zeros((6, 128, 3 * NCLS), np.float32)
    for bb in range(3):
        for ki in range(3):
            for ft in range(2):
                c = ki * 2 + ft
                rows = 768 * bb + 256 * ki + 128 * ft
                fcw3[c, :, bb * NCLS:(bb + 1) * NCLS] = fcw[rows:rows + 128]
    shared["fcw"] = fcw3
    shared["word_emb"] = f32(inputs["word_emb"])

    ids_all = np.stack([np.asarray(inputs[p + "_input_ids"])
                        for p in ("q", "a", "b")]).reshape(96, S)
    mask_all = np.stack([np.asarray(inputs[p + "_attention_mask"])
                         for p in ("q", "a", "b")]).reshape(96, S)
    lens_all = mask_all.sum(1).astype(int)

    in_maps = []
    for c in range(NCORES):
        m = dict(shared)
        idv = np.zeros(TP, np.int32)
        vmv = np.zeros(TP, np.float32)
        cmv = np.zeros(CWP, np.float32)
        pen = np.full((3, CWP), -1e30, np.float32)
        for j in range(NSL):
            sq = int(assign[j, c])
            l = int(lens_all[sq])
            idv[off[j]:off[j] + lam32[j]] = ids_all[sq][:lam32[j]]
            vmv[off[j]:off[j] + l] = 1.0
            cmv[coff[j]:coff[j] + l] = 1.0
            for ki, k in enumerate((1, 2, 3)):
                nw = l - k + 2
                pen[ki, coff[j]:coff[j] + nw] = 0.0
        m["ids"] = np.ascontiguousarray(idv.reshape(NTT, 128))
        m["vmask"] = np.ascontiguousarray(vmv.reshape(NTT, 128))
        m["convmask"] = np.ascontiguousarray(cmv.astype(BF))
        m["convpen"] = np.ascontiguousarray(pen)
        in_maps.append(m)
    return in_maps


def _get_program(fl, lay, debug=False):
    key = (tuple(sorted(fl.items())), lay["key"], debug)
    if key not in _CACHE:
        _CACHE[key] = _build_program(fl, lay, debug=debug)
    return _CACHE[key]


def run_sharded(inputs, debug=False, **run_kwargs):
    """Shard, run on 8 cores, gather. Returns (output, BassKernelResults)."""
    from concourse.bass_utils import run_bass_kernel_spmd
    fl = _flags(inputs)
    lens96 = np.concatenate([
        np.asarray(inputs[p + "_attention_mask"]).sum(1) for p in ("q", "a", "b")])
    lay = _layout(lens96)
    nc = _get_program(fl, lay, debug=debug)
    in_maps = _core_inputs(inputs, fl, lay)
    res = run_bass_kernel_spmd(nc, in_maps, core_ids=list(range(NCORES)),
                               **run_kwargs)
    border = {0: 0, 1: 2, 2: 1}   # branch q/a/b -> fc block q,b,a
    out = np.zeros((B, NCLS), np.float32)
    for c in range(NCORES):
        o3 = np.asarray(res.results[c]["out"], np.float32)   # [NSL, 12]
        for j in range(NSL):
            sq = int(lay["assign"][j, c])
            br, sample = sq // B, sq % B
            out[sample] += o3[j, border[br] * NCLS:(border[br] + 1) * NCLS]
    out += np.asarray(inputs["fc_b"], np.float32)[None, :]
    return out, res


def kernel(**inputs):
    out, _ = run_sharded(inputs)
    return out


# revision 26
# speedup vs baseline: 1.7055x; 1.0482x over previous
"""Trainium2 Bass kernel for nn_BertCNN (3x BERT-small encoder + CNN maxpool head).

Ragged-packed data-parallel strategy. The 96 sequences (3 branches x 32
samples) are sorted by actual length (from the attention mask), dealt
round-robin into 8 cores x 12 slots, and each core packs its 12 sequences
into one ~1900-token stream (slot budgets = max length in each rank group,
32-aligned starts). All encoder linear ops (QKV/O/FFN/conv) run over the
packed stream; attention runs per-slot with exact budget widths; key
validity is folded multiplicatively into V (invalid keys get zero V rows
and a zero softmax-denominator contribution), so no attention bias is
needed. The conv head runs over a separately packed layout with 2-token
gaps; per-slot maxpool ranges and window-validity penalties come from the
host. The fc output is computed for all 3 branch hypotheses per slot
([12, 12] per core) and the host scatter-adds the right 4 columns into the
final [32, 4].

The Bass program depends only on the slot-budget layout (not on per-core
data); it is built once per layout signature and cached.
"""

import numpy as np
import ml_dtypes

V, D, H, DH, NL, FF = 30522, 512, 8, 64, 4, 2048
NF, NCLS, B, S = 256, 4, 32, 256
NCORES = 8
NSL = 12                 # slots (sequences) per core
NSEQ = NSL               # test.py compat
SPC = 4                  # test.py compat
ND = D // 128
NFT = FF // 128
DH1 = DH + 1

BF = ml_dtypes.bfloat16
_CACHE = {}


def _flags(inputs):
    z = lambda a: bool(np.all(np.asarray(a) == 0))
    o = lambda a: bool(np.all(np.asarray(a) == 1))
    return {
        "bqk": z(inputs["bq"]) and z(inputs["bk"]),
        "bv": z(inputs["bv"]),
        "bo": z(inputs["bo"]),
        "bi": z(inputs["bi"]),
        "bo2": z(inputs["bo2"]),
        "ln": all(o(inputs[k]) for k in ("emb_ln_s", "ln1_s", "ln2_s"))
        and all(z(inputs[k]) for k in ("emb_ln_b", "ln1_b", "ln2_b")),
        "cb": z(inputs["conv_b1"]) and z(inputs["conv_b2"]) and z(inputs["conv_b3"]),
    }


def _layout(lens96):
    """Pack layout shared by all cores (program-shaping constants)."""
    lens96 = np.asarray(lens96, dtype=np.int64)
    order = np.argsort(-lens96, kind="stable")
    assign = order.reshape(NSL, NCORES)           # [slot, core] -> seq idx
    lam = lens96[assign].max(1).astype(int)       # slot budgets (max len)
    # 64-aligned slot starts (PE col/row tiling only supports base 0/64 for
    # >32-wide tiles); bump a start to the next 128 boundary when the slot
    # would otherwise span 3 token tiles (score/eT tiles hold 2 chunks)
    lam32 = ((lam + 63) // 64) * 64
    off = np.zeros(NSL + 1, np.int64)
    for j in range(NSL):
        o = off[j]
        if (o % 128) + lam[j] > 256:
            o = ((o + 127) // 128) * 128
            off[j] = o
        off[j + 1] = o + lam32[j]
    T32 = int(off[-1])
    NTT = -(-T32 // 128)
    TP = NTT * 128
    coff = np.zeros(NSL + 1, np.int64)
    coff[1:] = np.cumsum(lam + 2)
    CW = int(coff[-1])

    def greedy(offs, cap):
        groups, cur = [], [0]
        for j in range(1, NSL):
            if offs[j + 1] - offs[cur[0]] <= cap:
                cur.append(j)
            else:
                groups.append(cur)
                cur = [j]
        groups.append(cur)
        return groups

    qk_groups = greedy(off, 512)
    cv_groups = greedy(coff, 512)

    # per-slot key/query chunks: intersections with the global 128 grid
    chunks = []
    for j in range(NSL):
        lo, hi = int(off[j]), int(off[j] + lam[j])
        ch = []
        p = lo
        while p < hi:
            nt = p // 128
            e = min(hi, (nt + 1) * 128)
            ch.append((nt, p - nt * 128, e - p, p - lo))  # (tile, base, width, rel)
            p = e
        chunks.append(ch)
    return dict(
        assign=assign, lam=[int(x) for x in lam], lam32=[int(x) for x in lam32],
        off=[int(x) for x in off], coff=[int(x) for x in coff],
        T32=T32, NTT=NTT, TP=TP, CW=CW, CWP=CW + 2,
        qk_groups=qk_groups, cv_groups=cv_groups, chunks=chunks,
        key=(tuple(int(x) for x in lam), tuple(int(x) for x in lam32)),
    )


def _build_program(fl, lay, debug=False):
    import contextlib
    import concourse.bass as bass
    import concourse.mybir as mybir
    import concourse.tile as tile
    from concourse import bacc

    F32, BF16, I32 = mybir.dt.float32, mybir.dt.bfloat16, mybir.dt.int32
    AL, AF = mybir.AluOpType, mybir.ActivationFunctionType

    NTT, TP, CWP = lay["NTT"], lay["TP"], lay["CWP"]
    lam, lam32, off, coff = lay["lam"], lay["lam32"], lay["off"], lay["coff"]
    chunks = lay["chunks"]

    nc = bacc.Bacc("TRN2", target_bir_lowering=False, debug=False,
                   num_devices=NCORES)

    di = lambda n, s, d: nc.dram_tensor(n, s, d, kind="ExternalInput").ap()
    word = di("word_emb", [V, D], F32)
    ids_d = di("ids", [NTT, 128], I32)
    vm_d = di("vmask", [NTT, 128], F32)
    posty_d = di("posty", [NTT, 128, D], F32)
    cm_d = di("convmask", [CWP], BF16)
    cpen_d = di("convpen", [3, CWP], F32)
    wq_d = [di(f"wq{l}", [ND, 128, D], BF16) for l in range(NL)]
    wk_d = [di(f"wk{l}", [ND, 128, D], BF16) for l in range(NL)]
    wv_d = [di(f"wv{l}", [ND, 128, D], BF16) for l in range(NL)]
    wo_d = [di(f"wo{l}", [ND, 128, D], BF16) for l in range(NL)]
    wi_d = [di(f"wi{l}", [ND, 128, FF], BF16) for l in range(NL)]
    wo2_d = [di(f"wo2{l}", [NFT, 128, D], BF16) for l in range(NL)]
    cw_d = [di(f"cw{k}", [k, ND, 128, NF], BF16) for k in (1, 2, 3)]
    fcw_d = di("fcw", [6, 128, 3 * NCLS], F32)
    if not fl["bqk"]:
        bq_d = [di(f"bq{l}", [ND, 128], F32) for l in range(NL)]
        bk_d = [di(f"bk{l}", [ND, 128], F32) for l in range(NL)]
    if not fl["bv"]:
        bv_d = [di(f"bv{l}", [D], F32) for l in range(NL)]
    if not fl["bo"]:
        bo_d = [di(f"bo{l}", [D], F32) for l in range(NL)]
    if not fl["bi"]:
        bi_d = [di(f"bi{l}", [NFT, 128], F32) for l in range(NL)]
    if not fl["bo2"]:
        bo2_d = [di(f"bo2{l}", [D], F32) for l in range(NL)]
    if not fl["ln"]:
        elns_d = di("lnes", [D], F32)
        elnb_d = di("lneb", [D], F32)
        ln1s_d = [di(f"ln1s{l}", [D], F32) for l in range(NL)]
        ln1b_d = [di(f"ln1b{l}", [D], F32) for l in range(NL)]
        ln2s_d = [di(f"ln2s{l}", [D], F32) for l in range(NL)]
        ln2b_d = [di(f"ln2b{l}", [D], F32) for l in range(NL)]
    if not fl["cb"]:
        cb_d = di("convb", [3, 2, 128], F32)

    out_d = nc.dram_tensor("out", [NSL, 3 * NCLS], F32, kind="ExternalOutput").ap()
    if debug:
        dbgx_d = nc.dram_tensor("dbgx", [NTT, 128, D], F32,
                                kind="ExternalOutput").ap()

    with tile.TileContext(nc) as tc, contextlib.ExitStack() as ctx:
        consts = ctx.enter_context(tc.tile_pool(name="consts", bufs=1))
        state = ctx.enter_context(tc.tile_pool(name="state", bufs=1))
        wts = ctx.enter_context(tc.tile_pool(name="wts", bufs=1))
        qkp = ctx.enter_context(tc.tile_pool(name="qkp", bufs=2))
        etp = ctx.enter_context(tc.tile_pool(name="etp", bufs=2))
        htp = ctx.enter_context(tc.tile_pool(name="htp", bufs=1))
        work = ctx.enter_context(tc.tile_pool(name="work", bufs=2))
        cxp = ctx.enter_context(tc.tile_pool(name="cxp", bufs=3))
        small = ctx.enter_context(tc.tile_pool(name="small", bufs=4))
        ps_mm = ctx.enter_context(tc.tile_pool(name="ps_mm", bufs=3, space="PSUM"))
        ps_s = ctx.enter_context(tc.tile_pool(name="ps_s", bufs=3, space="PSUM"))
        ps_c = ctx.enter_context(tc.tile_pool(name="ps_c", bufs=2, space="PSUM"))

        # ---- constants ----
        eps_t = consts.tile([128, 1], F32, tag="eps")
        nc.vector.memset(eps_t[:], 1e-12)
        ones8 = consts.tile([128, H], BF16, tag="ones8")
        nc.vector.memset(ones8[:], 1.0)
        ids_sb = consts.tile([128, NTT], I32, tag="ids")
        nc.sync.dma_start(out=ids_sb[:], in_=ids_d.rearrange("t p -> p t"))
        vm_sb = consts.tile([128, NTT], F32, tag="vm")
        nc.sync.dma_start(out=vm_sb[:], in_=vm_d.rearrange("t p -> p t"))
        cm_sb = consts.tile([128, CWP], BF16, tag="cm")
        nc.sync.dma_start(out=cm_sb[:],
                          in_=cm_d[None, :].to_broadcast([128, CWP]))
        fcw_sb = consts.tile([128, 6, 3 * NCLS], F32, tag="fcw")
        nc.sync.dma_start(out=fcw_sb[:], in_=fcw_d.rearrange("c p n -> p c n"))
        bcast = lambda ap, n: ap[None, :].to_broadcast([128, n])
        if not fl["ln"]:
            elns = consts.tile([128, D], F32, tag="elns")
            nc.sync.dma_start(out=elns[:], in_=bcast(elns_d, D))
            elnb = consts.tile([128, D], F32, tag="elnb")
            nc.sync.dma_start(out=elnb[:], in_=bcast(elnb_d, D))
        if not fl["cb"]:
            cb = consts.tile([128, 3, 2], F32, tag="cb")
            nc.sync.dma_start(out=cb[:], in_=cb_d.rearrange("k t p -> p k t"))

        cw = {}
        for ki, k in enumerate((1, 2, 3)):
            for j in range(k):
                t = wts.tile([128, ND, NF], BF16, tag=f"cw{ki}_{j}")
                nc.sync.dma_start(out=t[:],
                                  in_=cw_d[ki][j].rearrange("t p f -> p t f"))
                cw[(k, j)] = t

        # ---- persistent state ----
        x_tm = state.tile([128, NTT, D], BF16, tag="x_tm")
        xTa = state.tile([128, ND, TP], BF16, tag="xTa")
        ctxT = state.tile([128, ND, TP], BF16, tag="ctxT")
        vA = state.tile([128, NTT, H, DH1], BF16, tag="vA")
        rep = state.tile([128, 6, NSL], F32, tag="rep")

        def load_layer_weights(l, w=None, part="all"):
            if w is None:
                w = {}
            names = {"qk": ("wq", "wk"), "rest": ("wv", "wo", "wi"),
                     "all": ("wq", "wk", "wv", "wo", "wi")}[part]
            for nm, dd, nfree in (("wq", wq_d, D), ("wk", wk_d, D),
                                  ("wv", wv_d, D), ("wo", wo_d, D),
                                  ("wi", wi_d, FF)):
                if nm not in names:
                    continue
                w[nm] = wts.tile([128, ND, nfree], BF16, tag=nm, name=f"{nm}_{l}",
                                 bufs=2 if nm in ("wq", "wk") else 1)
                for dt in range(ND):
                    nc.sync.dma_start(out=w[nm][:, dt, :], in_=dd[l][dt])
            if part == "qk":
                return w
            w["wo2"] = wts.tile([128, NFT, D], BF16, tag="wo2", name=f"wo2_{l}")
            for ft in range(0, NFT, 4):
                nc.sync.dma_start(
                    out=w["wo2"][:, ft:ft + 4, :],
                    in_=wo2_d[l][ft:ft + 4].rearrange("t p o -> p t o"))
            if not fl["bqk"]:
                w["bq"] = consts.tile([128, ND], F32, tag="bq", name=f"bq_{l}")
                nc.sync.dma_start(out=w["bq"][:], in_=bq_d[l].rearrange("t p -> p t"))
                w["bk"] = consts.tile([128, ND], F32, tag="bk", name=f"bk_{l}")
                nc.sync.dma_start(out=w["bk"][:], in_=bk_d[l].rearrange("t p -> p t"))
            if not fl["bv"]:
                w["bv"] = consts.tile([128, D], F32, tag="bv", name=f"bv_{l}")
                nc.sync.dma_start(out=w["bv"][:], in_=bcast(bv_d[l], D))
            if not fl["bo"]:
                w["bo"] = consts.tile([128, D], F32, tag="bo", name=f"bo_{l}")
                nc.sync.dma_start(out=w["bo"][:], in_=bcast(bo_d[l], D))
            if not fl["bi"]:
                w["bi"] = consts.tile([128, NFT], F32, tag="bi", name=f"bi_{l}")
                nc.sync.dma_start(out=w["bi"][:], in_=bi_d[l].rearrange("t p -> p t"))
            if not fl["bo2"]:
                w["bo2"] = consts.tile([128, D], F32, tag="bo2", name=f"bo2_{l}")
                nc.sync.dma_start(out=w["bo2"][:], in_=bcast(bo2_d[l], D))
            if not fl["ln"]:
                for nm, dd in (("ln1s", ln1s_d), ("ln1b", ln1b_d),
                               ("ln2s", ln2s_d), ("ln2b", ln2b_d)):
                    w[nm] = consts.tile([128, D], F32, tag=nm, name=f"{nm}_{l}")
                    nc.sync.dma_start(out=w[nm][:], in_=bcast(dd[l], D))
            return w

        def ln_start(src_ap, i=0):
            st = small.tile([128, 6], F32, tag="st", name=f"st{i}")
            mv = small.tile([128, 2], F32, tag="mv", name=f"mv{i}")
            nc.vector.bn_stats(out=st[:], in_=src_ap)
            nc.vector.bn_aggr(out=mv[:], in_=st[:])
            sd = small.tile([128, 1], F32, tag="sd", name=f"sd{i}")
            nc.scalar.activation(out=sd[:], in_=mv[:, 1:2], func=AF.Sqrt,
                                 bias=eps_t[:], scale=1.0)
            nmr = small.tile([128, 2], F32, tag="nmr", name=f"nmr{i}")
            nc.vector.reciprocal(out=nmr[:, 1:2], in_=sd[:])
            nc.vector.tensor_scalar(out=nmr[:, 0:1], in0=mv[:, 0:1],
                                    scalar1=nmr[:, 1:2], scalar2=-1.0,
                                    op0=AL.mult, op1=AL.mult)
            return nmr

        def ln_apply(src_ap, dst_ap, nmr, s_tile, b_tile):
            nc.vector.tensor_scalar(out=dst_ap, in0=src_ap,
                                    scalar1=nmr[:, 1:2], scalar2=nmr[:, 0:1],
                                    op0=AL.mult, op1=AL.add)
            if s_tile is not None:
                nc.vector.tensor_tensor(out=dst_ap, in0=dst_ap, in1=s_tile[:],
                                        op=AL.mult)
            if b_tile is not None:
                nc.vector.tensor_tensor(out=dst_ap, in0=dst_ap, in1=b_tile[:],
                                        op=AL.add)

        def embed(nt):
            g = work.tile([128, D], F32, tag="g", name=f"g{nt}")
            nc.gpsimd.indirect_dma_start(
                out=g[:], out_offset=None, in_=word[:],
                in_offset=bass.IndirectOffsetOnAxis(
                    ap=ids_sb[:, nt:nt + 1], axis=0))
            pt = work.tile([128, D], F32, tag="pt", name=f"pt{nt}")
            nc.sync.dma_start(out=pt[:], in_=posty_d[nt])
            nc.vector.tensor_tensor(out=g[:], in0=g[:], in1=pt[:], op=AL.add)
            dst = x_tm[:, nt, :]
            ln_apply(g[:], dst, ln_start(g[:], i=nt % 4),
                     None if fl["ln"] else elns, None if fl["ln"] else elnb)
            nc.sync.dma_start_transpose(xTa[:, :, nt * 128:(nt + 1) * 128], dst)

        def qk_group(gi, grp, w):
            g0, g1 = off[grp[0]], off[grp[-1] + 1]
            W = g1 - g0
            qkT = qkp.tile([128, 2, ND, 512], BF16, tag="qkT", name=f"qkT{gi}")
            for qi, wt, bt in ((0, w["wq"], "bq"), (1, w["wk"], "bk")):
                for ot in range(ND):
                    ps = ps_mm.tile([128, 512], F32, tag="mm")
                    for dt in range(ND):
                        nc.tensor.matmul(
                            ps[:, 0:W], wt[:, dt, ot * 128:(ot + 1) * 128],
                            xTa[:, dt, g0:g1], start=dt == 0, stop=dt == ND - 1)
                    if fl["bqk"]:
                        if qi == 1:
                            nc.scalar.copy(out=qkT[:, qi, ot, 0:W],
                                           in_=ps[:, 0:W])
                        else:
                            nc.vector.tensor_copy(out=qkT[:, qi, ot, 0:W],
                                                  in_=ps[:, 0:W])
                    else:
                        nc.scalar.activation(
                            out=qkT[:, qi, ot, 0:W], in_=ps[:, 0:W],
                            func=AF.Identity, bias=w[bt][:, ot:ot + 1], scale=1.0)
            return qkT

        def v_tile(nt, w):
            ps = ps_mm.tile([128, 512], F32, tag="mm")
            for dt in range(ND):
                nc.tensor.matmul(ps[:], xTa[:, dt, nt * 128:(nt + 1) * 128],
                                 w["wv"][:, dt, :], start=dt == 0,
                                 stop=dt == ND - 1)
            if fl["bv"]:
                nc.vector.tensor_scalar_mul(
                    out=vA[:, nt, :, 0:DH],
                    in0=ps.rearrange("p (h d) -> p h d", h=H),
                    scalar1=vm_sb[:, nt:nt + 1])
            else:
                nc.vector.tensor_tensor(
                    out=vA[:, nt, :, 0:DH],
                    in0=ps.rearrange("p (h d) -> p h d", h=H),
                    in1=w["bv"].rearrange("p (h d) -> p h d", h=H), op=AL.add)
                nc.vector.tensor_scalar_mul(
                    out=vA[:, nt, :, 0:DH], in0=vA[:, nt, :, 0:DH],
                    scalar1=vm_sb[:, nt:nt + 1])
            nc.vector.tensor_scalar_mul(
                out=vA[:, nt, :, DH], in0=ones8[:],
                scalar1=vm_sb[:, nt:nt + 1])

        def attn_slot(l, j, qkT, g0, ctxb_map, last_slot_of):
            L = lam[j]
            ch = chunks[j]
            q0 = off[j] - g0
            eT = etp.tile([128, H, 512], BF16, tag="eT", name=f"eT{l}_{j}")
            for h in range(H):
                hh, dtH = (h % 2) * DH, h // 2
                pss = ps_s.tile([128, 512], F32, tag="s")
                for ci, (nt, b, kw, rel) in enumerate(ch):
                    nc.tensor.matmul(
                        pss[b:b + kw, ci * L:ci * L + L],
                        qkT[hh:hh + DH, 1, dtH, q0 + rel:q0 + rel + kw],
                        qkT[hh:hh + DH, 0, dtH, q0:q0 + L],
                        start=True, stop=True)
                nc.scalar.activation(out=eT[:, h, 0:len(ch) * L],
                                     in_=pss[:, 0:len(ch) * L], func=AF.Exp,
                                     bias=0.0, scale=1.0)
            # ctx per query chunk
            for (qnt, qb, qw, qrel) in ch:
                for hg in range(2):
                    cps = ps_c.tile([128, 4 * DH1], F32, tag="c")
                    for hi in range(4):
                        h = hg * 4 + hi
                        sl = slice(hi * DH1, hi * DH1 + DH1)
                        for ci, (nt, b, kw, rel) in enumerate(ch):
                            nc.tensor.matmul(
                                cps[qb:qb + qw, sl],
                                eT[b:b + kw, h, ci * L + qrel:ci * L + qrel + qw],
                                vA[b:b + kw, nt, h, :],
                                start=ci == 0, stop=ci == len(ch) - 1)
                    if qnt not in ctxb_map:
                        ctxb_map[qnt] = cxp.tile([128, D], BF16, tag="ctxb",
                                                 name=f"cb{l}_{qnt}")
                    ctxb = ctxb_map[qnt]
                    rcp = small.tile([128, 4], F32, tag="rcp")
                    nc.vector.reciprocal(
                        out=rcp[qb:qb + qw, :],
                        in_=cps.rearrange("p (h c) -> p h c", c=DH1)[qb:qb + qw, :, DH])
                    nc.vector.tensor_tensor(
                        out=ctxb.rearrange("p (h d) -> p h d", d=DH)[
                            qb:qb + qw, hg * 4:hg * 4 + 4, :],
                        in0=cps.rearrange("p (h c) -> p h c", c=DH1)[qb:qb + qw, :, 0:DH],
                        in1=rcp[qb:qb + qw, :, None].to_broadcast([qw, 4, DH]),
                        op=AL.mult)
            # flush finished ctxb tiles
            for (qnt, qb, qw, qrel) in ch:
                if last_slot_of.get(qnt) == j:
                    nc.sync.dma_start_transpose(
                        ctxT[:, :, qnt * 128:(qnt + 1) * 128], ctxb_map[qnt][:])

        # deferred-LN machinery: residual-add lands pre-LN values in x_tm
        # (bf16, in place); per-tile bn stats collect into a batch buffer;
        # one Sqrt region per phase finalizes all tiles (ACT table stays
        # resident for Exp/Gelu — each table swap costs ~1.5us)
        mvb = {}
        for ph in ("ln1", "ln2"):
            mvb[ph] = state.tile([128, 2, NTT], F32, tag=f"mvb_{ph}",
                                 name=f"mvb_{ph}")

        def res_stats(nt, ps, bias_t, ph, i=0):
            nc.vector.tensor_tensor(out=x_tm[:, nt, :], in0=ps[:],
                                    in1=x_tm[:, nt, :], op=AL.add)
            if bias_t is not None:
                nc.vector.tensor_tensor(out=x_tm[:, nt, :], in0=x_tm[:, nt, :],
                                        in1=bias_t[:], op=AL.add)
            st = small.tile([128, 6], F32, tag="st", name=f"st{i}")
            nc.vector.bn_stats(out=st[:], in_=x_tm[:, nt, :])
            nc.vector.bn_aggr(out=mvb[ph][:, :, nt], in_=st[:])

        def ln_finalize(ph, tiles, s_tile, b_tile, lbl):
            mv = mvb[ph]
            sdb = small.tile([128, NTT], F32, tag="sdb", name=f"sdb{lbl}")
            nc.scalar.activation(out=sdb[:], in_=mv[:, 1, :], func=AF.Sqrt,
                                 bias=eps_t[:], scale=1.0)
            rsb = small.tile([128, NTT], F32, tag="rsb", name=f"rsb{lbl}")
            nc.vector.reciprocal(out=rsb[:], in_=sdb[:])
            nmb = small.tile([128, NTT], F32, tag="nmb", name=f"nmb{lbl}")
            nc.vector.tensor_tensor(out=nmb[:], in0=mv[:, 0, :], in1=rsb[:],
                                    op=AL.mult)
            nc.vector.tensor_scalar_mul(out=nmb[:], in0=nmb[:], scalar1=-1.0)
            for nt in tiles:
                nc.vector.tensor_scalar(
                    out=x_tm[:, nt, :], in0=x_tm[:, nt, :],
                    scalar1=rsb[:, nt:nt + 1], scalar2=nmb[:, nt:nt + 1],
                    op0=AL.mult, op1=AL.add)
                if s_tile is not None:
                    nc.vector.tensor_tensor(out=x_tm[:, nt, :],
                                            in0=x_tm[:, nt, :],
                                            in1=s_tile[:], op=AL.mult)
                if b_tile is not None:
                    nc.vector.tensor_tensor(out=x_tm[:, nt, :],
                                            in0=x_tm[:, nt, :],
                                            in1=b_tile[:], op=AL.add)
                nc.sync.dma_start_transpose(
                    xTa[:, :, nt * 128:(nt + 1) * 128], x_tm[:, nt, :])

        def wo_stats(nt, w):
            ps = ps_mm.tile([128, 512], F32, tag="mm")
            for dt in range(ND):
                nc.tensor.matmul(ps[:], ctxT[:, dt, nt * 128:(nt + 1) * 128],
                                 w["wo"][:, dt, :], start=dt == 0,
                                 stop=dt == ND - 1)
            res_stats(nt, ps, None if fl["bo"] else w["bo"], "ln1", i=nt % 4)

        def ffn_chunk(l, c0, c1, w):
            Wc = c1 - c0
            hT = htp.tile([128, NFT, 512], BF16, tag="hT", name=f"hT{l}_{c0}")
            for ft in range(NFT):
                ps = ps_mm.tile([128, 512], F32, tag="mm")
                for dt in range(ND):
                    nc.tensor.matmul(
                        ps[:, 0:Wc], w["wi"][:, dt, ft * 128:(ft + 1) * 128],
                        xTa[:, dt, c0:c1], start=dt == 0, stop=dt == ND - 1)
                nc.scalar.activation(
                    out=hT[:, ft, 0:Wc], in_=ps[:, 0:Wc], func=AF.Gelu,
                    bias=0.0 if fl["bi"] else w["bi"][:, ft:ft + 1], scale=1.0)
            for nt in range(c0 // 128, c1 // 128):
                toff = nt * 128 - c0
                ps = ps_mm.tile([128, 512], F32, tag="mm")
                for ft in range(NFT):
                    nc.tensor.matmul(ps[:], hT[:, ft, toff:toff + 128],
                                     w["wo2"][:, ft, :], start=ft == 0,
                                     stop=ft == NFT - 1)
                res_stats(nt, ps, None if fl["bo2"] else w["bo2"], "ln2",
                          i=nt % 4)

        # last slot writing each ctx tile (for flush scheduling)
        last_slot_of = {}
        for j in range(NSL):
            for (nt, b, kw, rel) in chunks[j]:
                last_slot_of[nt] = j

        NCH = TP // 512 + (1 if TP % 512 else 0)
        chunk_rng = [(ci * 512, min((ci + 1) * 512, TP)) for ci in range(NCH)]

        # tiles first touched by each qk group (for embed/V scheduling)
        emb_done = set()

        def new_tiles(grp):
            g0, g1 = off[grp[0]], off[grp[-1] + 1]
            ts = [t for t in range(g0 // 128, -(-g1 // 128)) if t not in emb_done]
            emb_done.update(ts)
            return ts

        # ---- program ----
        # All embeds first (gathers + posty DMAs ahead of the big weight
        # DMAs in the queues), in group order so group 0 finishes first.
        for grp in lay["qk_groups"]:
            for nt in new_tiles(grp):
                embed(nt)
        for nt in range(TP // 128):
            if nt not in emb_done:
                emb_done.add(nt)
                embed(nt)
        w_cur = load_layer_weights(0, part="qk")
        load_layer_weights(0, w=w_cur, part="rest")
        nc.vector.memset(ctxT[:], 0.0)
        for l in range(NL):
            ctxb_map = {}
            v_done = set()
            wo_done = set()

            def flush(j):
                # Wo matmuls + residual/stats (no ACT) for tiles whose
                # attention is complete — dense PE filler between the
                # ACT-paced softmax chains of consecutive slots.
                for nt in range(NTT):
                    if nt in wo_done or last_slot_of.get(nt, -1) > j:
                        continue
                    if nt not in ctxb_map and nt in last_slot_of:
                        continue  # not yet computed this pass
                    wo_done.add(nt)
                    wo_stats(nt, w_cur)

            for gi, grp in enumerate(lay["qk_groups"]):
                qkT = qk_group(gi, grp, w_cur)
                for j in grp:
                    for (nt, b, kw, rel) in chunks[j]:
                        if nt not in v_done:
                            v_done.add(nt)
                            v_tile(nt, w_cur)
                    attn_slot(l, j, qkT, off[grp[0]], ctxb_map, last_slot_of)
                    if j >= 1:
                        flush(j - 1)
                if gi == 0 and l + 1 < NL:
                    w_nxt = load_layer_weights(l + 1, part="qk")
            if l + 1 < NL:
                load_layer_weights(l + 1, w=w_nxt, part="rest")
            flush(NSL)
            ln_finalize("ln1", range(NTT),
                        None if fl["ln"] else w_cur["ln1s"],
                        None if fl["ln"] else w_cur["ln1b"], f"a{l}")
            for ci, (c0, c1) in enumerate(chunk_rng):
                ffn_chunk(l, c0, c1, w_cur)
            ln_finalize("ln2", range(NTT),
                        None if fl["ln"] else w_cur["ln2s"],
                        None if fl["ln"] else w_cur["ln2b"], f"f{l}")
            if l + 1 < NL:
                w_cur = w_nxt

        if debug:
            for nt in range(NTT):
                dx = work.tile([128, D], F32, tag="dbg", name=f"dbg{nt}")
                nc.vector.tensor_copy(out=dx[:], in_=x_tm[:, nt, :])
                nc.sync.dma_start(out=dbgx_d[nt], in_=dx[:])

        # ---- conv head ----
        # xcv reuses ctxT's slot (attention is done), pen/cvt reuse the
        # embed-phase work slots — keeps peak SBUF under the cap
        xcv = state.tile([128, ND, CWP], BF16, tag="ctxT", name="xcv")
        nc.vector.memset(xcv[:], 0.0)
        for j in range(NSL):
            o0, c0 = off[j], coff[j]
            for dt in range(ND):
                nc.vector.tensor_tensor(
                    out=xcv[:, dt, c0:c0 + lam[j]],
                    in0=xTa[:, dt, o0:o0 + lam[j]],
                    in1=cm_sb[:, c0:c0 + lam[j]], op=AL.mult)
        for cgi, cg in enumerate(lay["cv_groups"]):
            cs, ce = coff[cg[0]], coff[cg[-1] + 1]
            Wg = ce - cs
            for ki, k in enumerate((1, 2, 3)):
                pen = work.tile([128, 512], F32, tag="g", name=f"pn{cgi}_{ki}")
                nc.sync.dma_start(
                    out=pen[:, 0:Wg],
                    in_=cpen_d[ki, cs:ce][None, :].to_broadcast([128, Wg]))
                for ft in range(2):
                    ps = ps_mm.tile([128, 512], F32, tag="mm")
                    idx = 0
                    for dt in range(ND):
                        for jj in range(k):
                            nc.tensor.matmul(
                                ps[:, 0:Wg],
                                cw[(k, jj)][:, dt, ft * 128:(ft + 1) * 128],
                                xcv[:, dt, cs + jj:cs + jj + Wg],
                                start=idx == 0, stop=idx == ND * k - 1)
                            idx += 1
                    cvt = work.tile([128, 512], F32, tag="pt",
                                    name=f"cv{cgi}_{ki}_{ft}")
                    nc.vector.tensor_tensor(out=cvt[:, 0:Wg], in0=ps[:, 0:Wg],
                                            in1=pen[:, 0:Wg], op=AL.add)
                    for j in cg:
                        rs = coff[j] - cs
                        re = rs + lam[j] - k + 2
                        nc.vector.tensor_reduce(
                            out=rep[:, ki * 2 + ft, j:j + 1], in_=cvt[:, rs:re],
                            axis=mybir.AxisListType.X, op=AL.max)

        if not fl["cb"]:
            for ki in range(3):
                for ft in range(2):
                    nc.vector.tensor_scalar_add(
                        out=rep[:, ki * 2 + ft, :], in0=rep[:, ki * 2 + ft, :],
                        scalar1=cb[:, ki, ft:ft + 1])
        nc.scalar.activation(out=rep[:], in_=rep[:], func=AF.Relu)

        fps = ps_c.tile([128, 3 * NCLS], F32, tag="c")
        for c in range(6):
            nc.tensor.matmul(fps[:NSL, :], rep[:, c, :], fcw_sb[:, c, :],
                             start=c == 0, stop=c == 5)
        ob = small.tile([NSL, 3 * NCLS], F32, tag="ob")
        nc.scalar.copy(out=ob[:], in_=fps[:NSL, :])
        nc.sync.dma_start(out=out_d[:], in_=ob[:])

    nc.compile()
    return nc


def _core_inputs(inputs, fl, lay):
    """Build the 8 per-core input maps from the full problem inputs."""
    f32 = lambda a: np.ascontiguousarray(np.asarray(a, dtype=np.float32))
    tile_w = lambda w: np.ascontiguousarray(
        f32(w).reshape(w.shape[0] // 128, 128, w.shape[1]).astype(BF))

    NTT, TP, CWP = lay["NTT"], lay["TP"], lay["CWP"]
    lam, lam32, off, coff = lay["lam"], lay["lam32"], lay["off"], lay["coff"]
    assign = lay["assign"]

    shared = {}
    # packed position+type embedding
    posv = np.zeros((TP, D), np.float32)
    pe = f32(inputs["pos_emb"])
    for j in range(NSL):
        posv[off[j]:off[j] + lam32[j]] = pe[:lam32[j]]
    posv += f32(inputs["type_emb"][0])[None, :]
    shared["posty"] = np.ascontiguousarray(posv.reshape(NTT, 128, D))
    for l in range(NL):
        shared[f"wq{l}"] = tile_w(f32(inputs["Wq"][l]) / 8.0)
        shared[f"wk{l}"] = tile_w(inputs["Wk"][l])
        shared[f"wv{l}"] = tile_w(inputs["Wv"][l])
        shared[f"wo{l}"] = tile_w(inputs["Wo"][l])
        shared[f"wi{l}"] = tile_w(inputs["Wi"][l])
        shared[f"wo2{l}"] = tile_w(inputs["Wo2"][l])
        if not fl["bqk"]:
            shared[f"bq{l}"] = f32(inputs["bq"][l]).reshape(ND, 128) / 8.0
            shared[f"bk{l}"] = f32(inputs["bk"][l]).reshape(ND, 128)
        if not fl["bv"]:
            shared[f"bv{l}"] = f32(inputs["bv"][l])
        if not fl["bo"]:
            shared[f"bo{l}"] = f32(inputs["bo"][l])
        if not fl["bi"]:
            shared[f"bi{l}"] = f32(inputs["bi"][l]).reshape(NFT, 128)
        if not fl["bo2"]:
            shared[f"bo2{l}"] = f32(inputs["bo2"][l])
        if not fl["ln"]:
            shared[f"ln1s{l}"] = f32(inputs["ln1_s"][l])
            shared[f"ln1b{l}"] = f32(inputs["ln1_b"][l])
            shared[f"ln2s{l}"] = f32(inputs["ln2_s"][l])
            shared[f"ln2b{l}"] = f32(inputs["ln2_b"][l])
    if not fl["ln"]:
        shared["lnes"] = f32(inputs["emb_ln_s"])
        shared["lneb"] = f32(inputs["emb_ln_b"])
    for ki, k in enumerate((1, 2, 3)):
        w = f32(inputs[f"conv_w{k}"])                    # [NF, k, D]
        wt = np.ascontiguousarray(w.transpose(1, 2, 0))  # [k, D, NF]
        shared[f"cw{k}"] = np.ascontiguousarray(
            wt.reshape(k, ND, 128, NF).astype(BF))
    if not fl["cb"]:
        shared["convb"] = np.stack(
            [f32(inputs[f"conv_b{k}"]).reshape(2, 128) for k in (1, 2, 3)])
    # fc weights for all 3 branch-block hypotheses: [6, 128, 3*NCLS]
    fcw = f32(inputs["fc_w"])                            # [2304, NCLS]
    fcw3 = np.zeros((6, 128, 3 * NCLS), np.float32)
    for bb in range(3):
        for ki in range(3):
            for ft in range(2):
                c = ki * 2 + ft
                rows = 768 * bb + 256 * ki + 128 * ft
                fcw3[c, :, bb * NCLS:(bb + 1) * NCLS] = fcw[rows:rows + 128]
    shared["fcw"] = fcw3
    shared["word_emb"] = f32(inputs["word_emb"])

    ids_all = np.stack([np.asarray(inputs[p + "_input_ids"])
                        for p in ("q", "a", "b")]).reshape(96, S)
    mask_all = np.stack([np.asarray(inputs[p + "_attention_mask"])
                         for p in ("q", "a", "b")]).reshape(96, S)
    lens_all = mask_all.sum(1).astype(int)

    in_maps = []
    for c in range(NCORES):
        m = dict(shared)
        idv = np.zeros(TP, np.int32)
        vmv = np.zeros(TP, np.float32)
        cmv = np.zeros(CWP, np.float32)
        pen = np.full((3, CWP), -1e30, np.float32)
        for j in range(NSL):
            sq = int(assign[j, c])
            l = int(lens_all[sq])
            idv[off[j]:off[j] + lam32[j]] = ids_all[sq][:lam32[j]]
            vmv[off[j]:off[j] + l] = 1.0
            cmv[coff[j]:coff[j] + l] = 1.0
            for ki, k in enumerate((1, 2, 3)):
                nw = l - k + 2
                pen[ki, coff[j]:coff[j] + nw] = 0.0
        m["ids"] = np.ascontiguousarray(idv.reshape(NTT, 128))
        m["vmask"] = np.ascontiguousarray(vmv.reshape(NTT, 128))
        m["convmask"] = np.ascontiguousarray(cmv.astype(BF))
        m["convpen"] = np.ascontiguousarray(pen)
        in_maps.append(m)
    return in_maps


def _get_program(fl, lay, debug=False):
    key = (tuple(sorted(fl.items())), lay["key"], debug)
    if key not in _CACHE:
        _CACHE[key] = _build_program(fl, lay, debug=debug)
    return _CACHE[key]


def run_sharded(inputs, debug=False, **run_kwargs):
    """Shard, run on 8 cores, gather. Returns (output, BassKernelResults)."""
    from concourse.bass_utils import run_bass_kernel_spmd
    fl = _flags(inputs)
    lens96 = np.concatenate([
        np.asarray(inputs[p + "_attention_mask"]).sum(1) for p in ("q", "a", "b")])
    lay = _layout(lens96)
    nc = _get_program(fl, lay, debug=debug)
    in_maps = _core_inputs(inputs, fl, lay)
    res = run_bass_kernel_spmd(nc, in_maps, core_ids=list(range(NCORES)),
                               **run_kwargs)
    border = {0: 0, 1: 2, 2: 1}   # branch q/a/b -> fc block q,b,a
    out = np.zeros((B, NCLS), np.float32)
    for c in range(NCORES):
        o3 = np.asarray(res.results[c]["out"], np.float32)   # [NSL, 12]
        for j in range(NSL):
            sq = int(lay["assign"][j, c])
            br, sample = sq // B, sq % B
            out[sample] += o3[j, border[br] * NCLS:(border[br] + 1) * NCLS]
    out += np.asarray(inputs["fc_b"], np.float32)[None, :]
    return out, res


def kernel(**inputs):
    out, _ = run_sharded(inputs)
    return out


# revision 31
# speedup vs baseline: 1.7929x; 1.0513x over previous
"""Trainium2 Bass kernel for nn_BertCNN (3x BERT-small encoder + CNN maxpool head).

Ragged-packed data-parallel strategy. The 96 sequences (3 branches x 32
samples) are sorted by actual length (from the attention mask), dealt
round-robin into 8 cores x 12 slots, and each core packs its 12 sequences
into one ~1900-token stream (slot budgets = max length in each rank group,
32-aligned starts). All encoder linear ops (QKV/O/FFN/conv) run over the
packed stream; attention runs per-slot with exact budget widths; key
validity is folded multiplicatively into V (invalid keys get zero V rows
and a zero softmax-denominator contribution), so no attention bias is
needed. The conv head runs over a separately packed layout with 2-token
gaps; per-slot maxpool ranges and window-validity penalties come from the
host. The fc output is computed for all 3 branch hypotheses per slot
([12, 12] per core) and the host scatter-adds the right 4 columns into the
final [32, 4].

The Bass program depends only on the slot-budget layout (not on per-core
data); it is built once per layout signature and cached.
"""

import numpy as np
import ml_dtypes

V, D, H, DH, NL, FF = 30522, 512, 8, 64, 4, 2048
NF, NCLS, B, S = 256, 4, 32, 256
NCORES = 8
NSL = 12                 # slots (sequences) per core
NSEQ = NSL               # test.py compat
SPC = 4                  # test.py compat
ND = D // 128
NFT = FF // 128
DH1 = DH + 1

BF = ml_dtypes.bfloat16
_CACHE = {}


def _flags(inputs):
    z = lambda a: bool(np.all(np.asarray(a) == 0))
    o = lambda a: bool(np.all(np.asarray(a) == 1))
    return {
        "bqk": z(inputs["bq"]) and z(inputs["bk"]),
        "bv": z(inputs["bv"]),
        "bo": z(inputs["bo"]),
        "bi": z(inputs["bi"]),
        "bo2": z(inputs["bo2"]),
        "ln": all(o(inputs[k]) for k in ("emb_ln_s", "ln1_s", "ln2_s"))
        and all(z(inputs[k]) for k in ("emb_ln_b", "ln1_b", "ln2_b")),
        "cb": z(inputs["conv_b1"]) and z(inputs["conv_b2"]) and z(inputs["conv_b3"]),
    }


def _layout(lens96):
    """Pack layout shared by all cores (program-shaping constants)."""
    lens96 = np.asarray(lens96, dtype=np.int64)
    order = np.argsort(-lens96, kind="stable")
    assign = order.reshape(NSL, NCORES)           # [slot, core] -> seq idx
    lam = lens96[assign].max(1).astype(int)       # slot budgets (max len)
    # 64-aligned slot starts (PE col/row tiling only supports base 0/64 for
    # >32-wide tiles); bump a start to the next 128 boundary when the slot
    # would otherwise span 3 token tiles (score/eT tiles hold 2 chunks)
    lam32 = ((lam + 63) // 64) * 64
    off = np.zeros(NSL + 1, np.int64)
    for j in range(NSL):
        o = off[j]
        if (o % 128) + lam[j] > 256:
            o = ((o + 127) // 128) * 128
            off[j] = o
        off[j + 1] = o + lam32[j]
    T32 = int(off[-1])
    NTT = -(-T32 // 128)
    TP = NTT * 128
    coff = np.zeros(NSL + 1, np.int64)
    coff[1:] = np.cumsum(lam + 2)
    CW = int(coff[-1])

    def greedy(offs, cap):
        groups, cur = [], [0]
        for j in range(1, NSL):
            if offs[j + 1] - offs[cur[0]] <= cap:
                cur.append(j)
            else:
                groups.append(cur)
                cur = [j]
        groups.append(cur)
        return groups

    qk_groups = greedy(off, 512)
    cv_groups = greedy(coff, 512)

    # per-slot key/query chunks: intersections with the global 128 grid
    chunks = []
    for j in range(NSL):
        lo, hi = int(off[j]), int(off[j] + lam[j])
        ch = []
        p = lo
        while p < hi:
            nt = p // 128
            e = min(hi, (nt + 1) * 128)
            ch.append((nt, p - nt * 128, e - p, p - lo))  # (tile, base, width, rel)
            p = e
        chunks.append(ch)
    return dict(
        assign=assign, lam=[int(x) for x in lam], lam32=[int(x) for x in lam32],
        off=[int(x) for x in off], coff=[int(x) for x in coff],
        T32=T32, NTT=NTT, TP=TP, CW=CW, CWP=CW + 2,
        qk_groups=qk_groups, cv_groups=cv_groups, chunks=chunks,
        key=(tuple(int(x) for x in lam), tuple(int(x) for x in lam32)),
    )


def _build_program(fl, lay, debug=False):
    import contextlib
    import concourse.bass as bass
    import concourse.mybir as mybir
    import concourse.tile as tile
    from concourse import bacc

    F32, BF16, I32 = mybir.dt.float32, mybir.dt.bfloat16, mybir.dt.int32
    AL, AF = mybir.AluOpType, mybir.ActivationFunctionType

    NTT, TP, CWP = lay["NTT"], lay["TP"], lay["CWP"]
    lam, lam32, off, coff = lay["lam"], lay["lam32"], lay["off"], lay["coff"]
    chunks = lay["chunks"]

    nc = bacc.Bacc("TRN2", target_bir_lowering=False, debug=False,
                   num_devices=NCORES)

    di = lambda n, s, d: nc.dram_tensor(n, s, d, kind="ExternalInput").ap()
    word = di("word_emb", [V, D], F32)
    ids_d = di("ids", [NTT, 128], I32)
    vm_d = di("vmask", [NTT, 128], F32)
    posty_d = di("posty", [NTT, 128, D], F32)
    cm_d = di("convmask", [CWP], BF16)
    cpen_d = di("convpen", [3, CWP], F32)
    wq_d = [di(f"wq{l}", [ND, 128, D], BF16) for l in range(NL)]
    wk_d = [di(f"wk{l}", [ND, 128, D], BF16) for l in range(NL)]
    wv_d = [di(f"wv{l}", [ND, 128, D], BF16) for l in range(NL)]
    wo_d = [di(f"wo{l}", [ND, 128, D], BF16) for l in range(NL)]
    wi_d = [di(f"wi{l}", [ND, 128, FF], BF16) for l in range(NL)]
    wo2_d = [di(f"wo2{l}", [NFT, 128, D], BF16) for l in range(NL)]
    cw_d = [di(f"cw{k}", [k, ND, 128, NF], BF16) for k in (1, 2, 3)]
    fcw_d = di("fcw", [6, 128, 3 * NCLS], F32)
    if not fl["bqk"]:
        bq_d = [di(f"bq{l}", [ND, 128], F32) for l in range(NL)]
        bk_d = [di(f"bk{l}", [ND, 128], F32) for l in range(NL)]
    if not fl["bv"]:
        bv_d = [di(f"bv{l}", [D], F32) for l in range(NL)]
    if not fl["bo"]:
        bo_d = [di(f"bo{l}", [D], F32) for l in range(NL)]
    if not fl["bi"]:
        bi_d = [di(f"bi{l}", [NFT, 128], F32) for l in range(NL)]
    if not fl["bo2"]:
        bo2_d = [di(f"bo2{l}", [D], F32) for l in range(NL)]
    if not fl["ln"]:
        elns_d = di("lnes", [D], F32)
        elnb_d = di("lneb", [D], F32)
        ln1s_d = [di(f"ln1s{l}", [D], F32) for l in range(NL)]
        ln1b_d = [di(f"ln1b{l}", [D], F32) for l in range(NL)]
        ln2s_d = [di(f"ln2s{l}", [D], F32) for l in range(NL)]
        ln2b_d = [di(f"ln2b{l}", [D], F32) for l in range(NL)]
    if not fl["cb"]:
        cb_d = di("convb", [3, 2, 128], F32)

    out_d = nc.dram_tensor("out", [NSL, 3 * NCLS], F32, kind="ExternalOutput").ap()
    if debug:
        dbgx_d = nc.dram_tensor("dbgx", [NTT, 128, D], F32,
                                kind="ExternalOutput").ap()

    with tile.TileContext(nc) as tc, contextlib.ExitStack() as ctx:
        consts = ctx.enter_context(tc.tile_pool(name="consts", bufs=1))
        state = ctx.enter_context(tc.tile_pool(name="state", bufs=1))
        wts = ctx.enter_context(tc.tile_pool(name="wts", bufs=1))
        qkp = ctx.enter_context(tc.tile_pool(name="qkp", bufs=2))
        etp = ctx.enter_context(tc.tile_pool(name="etp", bufs=2))
        htp = ctx.enter_context(tc.tile_pool(name="htp", bufs=1))
        work = ctx.enter_context(tc.tile_pool(name="work", bufs=2))
        cxp = ctx.enter_context(tc.tile_pool(name="cxp", bufs=3))
        small = ctx.enter_context(tc.tile_pool(name="small", bufs=4))
        ps_mm = ctx.enter_context(tc.tile_pool(name="ps_mm", bufs=3, space="PSUM"))
        ps_s = ctx.enter_context(tc.tile_pool(name="ps_s", bufs=3, space="PSUM"))
        ps_c = ctx.enter_context(tc.tile_pool(name="ps_c", bufs=2, space="PSUM"))

        # ---- constants ----
        eps_t = consts.tile([128, 1], F32, tag="eps")
        nc.vector.memset(eps_t[:], 1e-12)
        ones8 = consts.tile([128, H], BF16, tag="ones8")
        nc.vector.memset(ones8[:], 1.0)
        ids_sb = consts.tile([128, NTT], I32, tag="ids")
        nc.sync.dma_start(out=ids_sb[:], in_=ids_d.rearrange("t p -> p t"))
        vm_sb = consts.tile([128, NTT], F32, tag="vm")
        nc.sync.dma_start(out=vm_sb[:], in_=vm_d.rearrange("t p -> p t"))
        cm_sb = consts.tile([128, CWP], BF16, tag="cm")
        nc.sync.dma_start(out=cm_sb[:],
                          in_=cm_d[None, :].to_broadcast([128, CWP]))
        fcw_sb = consts.tile([128, 6, 3 * NCLS], F32, tag="fcw")
        nc.sync.dma_start(out=fcw_sb[:], in_=fcw_d.rearrange("c p n -> p c n"))
        bcast = lambda ap, n: ap[None, :].to_broadcast([128, n])
        if not fl["ln"]:
            elns = consts.tile([128, D], F32, tag="elns")
            nc.sync.dma_start(out=elns[:], in_=bcast(elns_d, D))
            elnb = consts.tile([128, D], F32, tag="elnb")
            nc.sync.dma_start(out=elnb[:], in_=bcast(elnb_d, D))
        if not fl["cb"]:
            cb = consts.tile([128, 3, 2], F32, tag="cb")
            nc.sync.dma_start(out=cb[:], in_=cb_d.rearrange("k t p -> p k t"))

        cw = {}
        for ki, k in enumerate((1, 2, 3)):
            for j in range(k):
                t = wts.tile([128, ND, NF], BF16, tag=f"cw{ki}_{j}")
                nc.sync.dma_start(out=t[:],
                                  in_=cw_d[ki][j].rearrange("t p f -> p t f"))
                cw[(k, j)] = t

        # ---- persistent state ----
        x_tm = state.tile([128, NTT, D], BF16, tag="x_tm")
        xTa = state.tile([128, ND, TP], BF16, tag="xTa")
        ctxT = state.tile([128, ND, TP], BF16, tag="ctxT")
        vA = state.tile([128, NTT, H, DH1], BF16, tag="vA")
        rep = state.tile([128, 6, NSL], F32, tag="rep")

        def load_layer_weights(l, w=None, part="all"):
            if w is None:
                w = {}
            names = {"qk": ("wq", "wk"), "rest": ("wv", "wo", "wi"),
                     "all": ("wq", "wk", "wv", "wo", "wi")}[part]
            for nm, dd, nfree in (("wq", wq_d, D), ("wk", wk_d, D),
                                  ("wv", wv_d, D), ("wo", wo_d, D),
                                  ("wi", wi_d, FF)):
                if nm not in names:
                    continue
                w[nm] = wts.tile([128, ND, nfree], BF16, tag=nm, name=f"{nm}_{l}",
                                 bufs=2 if nm in ("wq", "wk") else 1)
                for dt in range(ND):
                    nc.sync.dma_start(out=w[nm][:, dt, :], in_=dd[l][dt])
            if part == "qk":
                return w
            w["wo2"] = wts.tile([128, NFT, D], BF16, tag="wo2", name=f"wo2_{l}")
            for ft in range(0, NFT, 4):
                nc.sync.dma_start(
                    out=w["wo2"][:, ft:ft + 4, :],
                    in_=wo2_d[l][ft:ft + 4].rearrange("t p o -> p t o"))
            if not fl["bqk"]:
                w["bq"] = consts.tile([128, ND], F32, tag="bq", name=f"bq_{l}")
                nc.sync.dma_start(out=w["bq"][:], in_=bq_d[l].rearrange("t p -> p t"))
                w["bk"] = consts.tile([128, ND], F32, tag="bk", name=f"bk_{l}")
                nc.sync.dma_start(out=w["bk"][:], in_=bk_d[l].rearrange("t p -> p t"))
            if not fl["bv"]:
                w["bv"] = consts.tile([128, D], F32, tag="bv", name=f"bv_{l}")
                nc.sync.dma_start(out=w["bv"][:], in_=bcast(bv_d[l], D))
            if not fl["bo"]:
                w["bo"] = consts.tile([128, D], F32, tag="bo", name=f"bo_{l}")
                nc.sync.dma_start(out=w["bo"][:], in_=bcast(bo_d[l], D))
            if not fl["bi"]:
                w["bi"] = consts.tile([128, NFT], F32, tag="bi", name=f"bi_{l}")
                nc.sync.dma_start(out=w["bi"][:], in_=bi_d[l].rearrange("t p -> p t"))
            if not fl["bo2"]:
                w["bo2"] = consts.tile([128, D], F32, tag="bo2", name=f"bo2_{l}")
                nc.sync.dma_start(out=w["bo2"][:], in_=bcast(bo2_d[l], D))
            if not fl["ln"]:
                for nm, dd in (("ln1s", ln1s_d), ("ln1b", ln1b_d),
                               ("ln2s", ln2s_d), ("ln2b", ln2b_d)):
                    w[nm] = consts.tile([128, D], F32, tag=nm, name=f"{nm}_{l}")
                    nc.sync.dma_start(out=w[nm][:], in_=bcast(dd[l], D))
            return w

        def ln_start(src_ap, i=0):
            st = small.tile([128, 6], F32, tag="st", name=f"st{i}")
            mv = small.tile([128, 2], F32, tag="mv", name=f"mv{i}")
            nc.vector.bn_stats(out=st[:], in_=src_ap)
            nc.vector.bn_aggr(out=mv[:], in_=st[:])
            sd = small.tile([128, 1], F32, tag="sd", name=f"sd{i}")
            nc.scalar.activation(out=sd[:], in_=mv[:, 1:2], func=AF.Sqrt,
                                 bias=eps_t[:], scale=1.0)
            nmr = small.tile([128, 2], F32, tag="nmr", name=f"nmr{i}")
            nc.vector.reciprocal(out=nmr[:, 1:2], in_=sd[:])
            nc.vector.tensor_scalar(out=nmr[:, 0:1], in0=mv[:, 0:1],
                                    scalar1=nmr[:, 1:2], scalar2=-1.0,
                                    op0=AL.mult, op1=AL.mult)
            return nmr

        def ln_apply(src_ap, dst_ap, nmr, s_tile, b_tile):
            nc.vector.tensor_scalar(out=dst_ap, in0=src_ap,
                                    scalar1=nmr[:, 1:2], scalar2=nmr[:, 0:1],
                                    op0=AL.mult, op1=AL.add)
            if s_tile is not None:
                nc.vector.tensor_tensor(out=dst_ap, in0=dst_ap, in1=s_tile[:],
                                        op=AL.mult)
            if b_tile is not None:
                nc.vector.tensor_tensor(out=dst_ap, in0=dst_ap, in1=b_tile[:],
                                        op=AL.add)

        def embed(nt):
            g = work.tile([128, D], F32, tag="g", name=f"g{nt}")
            nc.gpsimd.indirect_dma_start(
                out=g[:], out_offset=None, in_=word[:],
                in_offset=bass.IndirectOffsetOnAxis(
                    ap=ids_sb[:, nt:nt + 1], axis=0))
            pt = work.tile([128, D], F32, tag="pt", name=f"pt{nt}")
            nc.sync.dma_start(out=pt[:], in_=posty_d[nt])
            nc.vector.tensor_tensor(out=g[:], in0=g[:], in1=pt[:], op=AL.add)
            dst = x_tm[:, nt, :]
            ln_apply(g[:], dst, ln_start(g[:], i=nt % 4),
                     None if fl["ln"] else elns, None if fl["ln"] else elnb)
            nc.sync.dma_start_transpose(xTa[:, :, nt * 128:(nt + 1) * 128], dst)

        def qk_group(gi, grp, w):
            g0, g1 = off[grp[0]], off[grp[-1] + 1]
            W = g1 - g0
            qkT = qkp.tile([128, 2, ND, 512], BF16, tag="qkT", name=f"qkT{gi}")
            for qi, wt, bt in ((0, w["wq"], "bq"), (1, w["wk"], "bk")):
                for ot in range(ND):
                    ps = ps_mm.tile([128, 512], F32, tag="mm")
                    for dt in range(ND):
                        nc.tensor.matmul(
                            ps[:, 0:W], wt[:, dt, ot * 128:(ot + 1) * 128],
                            xTa[:, dt, g0:g1], start=dt == 0, stop=dt == ND - 1)
                    if fl["bqk"]:
                        if qi == 1:
                            nc.scalar.copy(out=qkT[:, qi, ot, 0:W],
                                           in_=ps[:, 0:W])
                        else:
                            nc.vector.tensor_copy(out=qkT[:, qi, ot, 0:W],
                                                  in_=ps[:, 0:W])
                    else:
                        nc.scalar.activation(
                            out=qkT[:, qi, ot, 0:W], in_=ps[:, 0:W],
                            func=AF.Identity, bias=w[bt][:, ot:ot + 1], scale=1.0)
            return qkT

        def v_tile(nt, w):
            ps = ps_mm.tile([128, 512], F32, tag="mm")
            for dt in range(ND):
                nc.tensor.matmul(ps[:], xTa[:, dt, nt * 128:(nt + 1) * 128],
                                 w["wv"][:, dt, :], start=dt == 0,
                                 stop=dt == ND - 1)
            if fl["bv"]:
                nc.vector.tensor_scalar_mul(
                    out=vA[:, nt, :, 0:DH],
                    in0=ps.rearrange("p (h d) -> p h d", h=H),
                    scalar1=vm_sb[:, nt:nt + 1])
            else:
                nc.vector.tensor_tensor(
                    out=vA[:, nt, :, 0:DH],
                    in0=ps.rearrange("p (h d) -> p h d", h=H),
                    in1=w["bv"].rearrange("p (h d) -> p h d", h=H), op=AL.add)
                nc.vector.tensor_scalar_mul(
                    out=vA[:, nt, :, 0:DH], in0=vA[:, nt, :, 0:DH],
                    scalar1=vm_sb[:, nt:nt + 1])
            nc.vector.tensor_scalar_mul(
                out=vA[:, nt, :, DH], in0=ones8[:],
                scalar1=vm_sb[:, nt:nt + 1])

        def attn_slot(l, j, qkT, g0, ctxb_map, last_slot_of):
            L = lam[j]
            ch = chunks[j]
            q0 = off[j] - g0
            eT = etp.tile([128, H, 512], BF16, tag="eT", name=f"eT{l}_{j}")
            for h in range(H):
                hh, dtH = (h % 2) * DH, h // 2
                pss = ps_s.tile([128, 512], F32, tag="s")
                for ci, (nt, b, kw, rel) in enumerate(ch):
                    nc.tensor.matmul(
                        pss[b:b + kw, ci * L:ci * L + L],
                        qkT[hh:hh + DH, 1, dtH, q0 + rel:q0 + rel + kw],
                        qkT[hh:hh + DH, 0, dtH, q0:q0 + L],
                        start=True, stop=True)
                nc.scalar.activation(out=eT[:, h, 0:len(ch) * L],
                                     in_=pss[:, 0:len(ch) * L], func=AF.Exp,
                                     bias=0.0, scale=1.0)
            # ctx per query chunk
            for (qnt, qb, qw, qrel) in ch:
                for hg in range(2):
                    cps = ps_c.tile([128, 4 * DH1], F32, tag="c")
                    for hi in range(4):
                        h = hg * 4 + hi
                        sl = slice(hi * DH1, hi * DH1 + DH1)
                        for ci, (nt, b, kw, rel) in enumerate(ch):
                            nc.tensor.matmul(
                                cps[qb:qb + qw, sl],
                                eT[b:b + kw, h, ci * L + qrel:ci * L + qrel + qw],
                                vA[b:b + kw, nt, h, :],
                                start=ci == 0, stop=ci == len(ch) - 1)
                    if qnt not in ctxb_map:
                        ctxb_map[qnt] = cxp.tile([128, D], BF16, tag="ctxb",
                                                 name=f"cb{l}_{qnt}")
                    ctxb = ctxb_map[qnt]
                    rcp = small.tile([128, 4], F32, tag="rcp")
                    nc.vector.reciprocal(
                        out=rcp[qb:qb + qw, :],
                        in_=cps.rearrange("p (h c) -> p h c", c=DH1)[qb:qb + qw, :, DH])
                    nc.vector.tensor_tensor(
                        out=ctxb.rearrange("p (h d) -> p h d", d=DH)[
                            qb:qb + qw, hg * 4:hg * 4 + 4, :],
                        in0=cps.rearrange("p (h c) -> p h c", c=DH1)[qb:qb + qw, :, 0:DH],
                        in1=rcp[qb:qb + qw, :, None].to_broadcast([qw, 4, DH]),
                        op=AL.mult)
            # flush finished ctxb tiles
            for (qnt, qb, qw, qrel) in ch:
                if last_slot_of.get(qnt) == j:
                    nc.sync.dma_start_transpose(
                        ctxT[:, :, qnt * 128:(qnt + 1) * 128], ctxb_map[qnt][:])

        # deferred-LN machinery: residual-add lands pre-LN values in x_tm
        # (bf16, in place); per-tile bn stats collect into a batch buffer;
        # one Sqrt region per phase finalizes all tiles (ACT table stays
        # resident for Exp/Gelu — each table swap costs ~1.5us)
        mvb = {}
        for ph in ("ln1", "ln2"):
            mvb[ph] = state.tile([128, 2, NTT], F32, tag=f"mvb_{ph}",
                                 name=f"mvb_{ph}")

        def res_stats(nt, ps, bias_t, ph, i=0):
            nc.vector.tensor_tensor(out=x_tm[:, nt, :], in0=ps[:],
                                    in1=x_tm[:, nt, :], op=AL.add)
            if bias_t is not None:
                nc.vector.tensor_tensor(out=x_tm[:, nt, :], in0=x_tm[:, nt, :],
                                        in1=bias_t[:], op=AL.add)
            st = small.tile([128, 6], F32, tag="st", name=f"st{i}")
            nc.vector.bn_stats(out=st[:], in_=x_tm[:, nt, :])
            nc.vector.bn_aggr(out=mvb[ph][:, :, nt], in_=st[:])

        def ln_finalize(ph, tiles, s_tile, b_tile, lbl):
            """Batched LN finalize for `tiles`: one Sqrt region, DVE applies,
            transposes into xTa. Safe to call per-chunk (mini batches)."""
            tiles = [t for t in tiles]
            if not tiles:
                return
            mv = mvb[ph]
            t0, t1 = min(tiles), max(tiles) + 1
            sdb = small.tile([128, NTT], F32, tag="sdb", name=f"sdb{lbl}")
            nc.scalar.activation(out=sdb[:, t0:t1], in_=mv[:, 1, t0:t1],
                                 func=AF.Sqrt, bias=eps_t[:], scale=1.0)
            rsb = small.tile([128, NTT], F32, tag="rsb", name=f"rsb{lbl}")
            nc.vector.reciprocal(out=rsb[:, t0:t1], in_=sdb[:, t0:t1])
            nmb = small.tile([128, NTT], F32, tag="nmb", name=f"nmb{lbl}")
            nc.vector.tensor_tensor(out=nmb[:, t0:t1], in0=mv[:, 0, t0:t1],
                                    in1=rsb[:, t0:t1], op=AL.mult)
            nc.vector.tensor_scalar_mul(out=nmb[:, t0:t1], in0=nmb[:, t0:t1],
                                        scalar1=-1.0)
            for nt in tiles:
                nc.vector.tensor_scalar(
                    out=x_tm[:, nt, :], in0=x_tm[:, nt, :],
                    scalar1=rsb[:, nt:nt + 1], scalar2=nmb[:, nt:nt + 1],
                    op0=AL.mult, op1=AL.add)
                if s_tile is not None:
                    nc.vector.tensor_tensor(out=x_tm[:, nt, :],
                                            in0=x_tm[:, nt, :],
                                            in1=s_tile[:], op=AL.mult)
                if b_tile is not None:
                    nc.vector.tensor_tensor(out=x_tm[:, nt, :],
                                            in0=x_tm[:, nt, :],
                                            in1=b_tile[:], op=AL.add)
                nc.sync.dma_start_transpose(
                    xTa[:, :, nt * 128:(nt + 1) * 128], x_tm[:, nt, :])

        def wo_stats(nt, w):
            ps = ps_mm.tile([128, 512], F32, tag="mm")
            for dt in range(ND):
                nc.tensor.matmul(ps[:], ctxT[:, dt, nt * 128:(nt + 1) * 128],
                                 w["wo"][:, dt, :], start=dt == 0,
                                 stop=dt == ND - 1)
            res_stats(nt, ps, None if fl["bo"] else w["bo"], "ln1", i=nt % 4)

        def ffn_chunk(l, c0, c1, w):
            Wc = c1 - c0
            hT = htp.tile([128, NFT, 512], BF16, tag="hT", name=f"hT{l}_{c0}")
            for ft in range(NFT):
                ps = ps_mm.tile([128, 512], F32, tag="mm")
                for dt in range(ND):
                    nc.tensor.matmul(
                        ps[:, 0:Wc], w["wi"][:, dt, ft * 128:(ft + 1) * 128],
                        xTa[:, dt, c0:c1], start=dt == 0, stop=dt == ND - 1)
                nc.scalar.activation(
                    out=hT[:, ft, 0:Wc], in_=ps[:, 0:Wc], func=AF.Gelu,
                    bias=0.0 if fl["bi"] else w["bi"][:, ft:ft + 1], scale=1.0)
            for nt in range(c0 // 128, c1 // 128):
                toff = nt * 128 - c0
                ps = ps_mm.tile([128, 512], F32, tag="mm")
                for ft in range(NFT):
                    nc.tensor.matmul(ps[:], hT[:, ft, toff:toff + 128],
                                     w["wo2"][:, ft, :], start=ft == 0,
                                     stop=ft == NFT - 1)
                res_stats(nt, ps, None if fl["bo2"] else w["bo2"], "ln2",
                          i=nt % 4)

        # last slot writing each ctx tile (for flush scheduling)
        last_slot_of = {}
        for j in range(NSL):
            for (nt, b, kw, rel) in chunks[j]:
                last_slot_of[nt] = j

        # ---- conv head, interleaved with layer-3 FFN ----
        # xcv reuses ctxT's slot (attention is done by then); slot assembly
        # runs on GPSIMD (idle) as its tiles finish LN2; a conv group's
        # matmuls+maxpools run once all its slots are assembled.
        conv_st = {"xcv": None, "asm": set(), "grp": set()}

        def conv_ready(max_tile):
            if conv_st["xcv"] is None:
                conv_st["xcv"] = state.tile([128, ND, CWP], BF16, tag="ctxT",
                                            name="xcv")
                nc.gpsimd.memset(conv_st["xcv"][:], 0.0)
            xcv = conv_st["xcv"]
            for j in range(NSL):
                if j in conv_st["asm"]:
                    continue
                if max(nt for (nt, b, kw, rel) in chunks[j]) > max_tile:
                    continue
                conv_st["asm"].add(j)
                o0, c0 = off[j], coff[j]
                for dt in range(ND):
                    nc.gpsimd.tensor_tensor(
                        out=xcv[:, dt, c0:c0 + lam[j]],
                        in0=xTa[:, dt, o0:o0 + lam[j]],
                        in1=cm_sb[:, c0:c0 + lam[j]], op=AL.mult)
            for cgi, cg in enumerate(lay["cv_groups"]):
                if cgi in conv_st["grp"]:
                    continue
                if not all(j in conv_st["asm"] for j in cg):
                    continue
                conv_st["grp"].add(cgi)
                cs, ce = coff[cg[0]], coff[cg[-1] + 1]
                Wg = ce - cs
                for ki, k in enumerate((1, 2, 3)):
                    pen = work.tile([128, 512], F32, tag="g",
                                    name=f"pn{cgi}_{ki}")
                    nc.sync.dma_start(
                        out=pen[:, 0:Wg],
                        in_=cpen_d[ki, cs:ce][None, :].to_broadcast([128, Wg]))
                    for ft in range(2):
                        ps = ps_mm.tile([128, 512], F32, tag="mm")
                        idx = 0
                        for dt in range(ND):
                            for jj in range(k):
                                nc.tensor.matmul(
                                    ps[:, 0:Wg],
                                    cw[(k, jj)][:, dt, ft * 128:(ft + 1) * 128],
                                    xcv[:, dt, cs + jj:cs + jj + Wg],
                                    start=idx == 0, stop=idx == ND * k - 1)
                                idx += 1
                        cvt = work.tile([128, 512], F32, tag="pt",
                                        name=f"cv{cgi}_{ki}_{ft}")
                        nc.vector.tensor_tensor(out=cvt[:, 0:Wg],
                                                in0=ps[:, 0:Wg],
                                                in1=pen[:, 0:Wg], op=AL.add)
                        for j in cg:
                            rs = coff[j] - cs
                            re = rs + lam[j] - k + 2
                            nc.vector.tensor_reduce(
                                out=rep[:, ki * 2 + ft, j:j + 1],
                                in_=cvt[:, rs:re],
                                axis=mybir.AxisListType.X, op=AL.max)

        NCH = TP // 512 + (1 if TP % 512 else 0)
        chunk_rng = [(ci * 512, min((ci + 1) * 512, TP)) for ci in range(NCH)]

        # tiles first touched by each qk group (for embed/V scheduling)
        emb_done = set()

        def new_tiles(grp):
            g0, g1 = off[grp[0]], off[grp[-1] + 1]
            ts = [t for t in range(g0 // 128, -(-g1 // 128)) if t not in emb_done]
            emb_done.update(ts)
            return ts

        # ---- program ----
        # All embeds first (gathers + posty DMAs ahead of the big weight
        # DMAs in the queues), in group order so group 0 finishes first.
        for grp in lay["qk_groups"]:
            for nt in new_tiles(grp):
                embed(nt)
        for nt in range(TP // 128):
            if nt not in emb_done:
                emb_done.add(nt)
                embed(nt)
        w_cur = load_layer_weights(0, part="qk")
        load_layer_weights(0, w=w_cur, part="rest")
        nc.vector.memset(ctxT[:], 0.0)
        for l in range(NL):
            ctxb_map = {}
            v_done = set()
            wo_done = set()
            fin1_done = set()
            ln1s = None if fl["ln"] else w_cur["ln1s"]
            ln1b = None if fl["ln"] else w_cur["ln1b"]
            ln2s = None if fl["ln"] else w_cur["ln2s"]
            ln2b = None if fl["ln"] else w_cur["ln2b"]

            def flush(j):
                # Wo matmuls + residual/stats (no ACT) for tiles whose
                # attention is complete — dense PE filler between the
                # ACT-paced softmax chains of consecutive slots. Chunk 0's
                # LN1 finalize runs mid-attention so FFN can start with
                # zero bubble at the phase boundary.
                for nt in range(NTT):
                    if nt in wo_done or last_slot_of.get(nt, -1) > j:
                        continue
                    if nt not in ctxb_map and nt in last_slot_of:
                        continue  # not yet computed this pass
                    wo_done.add(nt)
                    wo_stats(nt, w_cur)
                if 0 not in fin1_done and all(
                        t in wo_done for t in range(4)):
                    fin1_done.add(0)
                    ln_finalize("ln1", range(4), ln1s, ln1b, f"a{l}c0")

            for gi, grp in enumerate(lay["qk_groups"]):
                qkT = qk_group(gi, grp, w_cur)
                for j in grp:
                    for (nt, b, kw, rel) in chunks[j]:
                        if nt not in v_done:
                            v_done.add(nt)
                            v_tile(nt, w_cur)
                    attn_slot(l, j, qkT, off[grp[0]], ctxb_map, last_slot_of)
                    if j >= 1:
                        flush(j - 1)
                if gi == 0 and l + 1 < NL:
                    w_nxt = load_layer_weights(l + 1, part="qk")
            if l + 1 < NL:
                load_layer_weights(l + 1, w=w_nxt, part="rest")
            flush(NSL)
            if 0 not in fin1_done:
                ln_finalize("ln1", range(4), ln1s, ln1b, f"a{l}c0")
            ln_finalize("ln1", range(4, NTT), ln1s, ln1b, f"a{l}r")
            for ci, (c0, c1) in enumerate(chunk_rng):
                ffn_chunk(l, c0, c1, w_cur)
                if ci == 0:
                    # next layer's first QK group depends only on these
                    ln_finalize("ln2", range(4), ln2s, ln2b, f"f{l}c0")
                if l == NL - 1 and ci > 0:
                    ln_finalize("ln2", range(ci * 4, min(ci * 4 + 4, NTT)),
                                ln2s, ln2b, f"f{l}c{ci}")
                    conv_ready(ci * 4 + 3)
            if l < NL - 1:
                ln_finalize("ln2", range(4, NTT), ln2s, ln2b, f"f{l}r")
                w_cur = w_nxt
            else:
                conv_ready(NTT)

        if debug:
            for nt in range(NTT):
                dx = work.tile([128, D], F32, tag="dbg", name=f"dbg{nt}")
                nc.vector.tensor_copy(out=dx[:], in_=x_tm[:, nt, :])
                nc.sync.dma_start(out=dbgx_d[nt], in_=dx[:])

        if not fl["cb"]:
            for ki in range(3):
                for ft in range(2):
                    nc.vector.tensor_scalar_add(
                        out=rep[:, ki * 2 + ft, :], in0=rep[:, ki * 2 + ft, :],
                        scalar1=cb[:, ki, ft:ft + 1])
        nc.scalar.activation(out=rep[:], in_=rep[:], func=AF.Relu)

        fps = ps_c.tile([128, 3 * NCLS], F32, tag="c")
        for c in range(6):
            nc.tensor.matmul(fps[:NSL, :], rep[:, c, :], fcw_sb[:, c, :],
                             start=c == 0, stop=c == 5)
        ob = small.tile([NSL, 3 * NCLS], F32, tag="ob")
        nc.scalar.copy(out=ob[:], in_=fps[:NSL, :])
        nc.sync.dma_start(out=out_d[:], in_=ob[:])

    nc.compile()
    return nc


def _core_inputs(inputs, fl, lay):
    """Build the 8 per-core input maps from the full problem inputs."""
    f32 = lambda a: np.ascontiguousarray(np.asarray(a, dtype=np.float32))
    tile_w = lambda w: np.ascontiguousarray(
        f32(w).reshape(w.shape[0] // 128, 128, w.shape[1]).astype(BF))

    NTT, TP, CWP = lay["NTT"], lay["TP"], lay["CWP"]
    lam, lam32, off, coff = lay["lam"], lay["lam32"], lay["off"], lay["coff"]
    assign = lay["assign"]

    shared = {}
    # packed position+type embedding
    posv = np.zeros((TP, D), np.float32)
    pe = f32(inputs["pos_emb"])
    for j in range(NSL):
        posv[off[j]:off[j] + lam32[j]] = pe[:lam32[j]]
    posv += f32(inputs["type_emb"][0])[None, :]
    shared["posty"] = np.ascontiguousarray(posv.reshape(NTT, 128, D))
    for l in range(NL):
        shared[f"wq{l}"] = tile_w(f32(inputs["Wq"][l]) / 8.0)
        shared[f"wk{l}"] = tile_w(inputs["Wk"][l])
        shared[f"wv{l}"] = tile_w(inputs["Wv"][l])
        shared[f"wo{l}"] = tile_w(inputs["Wo"][l])
        shared[f"wi{l}"] = tile_w(inputs["Wi"][l])
        shared[f"wo2{l}"] = tile_w(inputs["Wo2"][l])
        if not fl["bqk"]:
            shared[f"bq{l}"] = f32(inputs["bq"][l]).reshape(ND, 128) / 8.0
            shared[f"bk{l}"] = f32(inputs["bk"][l]).reshape(ND, 128)
        if not fl["bv"]:
            shared[f"bv{l}"] = f32(inputs["bv"][l])
        if not fl["bo"]:
            shared[f"bo{l}"] = f32(inputs["bo"][l])
        if not fl["bi"]:
            shared[f"bi{l}"] = f32(inputs["bi"][l]).reshape(NFT, 128)
        if not fl["bo2"]:
            shared[f"bo2{l}"] = f32(inputs["bo2"][l])
        if not fl["ln"]:
            shared[f"ln1s{l}"] = f32(inputs["ln1_s"][l])
            shared[f"ln1b{l}"] = f32(inputs["ln1_b"][l])
            shared[f"ln2s{l}"] = f32(inputs["ln2_s"][l])
            shared[f"ln2b{l}"] = f32(inputs["ln2_b"][l])
    if not fl["ln"]:
        shared["lnes"] = f32(inputs["emb_ln_s"])
        shared["lneb"] = f32(inputs["emb_ln_b"])
    for ki, k in enumerate((1, 2, 3)):
        w = f32(inputs[f"conv_w{k}"])                    # [NF, k, D]
        wt = np.ascontiguousarray(w.transpose(1, 2, 0))  # [k, D, NF]
        shared[f"cw{k}"] = np.ascontiguousarray(
            wt.reshape(k, ND, 128, NF).astype(BF))
    if not fl["cb"]:
        shared["convb"] = np.stack(
            [f32(inputs[f"conv_b{k}"]).reshape(2, 128) for k in (1, 2, 3)])
    # fc weights for all 3 branch-block hypotheses: [6, 128, 3*NCLS]
    fcw = f32(inputs["fc_w"])                            # [2304, NCLS]
    fcw3 = np.zeros((6, 128, 3 * NCLS), np.float32)
    for bb in range(3):
        for ki in range(3):
            for ft in range(2):
                c = ki * 2 + ft
                rows = 768 * bb + 256 * ki + 128 * ft
                fcw3[c, :, bb * NCLS:(bb + 1) * NCLS] = fcw[rows:rows + 128]
    shared["fcw"] = fcw3
    shared["word_emb"] = f32(inputs["word_emb"])

    ids_all = np.stack([np.asarray(inputs[p + "_input_ids"])
                        for p in ("q", "a", "b")]).reshape(96, S)
    mask_all = np.stack([np.asarray(inputs[p + "_attention_mask"])
                         for p in ("q", "a", "b")]).reshape(96, S)
    lens_all = mask_all.sum(1).astype(int)

    in_maps = []
    for c in range(NCORES):
        m = dict(shared)
        idv = np.zeros(TP, np.int32)
        vmv = np.zeros(TP, np.float32)
        cmv = np.zeros(CWP, np.float32)
        pen = np.full((3, CWP), -1e30, np.float32)
        for j in range(NSL):
            sq = int(assign[j, c])
            l = int(lens_all[sq])
            idv[off[j]:off[j] + lam32[j]] = ids_all[sq][:lam32[j]]
            vmv[off[j]:off[j] + l] = 1.0
            cmv[coff[j]:coff[j] + l] = 1.0
            for ki, k in enumerate((1, 2, 3)):
                nw = l - k + 2
                pen[ki, coff[j]:coff[j] + nw] = 0.0
        m["ids"] = np.ascontiguousarray(idv.reshape(NTT, 128))
        m["vmask"] = np.ascontiguousarray(vmv.reshape(NTT, 128))
        m["convmask"] = np.ascontiguousarray(cmv.astype(BF))
        m["convpen"] = np.ascontiguousarray(pen)
        in_maps.append(m)
    return in_maps


def _get_program(fl, lay, debug=False):
    key = (tuple(sorted(fl.items())), lay["key"], debug)
    if key not in _CACHE:
        _CACHE[key] = _build_program(fl, lay, debug=debug)
    return _CACHE[key]


def run_sharded(inputs, debug=False, **run_kwargs):
    """Shard, run on 8 cores, gather. Returns (output, BassKernelResults)."""
    from concourse.bass_utils import run_bass_kernel_spmd
    fl = _flags(inputs)
    lens96 = np.concatenate([
        np.asarray(inputs[p + "_attention_mask"]).sum(1) for p in ("q", "a", "b")])
    lay = _layout(lens96)
    nc = _get_program(fl, lay, debug=debug)
    in_maps = _core_inputs(inputs, fl, lay)
    res = run_bass_kernel_spmd(nc, in_maps, core_ids=list(range(NCORES)),
                               **run_kwargs)
    border = {0: 0, 1: 2, 2: 1}   # branch q/a/b -> fc block q,b,a
    out = np.zeros((B, NCLS), np.float32)
    for c in range(NCORES):
        o3 = np.asarray(res.results[c]["out"], np.float32)   # [NSL, 12]
        for j in range(NSL):
            sq = int(lay["assign"][j, c])
            br, sample = sq // B, sq % B
            out[sample] += o3[j, border[br] * NCLS:(border[br] + 1) * NCLS]
    out += np.asarray(inputs["fc_b"], np.float32)[None, :]
    return out, res


def kernel(**inputs):
    out, _ = run_sharded(inputs)
    return out


# revision 34
# speedup vs baseline: 1.8371x; 1.0246x over previous
"""Trainium2 Bass kernel for nn_BertCNN (3x BERT-small encoder + CNN maxpool head).

Ragged-packed data-parallel strategy. The 96 sequences (3 branches x 32
samples) are sorted by actual length (from the attention mask), dealt
round-robin into 8 cores x 12 slots, and each core packs its 12 sequences
into one ~1900-token stream (slot budgets = max length in each rank group,
32-aligned starts). All encoder linear ops (QKV/O/FFN/conv) run over the
packed stream; attention runs per-slot with exact budget widths; key
validity is folded multiplicatively into V (invalid keys get zero V rows
and a zero softmax-denominator contribution), so no attention bias is
needed. The conv head runs over a separately packed layout with 2-token
gaps; per-slot maxpool ranges and window-validity penalties come from the
host. The fc output is computed for all 3 branch hypotheses per slot
([12, 12] per core) and the host scatter-adds the right 4 columns into the
final [32, 4].

The Bass program depends only on the slot-budget layout (not on per-core
data); it is built once per layout signature and cached.
"""

import numpy as np
import ml_dtypes

V, D, H, DH, NL, FF = 30522, 512, 8, 64, 4, 2048
NF, NCLS, B, S = 256, 4, 32, 256
NCORES = 8
NSL = 12                 # slots (sequences) per core
NSEQ = NSL               # test.py compat
SPC = 4                  # test.py compat
ND = D // 128
NFT = FF // 128
DH1 = DH + 1

BF = ml_dtypes.bfloat16
_CACHE = {}


def _flags(inputs):
    z = lambda a: bool(np.all(np.asarray(a) == 0))
    o = lambda a: bool(np.all(np.asarray(a) == 1))
    return {
        "bqk": z(inputs["bq"]) and z(inputs["bk"]),
        "bv": z(inputs["bv"]),
        "bo": z(inputs["bo"]),
        "bi": z(inputs["bi"]),
        "bo2": z(inputs["bo2"]),
        "ln": all(o(inputs[k]) for k in ("emb_ln_s", "ln1_s", "ln2_s"))
        and all(z(inputs[k]) for k in ("emb_ln_b", "ln1_b", "ln2_b")),
        "cb": z(inputs["conv_b1"]) and z(inputs["conv_b2"]) and z(inputs["conv_b3"]),
    }


def _layout(lens96):
    """Pack layout shared by all cores (program-shaping constants)."""
    lens96 = np.asarray(lens96, dtype=np.int64)
    order = np.argsort(-lens96, kind="stable")
    assign = order.reshape(NSL, NCORES)           # [slot, core] -> seq idx
    lam = lens96[assign].max(1).astype(int)       # slot budgets (max len)
    # 64-aligned slot starts (PE col/row tiling only supports base 0/64 for
    # >32-wide tiles); bump a start to the next 128 boundary when the slot
    # would otherwise span 3 token tiles (score/eT tiles hold 2 chunks)
    lam32 = ((lam + 63) // 64) * 64
    off = np.zeros(NSL + 1, np.int64)
    for j in range(NSL):
        o = off[j]
        if (o % 128) + lam[j] > 256:
            o = ((o + 127) // 128) * 128
            off[j] = o
        off[j + 1] = o + lam32[j]
    T32 = int(off[-1])
    NTT = -(-T32 // 128)
    TP = NTT * 128
    coff = np.zeros(NSL + 1, np.int64)
    coff[1:] = np.cumsum(lam + 2)
    CW = int(coff[-1])

    def greedy(offs, cap):
        groups, cur = [], [0]
        for j in range(1, NSL):
            if offs[j + 1] - offs[cur[0]] <= cap:
                cur.append(j)
            else:
                groups.append(cur)
                cur = [j]
        groups.append(cur)
        return groups

    qk_groups = greedy(off, 512)
    cv_groups = greedy(coff, 512)

    # per-slot key/query chunks: intersections with the global 128 grid
    chunks = []
    for j in range(NSL):
        lo, hi = int(off[j]), int(off[j] + lam[j])
        ch = []
        p = lo
        while p < hi:
            nt = p // 128
            e = min(hi, (nt + 1) * 128)
            ch.append((nt, p - nt * 128, e - p, p - lo))  # (tile, base, width, rel)
            p = e
        chunks.append(ch)
    return dict(
        assign=assign, lam=[int(x) for x in lam], lam32=[int(x) for x in lam32],
        off=[int(x) for x in off], coff=[int(x) for x in coff],
        T32=T32, NTT=NTT, TP=TP, CW=CW, CWP=CW + 2,
        qk_groups=qk_groups, cv_groups=cv_groups, chunks=chunks,
        key=(tuple(int(x) for x in lam), tuple(int(x) for x in lam32)),
    )


def _build_program(fl, lay, debug=False):
    import contextlib
    import concourse.bass as bass
    import concourse.mybir as mybir
    import concourse.tile as tile
    from concourse import bacc

    F32, BF16, I32 = mybir.dt.float32, mybir.dt.bfloat16, mybir.dt.int32
    AL, AF = mybir.AluOpType, mybir.ActivationFunctionType

    NTT, TP, CWP = lay["NTT"], lay["TP"], lay["CWP"]
    lam, lam32, off, coff = lay["lam"], lay["lam32"], lay["off"], lay["coff"]
    chunks = lay["chunks"]

    nc = bacc.Bacc("TRN2", target_bir_lowering=False, debug=False,
                   num_devices=NCORES)

    di = lambda n, s, d: nc.dram_tensor(n, s, d, kind="ExternalInput").ap()
    word = di("word_emb", [V, D], F32)
    ids_d = di("ids", [NTT, 128], I32)
    vm_d = di("vmask", [NTT, 128], F32)
    posty_d = di("posty", [NTT, 128, D], F32)
    cm_d = di("convmask", [CWP], BF16)
    cpen_d = di("convpen", [3, CWP], F32)
    wq_d = [di(f"wq{l}", [ND, 128, D], BF16) for l in range(NL)]
    wk_d = [di(f"wk{l}", [ND, 128, D], BF16) for l in range(NL)]
    wv_d = [di(f"wv{l}", [ND, 128, D], BF16) for l in range(NL)]
    wo_d = [di(f"wo{l}", [ND, 128, D], BF16) for l in range(NL)]
    wi_d = [di(f"wi{l}", [ND, 128, FF], BF16) for l in range(NL)]
    wo2_d = [di(f"wo2{l}", [NFT, 128, D], BF16) for l in range(NL)]
    cw_d = [di(f"cw{k}", [k, ND, 128, NF], BF16) for k in (1, 2, 3)]
    fcw_d = di("fcw", [6, 128, 3 * NCLS], F32)
    if not fl["bqk"]:
        bq_d = [di(f"bq{l}", [ND, 128], F32) for l in range(NL)]
        bk_d = [di(f"bk{l}", [ND, 128], F32) for l in range(NL)]
    if not fl["bv"]:
        bv_d = [di(f"bv{l}", [D], F32) for l in range(NL)]
    if not fl["bo"]:
        bo_d = [di(f"bo{l}", [D], F32) for l in range(NL)]
    if not fl["bi"]:
        bi_d = [di(f"bi{l}", [NFT, 128], F32) for l in range(NL)]
    if not fl["bo2"]:
        bo2_d = [di(f"bo2{l}", [D], F32) for l in range(NL)]
    if not fl["ln"]:
        elns_d = di("lnes", [D], F32)
        elnb_d = di("lneb", [D], F32)
        ln1s_d = [di(f"ln1s{l}", [D], F32) for l in range(NL)]
        ln1b_d = [di(f"ln1b{l}", [D], F32) for l in range(NL)]
        ln2s_d = [di(f"ln2s{l}", [D], F32) for l in range(NL)]
        ln2b_d = [di(f"ln2b{l}", [D], F32) for l in range(NL)]
    if not fl["cb"]:
        cb_d = di("convb", [3, 2, 128], F32)

    out_d = nc.dram_tensor("out", [NSL, 3 * NCLS], F32, kind="ExternalOutput").ap()
    if debug:
        dbgx_d = nc.dram_tensor("dbgx", [NTT, 128, D], F32,
                                kind="ExternalOutput").ap()

    with tile.TileContext(nc) as tc, contextlib.ExitStack() as ctx:
        consts = ctx.enter_context(tc.tile_pool(name="consts", bufs=1))
        state = ctx.enter_context(tc.tile_pool(name="state", bufs=1))
        wts = ctx.enter_context(tc.tile_pool(name="wts", bufs=1))
        qkp = ctx.enter_context(tc.tile_pool(name="qkp", bufs=2))
        etp = ctx.enter_context(tc.tile_pool(name="etp", bufs=2))
        htp = ctx.enter_context(tc.tile_pool(name="htp", bufs=1))
        work = ctx.enter_context(tc.tile_pool(name="work", bufs=2))
        cxp = ctx.enter_context(tc.tile_pool(name="cxp", bufs=3))
        small = ctx.enter_context(tc.tile_pool(name="small", bufs=4))
        ps_mm = ctx.enter_context(tc.tile_pool(name="ps_mm", bufs=3, space="PSUM"))
        ps_s = ctx.enter_context(tc.tile_pool(name="ps_s", bufs=3, space="PSUM"))
        ps_c = ctx.enter_context(tc.tile_pool(name="ps_c", bufs=2, space="PSUM"))

        # ---- constants ----
        eps_t = consts.tile([128, 1], F32, tag="eps")
        nc.vector.memset(eps_t[:], 1e-12)
        ones8 = consts.tile([128, H], BF16, tag="ones8")
        nc.vector.memset(ones8[:], 1.0)
        ids_sb = consts.tile([128, NTT], I32, tag="ids")
        nc.sync.dma_start(out=ids_sb[:], in_=ids_d.rearrange("t p -> p t"))
        vm_sb = consts.tile([128, NTT], F32, tag="vm")
        nc.sync.dma_start(out=vm_sb[:], in_=vm_d.rearrange("t p -> p t"))
        cm_sb = consts.tile([128, CWP], BF16, tag="cm")
        nc.sync.dma_start(out=cm_sb[:],
                          in_=cm_d[None, :].to_broadcast([128, CWP]))
        fcw_sb = consts.tile([128, 6, 3 * NCLS], F32, tag="fcw")
        nc.sync.dma_start(out=fcw_sb[:], in_=fcw_d.rearrange("c p n -> p c n"))
        bcast = lambda ap, n: ap[None, :].to_broadcast([128, n])
        if not fl["ln"]:
            elns = consts.tile([128, D], F32, tag="elns")
            nc.sync.dma_start(out=elns[:], in_=bcast(elns_d, D))
            elnb = consts.tile([128, D], F32, tag="elnb")
            nc.sync.dma_start(out=elnb[:], in_=bcast(elnb_d, D))
        if not fl["cb"]:
            cb = consts.tile([128, 3, 2], F32, tag="cb")
            nc.sync.dma_start(out=cb[:], in_=cb_d.rearrange("k t p -> p k t"))

        cw = {}
        for ki, k in enumerate((1, 2, 3)):
            for j in range(k):
                t = wts.tile([128, ND, NF], BF16, tag=f"cw{ki}_{j}")
                nc.sync.dma_start(out=t[:],
                                  in_=cw_d[ki][j].rearrange("t p f -> p t f"))
                cw[(k, j)] = t

        # ---- persistent state ----
        x_tm = state.tile([128, NTT, D], BF16, tag="x_tm")
        xTa = state.tile([128, ND, TP], BF16, tag="xTa")
        ctxT = state.tile([128, ND, TP], BF16, tag="ctxT")
        vA = state.tile([128, NTT, H, DH1], BF16, tag="vA")
        rep = state.tile([128, 6, NSL], F32, tag="rep")

        def load_layer_weights(l, w=None, part="all"):
            if w is None:
                w = {}
            names = {"qk": ("wq", "wk"), "rest": ("wv", "wo", "wi"),
                     "all": ("wq", "wk", "wv", "wo", "wi")}[part]
            for nm, dd, nfree in (("wq", wq_d, D), ("wk", wk_d, D),
                                  ("wv", wv_d, D), ("wo", wo_d, D),
                                  ("wi", wi_d, FF)):
                if nm not in names:
                    continue
                w[nm] = wts.tile([128, ND, nfree], BF16, tag=nm, name=f"{nm}_{l}",
                                 bufs=2 if nm in ("wq", "wk") else 1)
                for dt in range(ND):
                    nc.sync.dma_start(out=w[nm][:, dt, :], in_=dd[l][dt])
            if part == "qk":
                return w
            w["wo2"] = wts.tile([128, NFT, D], BF16, tag="wo2", name=f"wo2_{l}")
            for ft in range(0, NFT, 4):
                nc.sync.dma_start(
                    out=w["wo2"][:, ft:ft + 4, :],
                    in_=wo2_d[l][ft:ft + 4].rearrange("t p o -> p t o"))
            if not fl["bqk"]:
                w["bq"] = consts.tile([128, ND], F32, tag="bq", name=f"bq_{l}")
                nc.sync.dma_start(out=w["bq"][:], in_=bq_d[l].rearrange("t p -> p t"))
                w["bk"] = consts.tile([128, ND], F32, tag="bk", name=f"bk_{l}")
                nc.sync.dma_start(out=w["bk"][:], in_=bk_d[l].rearrange("t p -> p t"))
            if not fl["bv"]:
                w["bv"] = consts.tile([128, D], F32, tag="bv", name=f"bv_{l}")
                nc.sync.dma_start(out=w["bv"][:], in_=bcast(bv_d[l], D))
            if not fl["bo"]:
                w["bo"] = consts.tile([128, D], F32, tag="bo", name=f"bo_{l}")
                nc.sync.dma_start(out=w["bo"][:], in_=bcast(bo_d[l], D))
            if not fl["bi"]:
                w["bi"] = consts.tile([128, NFT], F32, tag="bi", name=f"bi_{l}")
                nc.sync.dma_start(out=w["bi"][:], in_=bi_d[l].rearrange("t p -> p t"))
            if not fl["bo2"]:
                w["bo2"] = consts.tile([128, D], F32, tag="bo2", name=f"bo2_{l}")
                nc.sync.dma_start(out=w["bo2"][:], in_=bcast(bo2_d[l], D))
            if not fl["ln"]:
                for nm, dd in (("ln1s", ln1s_d), ("ln1b", ln1b_d),
                               ("ln2s", ln2s_d), ("ln2b", ln2b_d)):
                    w[nm] = consts.tile([128, D], F32, tag=nm, name=f"{nm}_{l}")
                    nc.sync.dma_start(out=w[nm][:], in_=bcast(dd[l], D))
            return w

        def ln_start(src_ap, i=0):
            st = small.tile([128, 6], F32, tag="st", name=f"st{i}")
            mv = small.tile([128, 2], F32, tag="mv", name=f"mv{i}")
            nc.vector.bn_stats(out=st[:], in_=src_ap)
            nc.vector.bn_aggr(out=mv[:], in_=st[:])
            sd = small.tile([128, 1], F32, tag="sd", name=f"sd{i}")
            nc.scalar.activation(out=sd[:], in_=mv[:, 1:2], func=AF.Sqrt,
                                 bias=eps_t[:], scale=1.0)
            nmr = small.tile([128, 2], F32, tag="nmr", name=f"nmr{i}")
            nc.vector.reciprocal(out=nmr[:, 1:2], in_=sd[:])
            nc.vector.tensor_scalar(out=nmr[:, 0:1], in0=mv[:, 0:1],
                                    scalar1=nmr[:, 1:2], scalar2=-1.0,
                                    op0=AL.mult, op1=AL.mult)
            return nmr

        def ln_apply(src_ap, dst_ap, nmr, s_tile, b_tile):
            nc.vector.tensor_scalar(out=dst_ap, in0=src_ap,
                                    scalar1=nmr[:, 1:2], scalar2=nmr[:, 0:1],
                                    op0=AL.mult, op1=AL.add)
            if s_tile is not None:
                nc.vector.tensor_tensor(out=dst_ap, in0=dst_ap, in1=s_tile[:],
                                        op=AL.mult)
            if b_tile is not None:
                nc.vector.tensor_tensor(out=dst_ap, in0=dst_ap, in1=b_tile[:],
                                        op=AL.add)

        def embed(nt):
            g = work.tile([128, D], F32, tag="g", name=f"g{nt}")
            nc.gpsimd.indirect_dma_start(
                out=g[:], out_offset=None, in_=word[:],
                in_offset=bass.IndirectOffsetOnAxis(
                    ap=ids_sb[:, nt:nt + 1], axis=0))
            pt = work.tile([128, D], F32, tag="pt", name=f"pt{nt}")
            nc.sync.dma_start(out=pt[:], in_=posty_d[nt])
            nc.vector.tensor_tensor(out=g[:], in0=g[:], in1=pt[:], op=AL.add)
            dst = x_tm[:, nt, :]
            ln_apply(g[:], dst, ln_start(g[:], i=nt % 4),
                     None if fl["ln"] else elns, None if fl["ln"] else elnb)
            nc.sync.dma_start_transpose(xTa[:, :, nt * 128:(nt + 1) * 128], dst)

        def qk_group(gi, grp, w):
            g0, g1 = off[grp[0]], off[grp[-1] + 1]
            W = g1 - g0
            qkT = qkp.tile([128, 2, ND, 512], BF16, tag="qkT", name=f"qkT{gi}")
            for qi, wt, bt in ((0, w["wq"], "bq"), (1, w["wk"], "bk")):
                for ot in range(ND):
                    ps = ps_mm.tile([128, 512], F32, tag="mm")
                    for dt in range(ND):
                        nc.tensor.matmul(
                            ps[:, 0:W], wt[:, dt, ot * 128:(ot + 1) * 128],
                            xTa[:, dt, g0:g1], start=dt == 0, stop=dt == ND - 1)
                    if fl["bqk"]:
                        nc.vector.tensor_copy(out=qkT[:, qi, ot, 0:W],
                                              in_=ps[:, 0:W])
                    else:
                        nc.scalar.activation(
                            out=qkT[:, qi, ot, 0:W], in_=ps[:, 0:W],
                            func=AF.Identity, bias=w[bt][:, ot:ot + 1], scale=1.0)
            return qkT

        def v_tile(nt, w):
            ps = ps_mm.tile([128, 512], F32, tag="mm")
            for dt in range(ND):
                nc.tensor.matmul(ps[:], xTa[:, dt, nt * 128:(nt + 1) * 128],
                                 w["wv"][:, dt, :], start=dt == 0,
                                 stop=dt == ND - 1)
            if fl["bv"]:
                nc.vector.tensor_scalar_mul(
                    out=vA[:, nt, :, 0:DH],
                    in0=ps.rearrange("p (h d) -> p h d", h=H),
                    scalar1=vm_sb[:, nt:nt + 1])
            else:
                nc.vector.tensor_tensor(
                    out=vA[:, nt, :, 0:DH],
                    in0=ps.rearrange("p (h d) -> p h d", h=H),
                    in1=w["bv"].rearrange("p (h d) -> p h d", h=H), op=AL.add)
                nc.vector.tensor_scalar_mul(
                    out=vA[:, nt, :, 0:DH], in0=vA[:, nt, :, 0:DH],
                    scalar1=vm_sb[:, nt:nt + 1])
            nc.vector.tensor_scalar_mul(
                out=vA[:, nt, :, DH], in0=ones8[:],
                scalar1=vm_sb[:, nt:nt + 1])

        est = {}

        def attn_scores(l, j, qkT, g0):
            L = lam[j]
            ch = chunks[j]
            q0 = off[j] - g0
            eT = etp.tile([128, H, 512], BF16, tag="eT", name=f"eT{l}_{j}")
            est[j] = eT
            for h in range(H):
                hh, dtH = (h % 2) * DH, h // 2
                pss = ps_s.tile([128, 512], F32, tag="s")
                for ci, (nt, b, kw, rel) in enumerate(ch):
                    nc.tensor.matmul(
                        pss[b:b + kw, ci * L:ci * L + L],
                        qkT[hh:hh + DH, 1, dtH, q0 + rel:q0 + rel + kw],
                        qkT[hh:hh + DH, 0, dtH, q0:q0 + L],
                        start=True, stop=True)
                nc.scalar.activation(out=eT[:, h, 0:len(ch) * L],
                                     in_=pss[:, 0:len(ch) * L], func=AF.Exp,
                                     bias=0.0, scale=1.0)

        def attn_ctx(l, j, ctxb_map, last_slot_of):
            L = lam[j]
            ch = chunks[j]
            eT = est.pop(j)
            # ctx per query chunk
            for (qnt, qb, qw, qrel) in ch:
                for hg in range(2):
                    cps = ps_c.tile([128, 4 * DH1], F32, tag="c")
                    for hi in range(4):
                        h = hg * 4 + hi
                        sl = slice(hi * DH1, hi * DH1 + DH1)
                        for ci, (nt, b, kw, rel) in enumerate(ch):
                            nc.tensor.matmul(
                                cps[qb:qb + qw, sl],
                                eT[b:b + kw, h, ci * L + qrel:ci * L + qrel + qw],
                                vA[b:b + kw, nt, h, :],
                                start=ci == 0, stop=ci == len(ch) - 1)
                    if qnt not in ctxb_map:
                        ctxb_map[qnt] = cxp.tile([128, D], BF16, tag="ctxb",
                                                 name=f"cb{l}_{qnt}")
                    ctxb = ctxb_map[qnt]
                    rcp = small.tile([128, 4], F32, tag="rcp")
                    nc.vector.reciprocal(
                        out=rcp[qb:qb + qw, :],
                        in_=cps.rearrange("p (h c) -> p h c", c=DH1)[qb:qb + qw, :, DH])
                    nc.vector.tensor_tensor(
                        out=ctxb.rearrange("p (h d) -> p h d", d=DH)[
                            qb:qb + qw, hg * 4:hg * 4 + 4, :],
                        in0=cps.rearrange("p (h c) -> p h c", c=DH1)[qb:qb + qw, :, 0:DH],
                        in1=rcp[qb:qb + qw, :, None].to_broadcast([qw, 4, DH]),
                        op=AL.mult)
            # flush finished ctxb tiles
            for (qnt, qb, qw, qrel) in ch:
                if last_slot_of.get(qnt) == j:
                    nc.sync.dma_start_transpose(
                        ctxT[:, :, qnt * 128:(qnt + 1) * 128], ctxb_map[qnt][:])

        # deferred-LN machinery: residual-add lands pre-LN values in x_tm
        # (bf16, in place); per-tile bn stats collect into a batch buffer;
        # one Sqrt region per phase finalizes all tiles (ACT table stays
        # resident for Exp/Gelu — each table swap costs ~1.5us)
        mvb = {}
        for ph in ("ln1", "ln2"):
            mvb[ph] = state.tile([128, 2, NTT], F32, tag=f"mvb_{ph}",
                                 name=f"mvb_{ph}")

        def res_stats(nt, ps, bias_t, ph, i=0):
            nc.vector.tensor_tensor(out=x_tm[:, nt, :], in0=ps[:],
                                    in1=x_tm[:, nt, :], op=AL.add)
            if bias_t is not None:
                nc.vector.tensor_tensor(out=x_tm[:, nt, :], in0=x_tm[:, nt, :],
                                        in1=bias_t[:], op=AL.add)
            st = small.tile([128, 6], F32, tag="st", name=f"st{i}")
            nc.vector.bn_stats(out=st[:], in_=x_tm[:, nt, :])
            nc.vector.bn_aggr(out=mvb[ph][:, :, nt], in_=st[:])

        def ln_finalize(ph, tiles, s_tile, b_tile, lbl):
            """Batched LN finalize for `tiles`: one Sqrt region, DVE applies,
            transposes into xTa. Safe to call per-chunk (mini batches)."""
            tiles = [t for t in tiles]
            if not tiles:
                return
            mv = mvb[ph]
            t0, t1 = min(tiles), max(tiles) + 1
            sdb = small.tile([128, NTT], F32, tag="sdb", name=f"sdb{lbl}")
            nc.scalar.activation(out=sdb[:, t0:t1], in_=mv[:, 1, t0:t1],
                                 func=AF.Sqrt, bias=eps_t[:], scale=1.0)
            rsb = small.tile([128, NTT], F32, tag="rsb", name=f"rsb{lbl}")
            nc.vector.reciprocal(out=rsb[:, t0:t1], in_=sdb[:, t0:t1])
            nmb = small.tile([128, NTT], F32, tag="nmb", name=f"nmb{lbl}")
            nc.vector.tensor_tensor(out=nmb[:, t0:t1], in0=mv[:, 0, t0:t1],
                                    in1=rsb[:, t0:t1], op=AL.mult)
            nc.vector.tensor_scalar_mul(out=nmb[:, t0:t1], in0=nmb[:, t0:t1],
                                        scalar1=-1.0)
            for nt in tiles:
                nc.vector.tensor_scalar(
                    out=x_tm[:, nt, :], in0=x_tm[:, nt, :],
                    scalar1=rsb[:, nt:nt + 1], scalar2=nmb[:, nt:nt + 1],
                    op0=AL.mult, op1=AL.add)
                if s_tile is not None:
                    nc.vector.tensor_tensor(out=x_tm[:, nt, :],
                                            in0=x_tm[:, nt, :],
                                            in1=s_tile[:], op=AL.mult)
                if b_tile is not None:
                    nc.vector.tensor_tensor(out=x_tm[:, nt, :],
                                            in0=x_tm[:, nt, :],
                                            in1=b_tile[:], op=AL.add)
                nc.sync.dma_start_transpose(
                    xTa[:, :, nt * 128:(nt + 1) * 128], x_tm[:, nt, :])

        def wo_stats(nt, w):
            ps = ps_mm.tile([128, 512], F32, tag="mm")
            for dt in range(ND):
                nc.tensor.matmul(ps[:], ctxT[:, dt, nt * 128:(nt + 1) * 128],
                                 w["wo"][:, dt, :], start=dt == 0,
                                 stop=dt == ND - 1)
            res_stats(nt, ps, None if fl["bo"] else w["bo"], "ln1", i=nt % 4)

        def ffn_chunk(l, c0, c1, w):
            Wc = c1 - c0
            hT = htp.tile([128, NFT, 512], BF16, tag="hT", name=f"hT{l}_{c0}")
            for ft in range(NFT):
                ps = ps_mm.tile([128, 512], F32, tag="mm")
                for dt in range(ND):
                    nc.tensor.matmul(
                        ps[:, 0:Wc], w["wi"][:, dt, ft * 128:(ft + 1) * 128],
                        xTa[:, dt, c0:c1], start=dt == 0, stop=dt == ND - 1)
                nc.scalar.activation(
                    out=hT[:, ft, 0:Wc], in_=ps[:, 0:Wc], func=AF.Gelu,
                    bias=0.0 if fl["bi"] else w["bi"][:, ft:ft + 1], scale=1.0)
            for nt in range(c0 // 128, c1 // 128):
                toff = nt * 128 - c0
                ps = ps_mm.tile([128, 512], F32, tag="mm")
                for ft in range(NFT):
                    nc.tensor.matmul(ps[:], hT[:, ft, toff:toff + 128],
                                     w["wo2"][:, ft, :], start=ft == 0,
                                     stop=ft == NFT - 1)
                res_stats(nt, ps, None if fl["bo2"] else w["bo2"], "ln2",
                          i=nt % 4)

        # last slot writing each ctx tile (for flush scheduling)
        last_slot_of = {}
        for j in range(NSL):
            for (nt, b, kw, rel) in chunks[j]:
                last_slot_of[nt] = j

        # ---- conv head, interleaved with layer-3 FFN ----
        # xcv reuses ctxT's slot (attention is done by then); slot assembly
        # runs on GPSIMD (idle) as its tiles finish LN2; a conv group's
        # matmuls+maxpools run once all its slots are assembled.
        conv_st = {"xcv": None, "asm": set(), "grp": set()}

        def conv_ready(max_tile):
            if conv_st["xcv"] is None:
                conv_st["xcv"] = state.tile([128, ND, CWP], BF16, tag="ctxT",
                                            name="xcv")
                nc.gpsimd.memset(conv_st["xcv"][:], 0.0)
            xcv = conv_st["xcv"]
            for j in range(NSL):
                if j in conv_st["asm"]:
                    continue
                if max(nt for (nt, b, kw, rel) in chunks[j]) > max_tile:
                    continue
                conv_st["asm"].add(j)
                o0, c0 = off[j], coff[j]
                for dt in range(ND):
                    nc.gpsimd.tensor_tensor(
                        out=xcv[:, dt, c0:c0 + lam[j]],
                        in0=xTa[:, dt, o0:o0 + lam[j]],
                        in1=cm_sb[:, c0:c0 + lam[j]], op=AL.mult)
            for cgi, cg in enumerate(lay["cv_groups"]):
                if cgi in conv_st["grp"]:
                    continue
                if not all(j in conv_st["asm"] for j in cg):
                    continue
                conv_st["grp"].add(cgi)
                cs, ce = coff[cg[0]], coff[cg[-1] + 1]
                Wg = ce - cs
                for ki, k in enumerate((1, 2, 3)):
                    pen = work.tile([128, 512], F32, tag="g",
                                    name=f"pn{cgi}_{ki}")
                    nc.sync.dma_start(
                        out=pen[:, 0:Wg],
                        in_=cpen_d[ki, cs:ce][None, :].to_broadcast([128, Wg]))
                    for ft in range(2):
                        ps = ps_mm.tile([128, 512], F32, tag="mm")
                        idx = 0
                        for dt in range(ND):
                            for jj in range(k):
                                nc.tensor.matmul(
                                    ps[:, 0:Wg],
                                    cw[(k, jj)][:, dt, ft * 128:(ft + 1) * 128],
                                    xcv[:, dt, cs + jj:cs + jj + Wg],
                                    start=idx == 0, stop=idx == ND * k - 1)
                                idx += 1
                        cvt = work.tile([128, 512], F32, tag="pt",
                                        name=f"cv{cgi}_{ki}_{ft}")
                        nc.vector.tensor_tensor(out=cvt[:, 0:Wg],
                                                in0=ps[:, 0:Wg],
                                                in1=pen[:, 0:Wg], op=AL.add)
                        for j in cg:
                            rs = coff[j] - cs
                            re = rs + lam[j] - k + 2
                            nc.vector.tensor_reduce(
                                out=rep[:, ki * 2 + ft, j:j + 1],
                                in_=cvt[:, rs:re],
                                axis=mybir.AxisListType.X, op=AL.max)

        NCH = TP // 512 + (1 if TP % 512 else 0)
        chunk_rng = [(ci * 512, min((ci + 1) * 512, TP)) for ci in range(NCH)]

        # tiles first touched by each qk group (for embed/V scheduling)
        emb_done = set()

        def new_tiles(grp):
            g0, g1 = off[grp[0]], off[grp[-1] + 1]
            ts = [t for t in range(g0 // 128, -(-g1 // 128)) if t not in emb_done]
            emb_done.update(ts)
            return ts

        # ---- program ----
        # All embeds first (gathers + posty DMAs ahead of the big weight
        # DMAs in the queues), in group order so group 0 finishes first.
        for grp in lay["qk_groups"]:
            for nt in new_tiles(grp):
                embed(nt)
        for nt in range(TP // 128):
            if nt not in emb_done:
                emb_done.add(nt)
                embed(nt)
        w_cur = load_layer_weights(0, part="qk")
        load_layer_weights(0, w=w_cur, part="rest")
        nc.vector.memset(ctxT[:], 0.0)
        for l in range(NL):
            ctxb_map = {}
            v_done = set()
            wo_done = set()
            fin1_done = set()
            ln1s = None if fl["ln"] else w_cur["ln1s"]
            ln1b = None if fl["ln"] else w_cur["ln1b"]
            ln2s = None if fl["ln"] else w_cur["ln2s"]
            ln2b = None if fl["ln"] else w_cur["ln2b"]

            def flush(j):
                # Wo matmuls + residual/stats (no ACT) for tiles whose
                # attention is complete — dense PE filler between the
                # ACT-paced softmax chains of consecutive slots. Chunk 0's
                # LN1 finalize runs mid-attention so FFN can start with
                # zero bubble at the phase boundary.
                for nt in range(NTT):
                    if nt in wo_done or last_slot_of.get(nt, -1) > j:
                        continue
                    if nt not in ctxb_map and nt in last_slot_of:
                        continue  # not yet computed this pass
                    wo_done.add(nt)
                    wo_stats(nt, w_cur)
                if 0 not in fin1_done and all(
                        t in wo_done for t in range(4)):
                    fin1_done.add(0)
                    ln_finalize("ln1", range(4), ln1s, ln1b, f"a{l}c0")

            pend = None
            for gi, grp in enumerate(lay["qk_groups"]):
                qkT = qk_group(gi, grp, w_cur)
                for j in grp:
                    for (nt, b, kw, rel) in chunks[j]:
                        if nt not in v_done:
                            v_done.add(nt)
                            v_tile(nt, w_cur)
                    # software pipeline: slot j's scores (and Wo filler)
                    # are emitted before slot j-1's ctx matmuls so the PE
                    # has work while the ACT exp chain for j-1 completes
                    attn_scores(l, j, qkT, off[grp[0]])
                    if pend is not None:
                        attn_ctx(l, pend, ctxb_map, last_slot_of)
                        if pend >= 1:
                            flush(pend - 1)
                    pend = j
                if gi == 0 and l + 1 < NL:
                    w_nxt = load_layer_weights(l + 1, part="qk")
            attn_ctx(l, pend, ctxb_map, last_slot_of)
            if l + 1 < NL:
                load_layer_weights(l + 1, w=w_nxt, part="rest")
            flush(NSL)
            if 0 not in fin1_done:
                ln_finalize("ln1", range(4), ln1s, ln1b, f"a{l}c0")
            ln_finalize("ln1", range(4, NTT), ln1s, ln1b, f"a{l}r")
            for ci, (c0, c1) in enumerate(chunk_rng):
                ffn_chunk(l, c0, c1, w_cur)
                if ci == 0:
                    # next layer's first QK group depends only on these
                    ln_finalize("ln2", range(4), ln2s, ln2b, f"f{l}c0")
                if l == NL - 1 and ci > 0:
                    ln_finalize("ln2", range(ci * 4, min(ci * 4 + 4, NTT)),
                                ln2s, ln2b, f"f{l}c{ci}")
                    conv_ready(ci * 4 + 3)
            if l < NL - 1:
                ln_finalize("ln2", range(4, NTT), ln2s, ln2b, f"f{l}r")
                w_cur = w_nxt
            else:
                conv_ready(NTT)

        if debug:
            for nt in range(NTT):
                dx = work.tile([128, D], F32, tag="dbg", name=f"dbg{nt}")
                nc.vector.tensor_copy(out=dx[:], in_=x_tm[:, nt, :])
                nc.sync.dma_start(out=dbgx_d[nt], in_=dx[:])

        if not fl["cb"]:
            for ki in range(3):
                for ft in range(2):
                    nc.vector.tensor_scalar_add(
                        out=rep[:, ki * 2 + ft, :], in0=rep[:, ki * 2 + ft, :],
                        scalar1=cb[:, ki, ft:ft + 1])
        nc.scalar.activation(out=rep[:], in_=rep[:], func=AF.Relu)

        fps = ps_c.tile([128, 3 * NCLS], F32, tag="c")
        for c in range(6):
            nc.tensor.matmul(fps[:NSL, :], rep[:, c, :], fcw_sb[:, c, :],
                             start=c == 0, stop=c == 5)
        ob = small.tile([NSL, 3 * NCLS], F32, tag="ob")
        nc.scalar.copy(out=ob[:], in_=fps[:NSL, :])
        nc.sync.dma_start(out=out_d[:], in_=ob[:])

    nc.compile()
    return nc


def _core_inputs(inputs, fl, lay):
    """Build the 8 per-core input maps from the full problem inputs."""
    f32 = lambda a: np.ascontiguousarray(np.asarray(a, dtype=np.float32))
    tile_w = lambda w: np.ascontiguousarray(
        f32(w).reshape(w.shape[0] // 128, 128, w.shape[1]).astype(BF))

    NTT, TP, CWP = lay["NTT"], lay["TP"], lay["CWP"]
    lam, lam32, off, coff = lay["lam"], lay["lam32"], lay["off"], lay["coff"]
    assign = lay["assign"]

    shared = {}
    # packed position+type embedding
    posv = np.zeros((TP, D), np.float32)
    pe = f32(inputs["pos_emb"])
    for j in range(NSL):
        posv[off[j]:off[j] + lam32[j]] = pe[:lam32[j]]
    posv += f32(inputs["type_emb"][0])[None, :]
    shared["posty"] = np.ascontiguousarray(posv.reshape(NTT, 128, D))
    for l in range(NL):
        shared[f"wq{l}"] = tile_w(f32(inputs["Wq"][l]) / 8.0)
        shared[f"wk{l}"] = tile_w(inputs["Wk"][l])
        shared[f"wv{l}"] = tile_w(inputs["Wv"][l])
        shared[f"wo{l}"] = tile_w(inputs["Wo"][l])
        shared[f"wi{l}"] = tile_w(inputs["Wi"][l])
        shared[f"wo2{l}"] = tile_w(inputs["Wo2"][l])
        if not fl["bqk"]:
            shared[f"bq{l}"] = f32(inputs["bq"][l]).reshape(ND, 128) / 8.0
            shared[f"bk{l}"] = f32(inputs["bk"][l]).reshape(ND, 128)
        if not fl["bv"]:
            shared[f"bv{l}"] = f32(inputs["bv"][l])
        if not fl["bo"]:
            shared[f"bo{l}"] = f32(inputs["bo"][l])
        if not fl["bi"]:
            shared[f"bi{l}"] = f32(inputs["bi"][l]).reshape(NFT, 128)
        if not fl["bo2"]:
            shared[f"bo2{l}"] = f32(inputs["bo2"][l])
        if not fl["ln"]:
            shared[f"ln1s{l}"] = f32(inputs["ln1_s"][l])
            shared[f"ln1b{l}"] = f32(inputs["ln1_b"][l])
            shared[f"ln2s{l}"] = f32(inputs["ln2_s"][l])
            shared[f"ln2b{l}"] = f32(inputs["ln2_b"][l])
    if not fl["ln"]:
        shared["lnes"] = f32(inputs["emb_ln_s"])
        shared["lneb"] = f32(inputs["emb_ln_b"])
    for ki, k in enumerate((1, 2, 3)):
        w = f32(inputs[f"conv_w{k}"])                    # [NF, k, D]
        wt = np.ascontiguousarray(w.transpose(1, 2, 0))  # [k, D, NF]
        shared[f"cw{k}"] = np.ascontiguousarray(
            wt.reshape(k, ND, 128, NF).astype(BF))
    if not fl["cb"]:
        shared["convb"] = np.stack(
            [f32(inputs[f"conv_b{k}"]).reshape(2, 128) for k in (1, 2, 3)])
    # fc weights for all 3 branch-block hypotheses: [6, 128, 3*NCLS]
    fcw = f32(inputs["fc_w"])                            # [2304, NCLS]
    fcw3 = np.zeros((6, 128, 3 * NCLS), np.float32)
    for bb in range(3):
        for ki in range(3):
            for ft in range(2):
                c = ki * 2 + ft
                rows = 768 * bb + 256 * ki + 128 * ft
                fcw3[c, :, bb * NCLS:(bb + 1) * NCLS] = fcw[rows:rows + 128]
    shared["fcw"] = fcw3
    shared["word_emb"] = f32(inputs["word_emb"])

    ids_all = np.stack([np.asarray(inputs[p + "_input_ids"])
                        for p in ("q", "a", "b")]).reshape(96, S)
    mask_all = np.stack([np.asarray(inputs[p + "_attention_mask"])
                         for p in ("q", "a", "b")]).reshape(96, S)
    lens_all = mask_all.sum(1).astype(int)

    in_maps = []
    for c in range(NCORES):
        m = dict(shared)
        idv = np.zeros(TP, np.int32)
        vmv = np.zeros(TP, np.float32)
        cmv = np.zeros(CWP, np.float32)
        pen = np.full((3, CWP), -1e30, np.float32)
        for j in range(NSL):
            sq = int(assign[j, c])
            l = int(lens_all[sq])
            idv[off[j]:off[j] + lam32[j]] = ids_all[sq][:lam32[j]]
            vmv[off[j]:off[j] + l] = 1.0
            cmv[coff[j]:coff[j] + l] = 1.0
            for ki, k in enumerate((1, 2, 3)):
                nw = l - k + 2
                pen[ki, coff[j]:coff[j] + nw] = 0.0
        m["ids"] = np.ascontiguousarray(idv.reshape(NTT, 128))
        m["vmask"] = np.ascontiguousarray(vmv.reshape(NTT, 128))
        m["convmask"] = np.ascontiguousarray(cmv.astype(BF))
        m["convpen"] = np.ascontiguousarray(pen)
        in_maps.append(m)
    return in_maps


def _get_program(fl, lay, debug=False):
    key = (tuple(sorted(fl.items())), lay["key"], debug)
    if key not in _CACHE:
        _CACHE[key] = _build_program(fl, lay, debug=debug)
    return _CACHE[key]


def run_sharded(inputs, debug=False, **run_kwargs):
    """Shard, run on 8 cores, gather. Returns (output, BassKernelResults)."""
    from concourse.bass_utils import run_bass_kernel_spmd
    fl = _flags(inputs)
    lens96 = np.concatenate([
        np.asarray(inputs[p + "_attention_mask"]).sum(1) for p in ("q", "a", "b")])
    lay = _layout(lens96)
    nc = _get_program(fl, lay, debug=debug)
    in_maps = _core_inputs(inputs, fl, lay)
    res = run_bass_kernel_spmd(nc, in_maps, core_ids=list(range(NCORES)),
                               **run_kwargs)
    border = {0: 0, 1: 2, 2: 1}   # branch q/a/b -> fc block q,b,a
    out = np.zeros((B, NCLS), np.float32)
    for c in range(NCORES):
        o3 = np.asarray(res.results[c]["out"], np.float32)   # [NSL, 12]
        for j in range(NSL):
            sq = int(lay["assign"][j, c])
            br, sample = sq // B, sq % B
            out[sample] += o3[j, border[br] * NCLS:(border[br] + 1) * NCLS]
    out += np.asarray(inputs["fc_b"], np.float32)[None, :]
    return out, res


def kernel(**inputs):
    out, _ = run_sharded(inputs)
    return out


# revision 35
# speedup vs baseline: 1.8471x; 1.0055x over previous
"""Trainium2 Bass kernel for nn_BertCNN (3x BERT-small encoder + CNN maxpool head).

Ragged-packed data-parallel strategy. The 96 sequences (3 branches x 32
samples) are sorted by actual length (from the attention mask), dealt
round-robin into 8 cores x 12 slots, and each core packs its 12 sequences
into one ~1900-token stream (slot budgets = max length in each rank group,
32-aligned starts). All encoder linear ops (QKV/O/FFN/conv) run over the
packed stream; attention runs per-slot with exact budget widths; key
validity is folded multiplicatively into V (invalid keys get zero V rows
and a zero softmax-denominator contribution), so no attention bias is
needed. The conv head runs over a separately packed layout with 2-token
gaps; per-slot maxpool ranges and window-validity penalties come from the
host. The fc output is computed for all 3 branch hypotheses per slot
([12, 12] per core) and the host scatter-adds the right 4 columns into the
final [32, 4].

The Bass program depends only on the slot-budget layout (not on per-core
data); it is built once per layout signature and cached.
"""

import numpy as np
import ml_dtypes

V, D, H, DH, NL, FF = 30522, 512, 8, 64, 4, 2048
NF, NCLS, B, S = 256, 4, 32, 256
NCORES = 8
NSL = 12                 # slots (sequences) per core
NSEQ = NSL               # test.py compat
SPC = 4                  # test.py compat
ND = D // 128
NFT = FF // 128
DH1 = DH + 1

BF = ml_dtypes.bfloat16
_CACHE = {}


def _flags(inputs):
    z = lambda a: bool(np.all(np.asarray(a) == 0))
    o = lambda a: bool(np.all(np.asarray(a) == 1))
    return {
        "bqk": z(inputs["bq"]) and z(inputs["bk"]),
        "bv": z(inputs["bv"]),
        "bo": z(inputs["bo"]),
        "bi": z(inputs["bi"]),
        "bo2": z(inputs["bo2"]),
        "ln": all(o(inputs[k]) for k in ("emb_ln_s", "ln1_s", "ln2_s"))
        and all(z(inputs[k]) for k in ("emb_ln_b", "ln1_b", "ln2_b")),
        "cb": z(inputs["conv_b1"]) and z(inputs["conv_b2"]) and z(inputs["conv_b3"]),
    }


def _layout(lens96):
    """Pack layout shared by all cores (program-shaping constants)."""
    lens96 = np.asarray(lens96, dtype=np.int64)
    order = np.argsort(-lens96, kind="stable")
    assign = order.reshape(NSL, NCORES)           # [slot, core] -> seq idx
    lam = lens96[assign].max(1).astype(int)       # slot budgets (max len)
    # 64-aligned slot starts (PE col/row tiling only supports base 0/64 for
    # >32-wide tiles); bump a start to the next 128 boundary when the slot
    # would otherwise span 3 token tiles (score/eT tiles hold 2 chunks)
    lam32 = ((lam + 63) // 64) * 64
    off = np.zeros(NSL + 1, np.int64)
    for j in range(NSL):
        o = off[j]
        if (o % 128) + lam[j] > 256:
            o = ((o + 127) // 128) * 128
            off[j] = o
        off[j + 1] = o + lam32[j]
    T32 = int(off[-1])
    NTT = -(-T32 // 128)
    TP = NTT * 128
    coff = np.zeros(NSL + 1, np.int64)
    coff[1:] = np.cumsum(lam + 2)
    CW = int(coff[-1])

    def greedy(offs, cap):
        groups, cur = [], [0]
        for j in range(1, NSL):
            if offs[j + 1] - offs[cur[0]] <= cap:
                cur.append(j)
            else:
                groups.append(cur)
                cur = [j]
        groups.append(cur)
        return groups

    qk_groups = greedy(off, 512)
    cv_groups = greedy(coff, 512)

    # per-slot key/query chunks: intersections with the global 128 grid
    chunks = []
    for j in range(NSL):
        lo, hi = int(off[j]), int(off[j] + lam[j])
        ch = []
        p = lo
        while p < hi:
            nt = p // 128
            e = min(hi, (nt + 1) * 128)
            ch.append((nt, p - nt * 128, e - p, p - lo))  # (tile, base, width, rel)
            p = e
        chunks.append(ch)
    return dict(
        assign=assign, lam=[int(x) for x in lam], lam32=[int(x) for x in lam32],
        off=[int(x) for x in off], coff=[int(x) for x in coff],
        T32=T32, NTT=NTT, TP=TP, CW=CW, CWP=CW + 2,
        qk_groups=qk_groups, cv_groups=cv_groups, chunks=chunks,
        key=(tuple(int(x) for x in lam), tuple(int(x) for x in lam32)),
    )


def _build_program(fl, lay, debug=False):
    import contextlib
    import concourse.bass as bass
    import concourse.mybir as mybir
    import concourse.tile as tile
    from concourse import bacc

    F32, BF16, I32 = mybir.dt.float32, mybir.dt.bfloat16, mybir.dt.int32
    AL, AF = mybir.AluOpType, mybir.ActivationFunctionType

    NTT, TP, CWP = lay["NTT"], lay["TP"], lay["CWP"]
    lam, lam32, off, coff = lay["lam"], lay["lam32"], lay["off"], lay["coff"]
    chunks = lay["chunks"]

    nc = bacc.Bacc("TRN2", target_bir_lowering=False, debug=False,
                   num_devices=NCORES)

    di = lambda n, s, d: nc.dram_tensor(n, s, d, kind="ExternalInput").ap()
    word = di("word_emb", [V, D], F32)
    ids_d = di("ids", [NTT, 128], I32)
    vm_d = di("vmask", [NTT, 128], F32)
    posty_d = di("posty", [NTT, 128, D], F32)
    cm_d = di("convmask", [CWP], BF16)
    cpen_d = di("convpen", [3, CWP], F32)
    wq_d = [di(f"wq{l}", [ND, 128, D], BF16) for l in range(NL)]
    wk_d = [di(f"wk{l}", [ND, 128, D], BF16) for l in range(NL)]
    wv_d = [di(f"wv{l}", [ND, 128, D], BF16) for l in range(NL)]
    wo_d = [di(f"wo{l}", [ND, 128, D], BF16) for l in range(NL)]
    wi_d = [di(f"wi{l}", [ND, 128, FF], BF16) for l in range(NL)]
    wo2_d = [di(f"wo2{l}", [NFT, 128, D], BF16) for l in range(NL)]
    cw_d = [di(f"cw{k}", [k, ND, 128, NF], BF16) for k in (1, 2, 3)]
    fcw_d = di("fcw", [6, 128, 3 * NCLS], F32)
    if not fl["bqk"]:
        bq_d = [di(f"bq{l}", [ND, 128], F32) for l in range(NL)]
        bk_d = [di(f"bk{l}", [ND, 128], F32) for l in range(NL)]
    if not fl["bv"]:
        bv_d = [di(f"bv{l}", [D], F32) for l in range(NL)]
    if not fl["bo"]:
        bo_d = [di(f"bo{l}", [D], F32) for l in range(NL)]
    if not fl["bi"]:
        bi_d = [di(f"bi{l}", [NFT, 128], F32) for l in range(NL)]
    if not fl["bo2"]:
        bo2_d = [di(f"bo2{l}", [D], F32) for l in range(NL)]
    if not fl["ln"]:
        elns_d = di("lnes", [D], F32)
        elnb_d = di("lneb", [D], F32)
        ln1s_d = [di(f"ln1s{l}", [D], F32) for l in range(NL)]
        ln1b_d = [di(f"ln1b{l}", [D], F32) for l in range(NL)]
        ln2s_d = [di(f"ln2s{l}", [D], F32) for l in range(NL)]
        ln2b_d = [di(f"ln2b{l}", [D], F32) for l in range(NL)]
    if not fl["cb"]:
        cb_d = di("convb", [3, 2, 128], F32)

    out_d = nc.dram_tensor("out", [NSL, 3 * NCLS], F32, kind="ExternalOutput").ap()
    if debug:
        dbgx_d = nc.dram_tensor("dbgx", [NTT, 128, D], F32,
                                kind="ExternalOutput").ap()

    with tile.TileContext(nc) as tc, contextlib.ExitStack() as ctx:
        consts = ctx.enter_context(tc.tile_pool(name="consts", bufs=1))
        state = ctx.enter_context(tc.tile_pool(name="state", bufs=1))
        wts = ctx.enter_context(tc.tile_pool(name="wts", bufs=1))
        qkp = ctx.enter_context(tc.tile_pool(name="qkp", bufs=2))
        etp = ctx.enter_context(tc.tile_pool(name="etp", bufs=2))
        htp = ctx.enter_context(tc.tile_pool(name="htp", bufs=1))
        work = ctx.enter_context(tc.tile_pool(name="work", bufs=2))
        cxp = ctx.enter_context(tc.tile_pool(name="cxp", bufs=3))
        small = ctx.enter_context(tc.tile_pool(name="small", bufs=4))
        ps_mm = ctx.enter_context(tc.tile_pool(name="ps_mm", bufs=3, space="PSUM"))
        ps_s = ctx.enter_context(tc.tile_pool(name="ps_s", bufs=3, space="PSUM"))
        ps_c = ctx.enter_context(tc.tile_pool(name="ps_c", bufs=2, space="PSUM"))

        # ---- constants ----
        eps_t = consts.tile([128, 1], F32, tag="eps")
        nc.vector.memset(eps_t[:], 1e-12)
        ones8 = consts.tile([128, H], BF16, tag="ones8")
        nc.vector.memset(ones8[:], 1.0)
        ids_sb = consts.tile([128, NTT], I32, tag="ids")
        nc.sync.dma_start(out=ids_sb[:], in_=ids_d.rearrange("t p -> p t"))
        vm_sb = consts.tile([128, NTT], F32, tag="vm")
        nc.sync.dma_start(out=vm_sb[:], in_=vm_d.rearrange("t p -> p t"))
        cm_sb = consts.tile([128, CWP], BF16, tag="cm")
        nc.sync.dma_start(out=cm_sb[:],
                          in_=cm_d[None, :].to_broadcast([128, CWP]))
        fcw_sb = consts.tile([128, 6, 3 * NCLS], F32, tag="fcw")
        nc.sync.dma_start(out=fcw_sb[:], in_=fcw_d.rearrange("c p n -> p c n"))
        bcast = lambda ap, n: ap[None, :].to_broadcast([128, n])
        if not fl["ln"]:
            elns = consts.tile([128, D], F32, tag="elns")
            nc.sync.dma_start(out=elns[:], in_=bcast(elns_d, D))
            elnb = consts.tile([128, D], F32, tag="elnb")
            nc.sync.dma_start(out=elnb[:], in_=bcast(elnb_d, D))
        if not fl["cb"]:
            cb = consts.tile([128, 3, 2], F32, tag="cb")
            nc.sync.dma_start(out=cb[:], in_=cb_d.rearrange("k t p -> p k t"))

        cw = {}
        for ki, k in enumerate((1, 2, 3)):
            for j in range(k):
                t = wts.tile([128, ND, NF], BF16, tag=f"cw{ki}_{j}")
                nc.sync.dma_start(out=t[:],
                                  in_=cw_d[ki][j].rearrange("t p f -> p t f"))
                cw[(k, j)] = t

        # ---- persistent state ----
        x_tm = state.tile([128, NTT, D], BF16, tag="x_tm")
        xTa = state.tile([128, ND, TP], BF16, tag="xTa")
        ctxT = state.tile([128, ND, TP], BF16, tag="ctxT")
        vA = state.tile([128, NTT, H, DH1], BF16, tag="vA")
        rep = state.tile([128, 6, NSL], F32, tag="rep")

        def load_layer_weights(l, w=None, part="all"):
            if w is None:
                w = {}
            names = {"qk": ("wq", "wk"), "rest": ("wv", "wo", "wi"),
                     "all": ("wq", "wk", "wv", "wo", "wi")}[part]
            for nm, dd, nfree in (("wq", wq_d, D), ("wk", wk_d, D),
                                  ("wv", wv_d, D), ("wo", wo_d, D),
                                  ("wi", wi_d, FF)):
                if nm not in names:
                    continue
                w[nm] = wts.tile([128, ND, nfree], BF16, tag=nm, name=f"{nm}_{l}",
                                 bufs=2 if nm in ("wq", "wk") else 1)
                for dt in range(ND):
                    nc.sync.dma_start(out=w[nm][:, dt, :], in_=dd[l][dt])
            if part == "qk":
                return w
            w["wo2"] = wts.tile([128, NFT, D], BF16, tag="wo2", name=f"wo2_{l}")
            for ft in range(0, NFT, 4):
                nc.sync.dma_start(
                    out=w["wo2"][:, ft:ft + 4, :],
                    in_=wo2_d[l][ft:ft + 4].rearrange("t p o -> p t o"))
            if not fl["bqk"]:
                w["bq"] = consts.tile([128, ND], F32, tag="bq", name=f"bq_{l}")
                nc.sync.dma_start(out=w["bq"][:], in_=bq_d[l].rearrange("t p -> p t"))
                w["bk"] = consts.tile([128, ND], F32, tag="bk", name=f"bk_{l}")
                nc.sync.dma_start(out=w["bk"][:], in_=bk_d[l].rearrange("t p -> p t"))
            if not fl["bv"]:
                w["bv"] = consts.tile([128, D], F32, tag="bv", name=f"bv_{l}")
                nc.sync.dma_start(out=w["bv"][:], in_=bcast(bv_d[l], D))
            if not fl["bo"]:
                w["bo"] = consts.tile([128, D], F32, tag="bo", name=f"bo_{l}")
                nc.sync.dma_start(out=w["bo"][:], in_=bcast(bo_d[l], D))
            if not fl["bi"]:
                w["bi"] = consts.tile([128, NFT], F32, tag="bi", name=f"bi_{l}")
                nc.sync.dma_start(out=w["bi"][:], in_=bi_d[l].rearrange("t p -> p t"))
            if not fl["bo2"]:
                w["bo2"] = consts.tile([128, D], F32, tag="bo2", name=f"bo2_{l}")
                nc.sync.dma_start(out=w["bo2"][:], in_=bcast(bo2_d[l], D))
            if not fl["ln"]:
                for nm, dd in (("ln1s", ln1s_d), ("ln1b", ln1b_d),
                               ("ln2s", ln2s_d), ("ln2b", ln2b_d)):
                    w[nm] = consts.tile([128, D], F32, tag=nm, name=f"{nm}_{l}")
                    nc.sync.dma_start(out=w[nm][:], in_=bcast(dd[l], D))
            return w

        def ln_start(src_ap, i=0):
            st = small.tile([128, 6], F32, tag="st", name=f"st{i}")
            mv = small.tile([128, 2], F32, tag="mv", name=f"mv{i}")
            nc.vector.bn_stats(out=st[:], in_=src_ap)
            nc.vector.bn_aggr(out=mv[:], in_=st[:])
            sd = small.tile([128, 1], F32, tag="sd", name=f"sd{i}")
            nc.scalar.activation(out=sd[:], in_=mv[:, 1:2], func=AF.Sqrt,
                                 bias=eps_t[:], scale=1.0)
            nmr = small.tile([128, 2], F32, tag="nmr", name=f"nmr{i}")
            nc.vector.reciprocal(out=nmr[:, 1:2], in_=sd[:])
            nc.vector.tensor_scalar(out=nmr[:, 0:1], in0=mv[:, 0:1],
                                    scalar1=nmr[:, 1:2], scalar2=-1.0,
                                    op0=AL.mult, op1=AL.mult)
            return nmr

        def ln_apply(src_ap, dst_ap, nmr, s_tile, b_tile):
            nc.vector.tensor_scalar(out=dst_ap, in0=src_ap,
                                    scalar1=nmr[:, 1:2], scalar2=nmr[:, 0:1],
                                    op0=AL.mult, op1=AL.add)
            if s_tile is not None:
                nc.vector.tensor_tensor(out=dst_ap, in0=dst_ap, in1=s_tile[:],
                                        op=AL.mult)
            if b_tile is not None:
                nc.vector.tensor_tensor(out=dst_ap, in0=dst_ap, in1=b_tile[:],
                                        op=AL.add)

        def embed(nt):
            g = work.tile([128, D], F32, tag="g", name=f"g{nt}")
            nc.gpsimd.indirect_dma_start(
                out=g[:], out_offset=None, in_=word[:],
                in_offset=bass.IndirectOffsetOnAxis(
                    ap=ids_sb[:, nt:nt + 1], axis=0))
            pt = work.tile([128, D], F32, tag="pt", name=f"pt{nt}")
            nc.sync.dma_start(out=pt[:], in_=posty_d[nt])
            nc.vector.tensor_tensor(out=g[:], in0=g[:], in1=pt[:], op=AL.add)
            dst = x_tm[:, nt, :]
            ln_apply(g[:], dst, ln_start(g[:], i=nt % 4),
                     None if fl["ln"] else elns, None if fl["ln"] else elnb)
            nc.sync.dma_start_transpose(xTa[:, :, nt * 128:(nt + 1) * 128], dst)

        def qk_group(gi, grp, w):
            g0, g1 = off[grp[0]], off[grp[-1] + 1]
            W = g1 - g0
            qkT = qkp.tile([128, 2, ND, 512], BF16, tag="qkT", name=f"qkT{gi}")
            for qi, wt, bt in ((0, w["wq"], "bq"), (1, w["wk"], "bk")):
                for ot in range(ND):
                    ps = ps_mm.tile([128, 512], F32, tag="mm")
                    for dt in range(ND):
                        nc.tensor.matmul(
                            ps[:, 0:W], wt[:, dt, ot * 128:(ot + 1) * 128],
                            xTa[:, dt, g0:g1], start=dt == 0, stop=dt == ND - 1)
                    if fl["bqk"]:
                        nc.vector.tensor_copy(out=qkT[:, qi, ot, 0:W],
                                              in_=ps[:, 0:W])
                    else:
                        nc.scalar.activation(
                            out=qkT[:, qi, ot, 0:W], in_=ps[:, 0:W],
                            func=AF.Identity, bias=w[bt][:, ot:ot + 1], scale=1.0)
            return qkT

        def v_tile(nt, w):
            ps = ps_mm.tile([128, 512], F32, tag="mm")
            for dt in range(ND):
                nc.tensor.matmul(ps[:], xTa[:, dt, nt * 128:(nt + 1) * 128],
                                 w["wv"][:, dt, :], start=dt == 0,
                                 stop=dt == ND - 1)
            if fl["bv"]:
                nc.vector.tensor_scalar_mul(
                    out=vA[:, nt, :, 0:DH],
                    in0=ps.rearrange("p (h d) -> p h d", h=H),
                    scalar1=vm_sb[:, nt:nt + 1])
            else:
                nc.vector.tensor_tensor(
                    out=vA[:, nt, :, 0:DH],
                    in0=ps.rearrange("p (h d) -> p h d", h=H),
                    in1=w["bv"].rearrange("p (h d) -> p h d", h=H), op=AL.add)
                nc.vector.tensor_scalar_mul(
                    out=vA[:, nt, :, 0:DH], in0=vA[:, nt, :, 0:DH],
                    scalar1=vm_sb[:, nt:nt + 1])
            nc.vector.tensor_scalar_mul(
                out=vA[:, nt, :, DH], in0=ones8[:],
                scalar1=vm_sb[:, nt:nt + 1])

        est = {}

        def attn_scores(l, j, qkT, g0):
            L = lam[j]
            ch = chunks[j]
            q0 = off[j] - g0
            eT = etp.tile([128, H, 512], BF16, tag="eT", name=f"eT{l}_{j}")
            est[j] = eT
            for h in range(H):
                hh, dtH = (h % 2) * DH, h // 2
                pss = ps_s.tile([128, 512], F32, tag="s")
                for ci, (nt, b, kw, rel) in enumerate(ch):
                    nc.tensor.matmul(
                        pss[b:b + kw, ci * L:ci * L + L],
                        qkT[hh:hh + DH, 1, dtH, q0 + rel:q0 + rel + kw],
                        qkT[hh:hh + DH, 0, dtH, q0:q0 + L],
                        start=True, stop=True)
                nc.scalar.activation(out=eT[:, h, 0:len(ch) * L],
                                     in_=pss[:, 0:len(ch) * L], func=AF.Exp,
                                     bias=0.0, scale=1.0)

        def attn_ctx(l, j, ctxb_map, last_slot_of):
            L = lam[j]
            ch = chunks[j]
            eT = est.pop(j)
            # ctx per query chunk
            for (qnt, qb, qw, qrel) in ch:
                for hg in range(2):
                    cps = ps_c.tile([128, 4 * DH1], F32, tag="c")
                    for hi in range(4):
                        h = hg * 4 + hi
                        sl = slice(hi * DH1, hi * DH1 + DH1)
                        for ci, (nt, b, kw, rel) in enumerate(ch):
                            nc.tensor.matmul(
                                cps[qb:qb + qw, sl],
                                eT[b:b + kw, h, ci * L + qrel:ci * L + qrel + qw],
                                vA[b:b + kw, nt, h, :],
                                start=ci == 0, stop=ci == len(ch) - 1)
                    if qnt not in ctxb_map:
                        ctxb_map[qnt] = cxp.tile([128, D], BF16, tag="ctxb",
                                                 name=f"cb{l}_{qnt}")
                    ctxb = ctxb_map[qnt]
                    rcp = small.tile([128, 4], F32, tag="rcp")
                    nc.vector.reciprocal(
                        out=rcp[qb:qb + qw, :],
                        in_=cps.rearrange("p (h c) -> p h c", c=DH1)[qb:qb + qw, :, DH])
                    nc.vector.tensor_tensor(
                        out=ctxb.rearrange("p (h d) -> p h d", d=DH)[
                            qb:qb + qw, hg * 4:hg * 4 + 4, :],
                        in0=cps.rearrange("p (h c) -> p h c", c=DH1)[qb:qb + qw, :, 0:DH],
                        in1=rcp[qb:qb + qw, :, None].to_broadcast([qw, 4, DH]),
                        op=AL.mult)
            # flush finished ctxb tiles
            for (qnt, qb, qw, qrel) in ch:
                if last_slot_of.get(qnt) == j:
                    nc.sync.dma_start_transpose(
                        ctxT[:, :, qnt * 128:(qnt + 1) * 128], ctxb_map[qnt][:])

        # deferred-LN machinery: residual-add lands pre-LN values in x_tm
        # (bf16, in place); per-tile bn stats collect into a batch buffer;
        # one Sqrt region per phase finalizes all tiles (ACT table stays
        # resident for Exp/Gelu — each table swap costs ~1.5us)
        mvb = {}
        for ph in ("ln1", "ln2"):
            mvb[ph] = state.tile([128, 2, NTT], F32, tag=f"mvb_{ph}",
                                 name=f"mvb_{ph}")

        def res_stats(nt, ps, bias_t, ph, i=0):
            nc.vector.tensor_tensor(out=x_tm[:, nt, :], in0=ps[:],
                                    in1=x_tm[:, nt, :], op=AL.add)
            if bias_t is not None:
                nc.vector.tensor_tensor(out=x_tm[:, nt, :], in0=x_tm[:, nt, :],
                                        in1=bias_t[:], op=AL.add)
            st = small.tile([128, 6], F32, tag="st", name=f"st{i}")
            nc.vector.bn_stats(out=st[:], in_=x_tm[:, nt, :])
            nc.vector.bn_aggr(out=mvb[ph][:, :, nt], in_=st[:])

        def ln_finalize(ph, tiles, s_tile, b_tile, lbl):
            """Batched LN finalize for `tiles`: one Sqrt region, DVE applies,
            transposes into xTa. Safe to call per-chunk (mini batches)."""
            tiles = [t for t in tiles]
            if not tiles:
                return
            mv = mvb[ph]
            t0, t1 = min(tiles), max(tiles) + 1
            sdb = small.tile([128, NTT], F32, tag="sdb", name=f"sdb{lbl}")
            nc.scalar.activation(out=sdb[:, t0:t1], in_=mv[:, 1, t0:t1],
                                 func=AF.Sqrt, bias=eps_t[:], scale=1.0)
            rsb = small.tile([128, NTT], F32, tag="rsb", name=f"rsb{lbl}")
            nc.vector.reciprocal(out=rsb[:, t0:t1], in_=sdb[:, t0:t1])
            nmb = small.tile([128, NTT], F32, tag="nmb", name=f"nmb{lbl}")
            nc.vector.tensor_tensor(out=nmb[:, t0:t1], in0=mv[:, 0, t0:t1],
                                    in1=rsb[:, t0:t1], op=AL.mult)
            nc.vector.tensor_scalar_mul(out=nmb[:, t0:t1], in0=nmb[:, t0:t1],
                                        scalar1=-1.0)
            for nt in tiles:
                nc.vector.tensor_scalar(
                    out=x_tm[:, nt, :], in0=x_tm[:, nt, :],
                    scalar1=rsb[:, nt:nt + 1], scalar2=nmb[:, nt:nt + 1],
                    op0=AL.mult, op1=AL.add)
                if s_tile is not None:
                    nc.vector.tensor_tensor(out=x_tm[:, nt, :],
                                            in0=x_tm[:, nt, :],
                                            in1=s_tile[:], op=AL.mult)
                if b_tile is not None:
                    nc.vector.tensor_tensor(out=x_tm[:, nt, :],
                                            in0=x_tm[:, nt, :],
                                            in1=b_tile[:], op=AL.add)
                nc.sync.dma_start_transpose(
                    xTa[:, :, nt * 128:(nt + 1) * 128], x_tm[:, nt, :])

        def wo_stats(nt, w):
            ps = ps_mm.tile([128, 512], F32, tag="mm")
            for dt in range(ND):
                nc.tensor.matmul(ps[:], ctxT[:, dt, nt * 128:(nt + 1) * 128],
                                 w["wo"][:, dt, :], start=dt == 0,
                                 stop=dt == ND - 1)
            res_stats(nt, ps, None if fl["bo"] else w["bo"], "ln1", i=nt % 4)

        def ffn_chunk(l, c0, c1, w):
            Wc = c1 - c0
            hT = htp.tile([128, NFT, 512], BF16, tag="hT", name=f"hT{l}_{c0}")
            for ft in range(NFT):
                ps = ps_mm.tile([128, 512], F32, tag="mm")
                for dt in range(ND):
                    nc.tensor.matmul(
                        ps[:, 0:Wc], w["wi"][:, dt, ft * 128:(ft + 1) * 128],
                        xTa[:, dt, c0:c1], start=dt == 0, stop=dt == ND - 1)
                nc.scalar.activation(
                    out=hT[:, ft, 0:Wc], in_=ps[:, 0:Wc], func=AF.Gelu,
                    bias=0.0 if fl["bi"] else w["bi"][:, ft:ft + 1], scale=1.0)
            for nt in range(c0 // 128, c1 // 128):
                toff = nt * 128 - c0
                ps = ps_mm.tile([128, 512], F32, tag="mm")
                for ft in range(NFT):
                    nc.tensor.matmul(ps[:], hT[:, ft, toff:toff + 128],
                                     w["wo2"][:, ft, :], start=ft == 0,
                                     stop=ft == NFT - 1)
                res_stats(nt, ps, None if fl["bo2"] else w["bo2"], "ln2",
                          i=nt % 4)

        # last slot writing each ctx tile (for flush scheduling)
        last_slot_of = {}
        for j in range(NSL):
            for (nt, b, kw, rel) in chunks[j]:
                last_slot_of[nt] = j

        # ---- conv head, interleaved with layer-3 FFN ----
        # xcv reuses ctxT's slot (attention is done by then); slot assembly
        # runs on GPSIMD (idle) as its tiles finish LN2; a conv group's
        # matmuls+maxpools run once all its slots are assembled.
        conv_st = {"xcv": None, "asm": set(), "grp": set()}

        def conv_ready(max_tile):
            if conv_st["xcv"] is None:
                conv_st["xcv"] = state.tile([128, ND, CWP], BF16, tag="ctxT",
                                            name="xcv")
                nc.gpsimd.memset(conv_st["xcv"][:], 0.0)
            xcv = conv_st["xcv"]
            for j in range(NSL):
                if j in conv_st["asm"]:
                    continue
                if max(nt for (nt, b, kw, rel) in chunks[j]) > max_tile:
                    continue
                conv_st["asm"].add(j)
                o0, c0 = off[j], coff[j]
                for dt in range(ND):
                    nc.gpsimd.tensor_tensor(
                        out=xcv[:, dt, c0:c0 + lam[j]],
                        in0=xTa[:, dt, o0:o0 + lam[j]],
                        in1=cm_sb[:, c0:c0 + lam[j]], op=AL.mult)
            for cgi, cg in enumerate(lay["cv_groups"]):
                if cgi in conv_st["grp"]:
                    continue
                if not all(j in conv_st["asm"] for j in cg):
                    continue
                conv_st["grp"].add(cgi)
                cs, ce = coff[cg[0]], coff[cg[-1] + 1]
                Wg = ce - cs
                for ki, k in enumerate((1, 2, 3)):
                    pen = work.tile([128, 512], F32, tag="g",
                                    name=f"pn{cgi}_{ki}")
                    nc.sync.dma_start(
                        out=pen[:, 0:Wg],
                        in_=cpen_d[ki, cs:ce][None, :].to_broadcast([128, Wg]))
                    for ft in range(2):
                        ps = ps_mm.tile([128, 512], F32, tag="mm")
                        idx = 0
                        for dt in range(ND):
                            for jj in range(k):
                                nc.tensor.matmul(
                                    ps[:, 0:Wg],
                                    cw[(k, jj)][:, dt, ft * 128:(ft + 1) * 128],
                                    xcv[:, dt, cs + jj:cs + jj + Wg],
                                    start=idx == 0, stop=idx == ND * k - 1)
                                idx += 1
                        cvt = work.tile([128, 512], F32, tag="pt",
                                        name=f"cv{cgi}_{ki}_{ft}")
                        nc.vector.tensor_tensor(out=cvt[:, 0:Wg],
                                                in0=ps[:, 0:Wg],
                                                in1=pen[:, 0:Wg], op=AL.add)
                        for j in cg:
                            rs = coff[j] - cs
                            re = rs + lam[j] - k + 2
                            nc.vector.tensor_reduce(
                                out=rep[:, ki * 2 + ft, j:j + 1],
                                in_=cvt[:, rs:re],
                                axis=mybir.AxisListType.X, op=AL.max)

        NCH = TP // 512 + (1 if TP % 512 else 0)
        chunk_rng = [(ci * 512, min((ci + 1) * 512, TP)) for ci in range(NCH)]

        # tiles first touched by each qk group (for embed/V scheduling)
        emb_done = set()

        def new_tiles(grp):
            g0, g1 = off[grp[0]], off[grp[-1] + 1]
            ts = [t for t in range(g0 // 128, -(-g1 // 128)) if t not in emb_done]
            emb_done.update(ts)
            return ts

        # ---- program ----
        # All embeds first (gathers + posty DMAs ahead of the big weight
        # DMAs in the queues), in group order so group 0 finishes first.
        for grp in lay["qk_groups"]:
            for nt in new_tiles(grp):
                embed(nt)
        for nt in range(TP // 128):
            if nt not in emb_done:
                emb_done.add(nt)
                embed(nt)
        w_cur = load_layer_weights(0, part="qk")
        load_layer_weights(0, w=w_cur, part="rest")
        nc.vector.memset(ctxT[:], 0.0)
        for l in range(NL):
            ctxb_map = {}
            v_done = set()
            wo_done = set()
            fin1_done = set()
            ln1s = None if fl["ln"] else w_cur["ln1s"]
            ln1b = None if fl["ln"] else w_cur["ln1b"]
            ln2s = None if fl["ln"] else w_cur["ln2s"]
            ln2b = None if fl["ln"] else w_cur["ln2b"]

            def flush(j):
                # Wo matmuls + residual/stats (no ACT) for tiles whose
                # attention is complete — dense PE filler between the
                # ACT-paced softmax chains of consecutive slots. Chunk 0's
                # LN1 finalize runs mid-attention so FFN can start with
                # zero bubble at the phase boundary.
                for nt in range(NTT):
                    if nt in wo_done or last_slot_of.get(nt, -1) > j:
                        continue
                    if nt not in ctxb_map and nt in last_slot_of:
                        continue  # not yet computed this pass
                    wo_done.add(nt)
                    wo_stats(nt, w_cur)
                if 0 not in fin1_done and all(
                        t in wo_done for t in range(4)):
                    fin1_done.add(0)
                    ln_finalize("ln1", range(4), ln1s, ln1b, f"a{l}c0")

            pend = None
            for gi, grp in enumerate(lay["qk_groups"]):
                qkT = qk_group(gi, grp, w_cur)
                for j in grp:
                    for (nt, b, kw, rel) in chunks[j]:
                        if nt not in v_done:
                            v_done.add(nt)
                            v_tile(nt, w_cur)
                    # software pipeline: slot j's scores (and Wo filler)
                    # are emitted before slot j-1's ctx matmuls so the PE
                    # has work while the ACT exp chain for j-1 completes
                    attn_scores(l, j, qkT, off[grp[0]])
                    if pend is not None:
                        if pend >= 1:
                            flush(pend - 1)
                        attn_ctx(l, pend, ctxb_map, last_slot_of)
                    pend = j
                if gi == 0 and l + 1 < NL:
                    w_nxt = load_layer_weights(l + 1, part="qk")
            attn_ctx(l, pend, ctxb_map, last_slot_of)
            if l + 1 < NL:
                load_layer_weights(l + 1, w=w_nxt, part="rest")
            flush(NSL)
            if 0 not in fin1_done:
                ln_finalize("ln1", range(4), ln1s, ln1b, f"a{l}c0")
            ln_finalize("ln1", range(4, NTT), ln1s, ln1b, f"a{l}r")
            for ci, (c0, c1) in enumerate(chunk_rng):
                ffn_chunk(l, c0, c1, w_cur)
                if ci == 0:
                    # next layer's first QK group depends only on these
                    ln_finalize("ln2", range(4), ln2s, ln2b, f"f{l}c0")
                if l == NL - 1 and ci > 0:
                    ln_finalize("ln2", range(ci * 4, min(ci * 4 + 4, NTT)),
                                ln2s, ln2b, f"f{l}c{ci}")
                    conv_ready(ci * 4 + 3)
            if l < NL - 1:
                ln_finalize("ln2", range(4, NTT), ln2s, ln2b, f"f{l}r")
                w_cur = w_nxt
            else:
                conv_ready(NTT)

        if debug:
            for nt in range(NTT):
                dx = work.tile([128, D], F32, tag="dbg", name=f"dbg{nt}")
                nc.vector.tensor_copy(out=dx[:], in_=x_tm[:, nt, :])
                nc.sync.dma_start(out=dbgx_d[nt], in_=dx[:])

        if not fl["cb"]:
            for ki in range(3):
                for ft in range(2):
                    nc.vector.tensor_scalar_add(
                        out=rep[:, ki * 2 + ft, :], in0=rep[:, ki * 2 + ft, :],
                        scalar1=cb[:, ki, ft:ft + 1])
        nc.scalar.activation(out=rep[:], in_=rep[:], func=AF.Relu)

        fps = ps_c.tile([128, 3 * NCLS], F32, tag="c")
        for c in range(6):
            nc.tensor.matmul(fps[:NSL, :], rep[:, c, :], fcw_sb[:, c, :],
                             start=c == 0, stop=c == 5)
        ob = small.tile([NSL, 3 * NCLS], F32, tag="ob")
        nc.scalar.copy(out=ob[:], in_=fps[:NSL, :])
        nc.sync.dma_start(out=out_d[:], in_=ob[:])

    nc.compile()
    return nc


def _core_inputs(inputs, fl, lay):
    """Build the 8 per-core input maps from the full problem inputs."""
    f32 = lambda a: np.ascontiguousarray(np.asarray(a, dtype=np.float32))
    tile_w = lambda w: np.ascontiguousarray(
        f32(w).reshape(w.shape[0] // 128, 128, w.shape[1]).astype(BF))

    NTT, TP, CWP = lay["NTT"], lay["TP"], lay["CWP"]
    lam, lam32, off, coff = lay["lam"], lay["lam32"], lay["off"], lay["coff"]
    assign = lay["assign"]

    shared = {}
    # packed position+type embedding
    posv = np.zeros((TP, D), np.float32)
    pe = f32(inputs["pos_emb"])
    for j in range(NSL):
        posv[off[j]:off[j] + lam32[j]] = pe[:lam32[j]]
    posv += f32(inputs["type_emb"][0])[None, :]
    shared["posty"] = np.ascontiguousarray(posv.reshape(NTT, 128, D))
    for l in range(NL):
        shared[f"wq{l}"] = tile_w(f32(inputs["Wq"][l]) / 8.0)
        shared[f"wk{l}"] = tile_w(inputs["Wk"][l])
        shared[f"wv{l}"] = tile_w(inputs["Wv"][l])
        shared[f"wo{l}"] = tile_w(inputs["Wo"][l])
        shared[f"wi{l}"] = tile_w(inputs["Wi"][l])
        shared[f"wo2{l}"] = tile_w(inputs["Wo2"][l])
        if not fl["bqk"]:
            shared[f"bq{l}"] = f32(inputs["bq"][l]).reshape(ND, 128) / 8.0
            shared[f"bk{l}"] = f32(inputs["bk"][l]).reshape(ND, 128)
        if not fl["bv"]:
            shared[f"bv{l}"] = f32(inputs["bv"][l])
        if not fl["bo"]:
            shared[f"bo{l}"] = f32(inputs["bo"][l])
        if not fl["bi"]:
            shared[f"bi{l}"] = f32(inputs["bi"][l]).reshape(NFT, 128)
        if not fl["bo2"]:
            shared[f"bo2{l}"] = f32(inputs["bo2"][l])
        if not fl["ln"]:
            shared[f"ln1s{l}"] = f32(inputs["ln1_s"][l])
            shared[f"ln1b{l}"] = f32(inputs["ln1_b"][l])
            shared[f"ln2s{l}"] = f32(inputs["ln2_s"][l])
            shared[f"ln2b{l}"] = f32(inputs["ln2_b"][l])
    if not fl["ln"]:
        shared["lnes"] = f32(inputs["emb_ln_s"])
        shared["lneb"] = f32(inputs["emb_ln_b"])
    for ki, k in enumerate((1, 2, 3)):
        w = f32(inputs[f"conv_w{k}"])                    # [NF, k, D]
        wt = np.ascontiguousarray(w.transpose(1, 2, 0))  # [k, D, NF]
        shared[f"cw{k}"] = np.ascontiguousarray(
            wt.reshape(k, ND, 128, NF).astype(BF))
    if not fl["cb"]:
        shared["convb"] = np.stack(
            [f32(inputs[f"conv_b{k}"]).reshape(2, 128) for k in (1, 2, 3)])
    # fc weights for all 3 branch-block hypotheses: [6, 128, 3*NCLS]
    fcw = f32(inputs["fc_w"])                            # [2304, NCLS]
    fcw3 = np.zeros((6, 128, 3 * NCLS), np.float32)
    for bb in range(3):
        for ki in range(3):
            for ft in range(2):
                c = ki * 2 + ft
                rows = 768 * bb + 256 * ki + 128 * ft
                fcw3[c, :, bb * NCLS:(bb + 1) * NCLS] = fcw[rows:rows + 128]
    shared["fcw"] = fcw3
    shared["word_emb"] = f32(inputs["word_emb"])

    ids_all = np.stack([np.asarray(inputs[p + "_input_ids"])
                        for p in ("q", "a", "b")]).reshape(96, S)
    mask_all = np.stack([np.asarray(inputs[p + "_attention_mask"])
                         for p in ("q", "a", "b")]).reshape(96, S)
    lens_all = mask_all.sum(1).astype(int)

    in_maps = []
    for c in range(NCORES):
        m = dict(shared)
        idv = np.zeros(TP, np.int32)
        vmv = np.zeros(TP, np.float32)
        cmv = np.zeros(CWP, np.float32)
        pen = np.full((3, CWP), -1e30, np.float32)
        for j in range(NSL):
            sq = int(assign[j, c])
            l = int(lens_all[sq])
            idv[off[j]:off[j] + lam32[j]] = ids_all[sq][:lam32[j]]
            vmv[off[j]:off[j] + l] = 1.0
            cmv[coff[j]:coff[j] + l] = 1.0
            for ki, k in enumerate((1, 2, 3)):
                nw = l - k + 2
                pen[ki, coff[j]:coff[j] + nw] = 0.0
        m["ids"] = np.ascontiguousarray(idv.reshape(NTT, 128))
        m["vmask"] = np.ascontiguousarray(vmv.reshape(NTT, 128))
        m["convmask"] = np.ascontiguousarray(cmv.astype(BF))
        m["convpen"] = np.ascontiguousarray(pen)
        in_maps.append(m)
    return in_maps


def _get_program(fl, lay, debug=False):
    key = (tuple(sorted(fl.items())), lay["key"], debug)
    if key not in _CACHE:
        _CACHE[key] = _build_program(fl, lay, debug=debug)
    return _CACHE[key]


def run_sharded(inputs, debug=False, **run_kwargs):
    """Shard, run on 8 cores, gather. Returns (output, BassKernelResults)."""
    from concourse.bass_utils import run_bass_kernel_spmd
    fl = _flags(inputs)
    lens96 = np.concatenate([
        np.asarray(inputs[p + "_attention_mask"]).sum(1) for p in ("q", "a", "b")])
    lay = _layout(lens96)
    nc = _get_program(fl, lay, debug=debug)
    in_maps = _core_inputs(inputs, fl, lay)
    res = run_bass_kernel_spmd(nc, in_maps, core_ids=list(range(NCORES)),
                               **run_kwargs)
    border = {0: 0, 1: 2, 2: 1}   # branch q/a/b -> fc block q,b,a
    out = np.zeros((B, NCLS), np.float32)
    for c in range(NCORES):
        o3 = np.asarray(res.results[c]["out"], np.float32)   # [NSL, 12]
        for j in range(NSL):
            sq = int(lay["assign"][j, c])
            br, sample = sq // B, sq % B
            out[sample] += o3[j, border[br] * NCLS:(border[br] + 1) * NCLS]
    out += np.asarray(inputs["fc_b"], np.float32)[None, :]
    return out, res


def kernel(**inputs):
    out, _ = run_sharded(inputs)
    return out


# revision 39
# speedup vs baseline: 1.8677x; 1.0111x over previous
"""Trainium2 Bass kernel for nn_BertCNN (3x BERT-small encoder + CNN maxpool head).

Ragged-packed data-parallel strategy. The 96 sequences (3 branches x 32
samples) are sorted by actual length (from the attention mask), dealt
round-robin into 8 cores x 12 slots, and each core packs its 12 sequences
into one ~1900-token stream (slot budgets = max length in each rank group,
32-aligned starts). All encoder linear ops (QKV/O/FFN/conv) run over the
packed stream; attention runs per-slot with exact budget widths; key
validity is folded multiplicatively into V (invalid keys get zero V rows
and a zero softmax-denominator contribution), so no attention bias is
needed. The conv head runs over a separately packed layout with 2-token
gaps; per-slot maxpool ranges and window-validity penalties come from the
host. The fc output is computed for all 3 branch hypotheses per slot
([12, 12] per core) and the host scatter-adds the right 4 columns into the
final [32, 4].

The Bass program depends only on the slot-budget layout (not on per-core
data); it is built once per layout signature and cached.
"""

import numpy as np
import ml_dtypes

V, D, H, DH, NL, FF = 30522, 512, 8, 64, 4, 2048
NF, NCLS, B, S = 256, 4, 32, 256
NCORES = 8
NSL = 12                 # slots (sequences) per core
NSEQ = NSL               # test.py compat
SPC = 4                  # test.py compat
ND = D // 128
NFT = FF // 128
DH1 = DH + 1

BF = ml_dtypes.bfloat16
_CACHE = {}


def _flags(inputs):
    z = lambda a: bool(np.all(np.asarray(a) == 0))
    o = lambda a: bool(np.all(np.asarray(a) == 1))
    return {
        "bqk": z(inputs["bq"]) and z(inputs["bk"]),
        "bv": z(inputs["bv"]),
        "bo": z(inputs["bo"]),
        "bi": z(inputs["bi"]),
        "bo2": z(inputs["bo2"]),
        "ln": all(o(inputs[k]) for k in ("emb_ln_s", "ln1_s", "ln2_s"))
        and all(z(inputs[k]) for k in ("emb_ln_b", "ln1_b", "ln2_b")),
        "cb": z(inputs["conv_b1"]) and z(inputs["conv_b2"]) and z(inputs["conv_b3"]),
    }


def _layout(lens96):
    """Pack layout shared by all cores (program-shaping constants)."""
    lens96 = np.asarray(lens96, dtype=np.int64)
    order = np.argsort(-lens96, kind="stable")
    assign = order.reshape(NSL, NCORES)           # [slot, core] -> seq idx
    lam = lens96[assign].max(1).astype(int)       # slot budgets (max len)
    # 64-aligned slot starts (PE col/row tiling only supports base 0/64 for
    # >32-wide tiles); bump a start to the next 128 boundary when the slot
    # would otherwise span 3 token tiles (score/eT tiles hold 2 chunks)
    lam32 = ((lam + 63) // 64) * 64
    off = np.zeros(NSL + 1, np.int64)
    for j in range(NSL):
        o = off[j]
        if (o % 128) + lam[j] > 256:
            o = ((o + 127) // 128) * 128
            off[j] = o
        off[j + 1] = o + lam32[j]
    T32 = int(off[-1])
    NTT = -(-T32 // 128)
    TP = NTT * 128
    coff = np.zeros(NSL + 1, np.int64)
    coff[1:] = np.cumsum(lam + 2)
    CW = int(coff[-1])

    def greedy(offs, cap):
        groups, cur = [], [0]
        for j in range(1, NSL):
            if offs[j + 1] - offs[cur[0]] <= cap:
                cur.append(j)
            else:
                groups.append(cur)
                cur = [j]
        groups.append(cur)
        return groups

    qk_groups = greedy(off, 512)
    cv_groups = greedy(coff, 512)

    # per-slot key/query chunks: intersections with the global 128 grid
    chunks = []
    for j in range(NSL):
        lo, hi = int(off[j]), int(off[j] + lam[j])
        ch = []
        p = lo
        while p < hi:
            nt = p // 128
            e = min(hi, (nt + 1) * 128)
            ch.append((nt, p - nt * 128, e - p, p - lo))  # (tile, base, width, rel)
            p = e
        chunks.append(ch)
    return dict(
        assign=assign, lam=[int(x) for x in lam], lam32=[int(x) for x in lam32],
        off=[int(x) for x in off], coff=[int(x) for x in coff],
        T32=T32, NTT=NTT, TP=TP, CW=CW, CWP=CW + 2,
        qk_groups=qk_groups, cv_groups=cv_groups, chunks=chunks,
        key=(tuple(int(x) for x in lam), tuple(int(x) for x in lam32)),
    )


def _build_program(fl, lay, debug=False):
    import contextlib
    import concourse.bass as bass
    import concourse.mybir as mybir
    import concourse.tile as tile
    from concourse import bacc

    F32, BF16, I32 = mybir.dt.float32, mybir.dt.bfloat16, mybir.dt.int32
    AL, AF = mybir.AluOpType, mybir.ActivationFunctionType

    NTT, TP, CWP = lay["NTT"], lay["TP"], lay["CWP"]
    lam, lam32, off, coff = lay["lam"], lay["lam32"], lay["off"], lay["coff"]
    chunks = lay["chunks"]

    nc = bacc.Bacc("TRN2", target_bir_lowering=False, debug=False,
                   num_devices=NCORES)

    di = lambda n, s, d: nc.dram_tensor(n, s, d, kind="ExternalInput").ap()
    word = di("word_emb", [V, D], F32)
    ids_d = di("ids", [NTT, 128], I32)
    vm_d = di("vmask", [NTT, 128], F32)
    posty_d = di("posty", [NTT, 128, D], F32)
    cm_d = di("convmask", [CWP], BF16)
    cpen_d = di("convpen", [3, CWP], F32)
    wq_d = [di(f"wq{l}", [ND, 128, D], BF16) for l in range(NL)]
    wk_d = [di(f"wk{l}", [ND, 128, D], BF16) for l in range(NL)]
    wv_d = [di(f"wv{l}", [ND, 128, D], BF16) for l in range(NL)]
    wo_d = [di(f"wo{l}", [ND, 128, D], BF16) for l in range(NL)]
    wi_d = [di(f"wi{l}", [ND, 128, FF], BF16) for l in range(NL)]
    wo2_d = [di(f"wo2{l}", [NFT, 128, D], BF16) for l in range(NL)]
    cw_d = [di(f"cw{k}", [k, ND, 128, NF], BF16) for k in (1, 2, 3)]
    fcw_d = di("fcw", [6, 128, 3 * NCLS], F32)
    if not fl["bqk"]:
        bq_d = [di(f"bq{l}", [ND, 128], F32) for l in range(NL)]
        bk_d = [di(f"bk{l}", [ND, 128], F32) for l in range(NL)]
    if not fl["bv"]:
        bv_d = [di(f"bv{l}", [D], F32) for l in range(NL)]
    if not fl["bo"]:
        bo_d = [di(f"bo{l}", [D], F32) for l in range(NL)]
    if not fl["bi"]:
        bi_d = [di(f"bi{l}", [NFT, 128], F32) for l in range(NL)]
    if not fl["bo2"]:
        bo2_d = [di(f"bo2{l}", [D], F32) for l in range(NL)]
    if not fl["ln"]:
        elns_d = di("lnes", [D], F32)
        elnb_d = di("lneb", [D], F32)
        ln1s_d = [di(f"ln1s{l}", [D], F32) for l in range(NL)]
        ln1b_d = [di(f"ln1b{l}", [D], F32) for l in range(NL)]
        ln2s_d = [di(f"ln2s{l}", [D], F32) for l in range(NL)]
        ln2b_d = [di(f"ln2b{l}", [D], F32) for l in range(NL)]
    if not fl["cb"]:
        cb_d = di("convb", [3, 2, 128], F32)

    out_d = nc.dram_tensor("out", [NSL, 3 * NCLS], F32, kind="ExternalOutput").ap()
    if debug:
        dbgx_d = nc.dram_tensor("dbgx", [NTT, 128, D], F32,
                                kind="ExternalOutput").ap()

    with tile.TileContext(nc) as tc, contextlib.ExitStack() as ctx:
        consts = ctx.enter_context(tc.tile_pool(name="consts", bufs=1))
        state = ctx.enter_context(tc.tile_pool(name="state", bufs=1))
        wts = ctx.enter_context(tc.tile_pool(name="wts", bufs=1))
        qkp = ctx.enter_context(tc.tile_pool(name="qkp", bufs=2))
        etp = ctx.enter_context(tc.tile_pool(name="etp", bufs=2))
        htp = ctx.enter_context(tc.tile_pool(name="htp", bufs=1))
        work = ctx.enter_context(tc.tile_pool(name="work", bufs=2))
        cxp = ctx.enter_context(tc.tile_pool(name="cxp", bufs=3))
        small = ctx.enter_context(tc.tile_pool(name="small", bufs=4))
        ps_mm = ctx.enter_context(tc.tile_pool(name="ps_mm", bufs=3, space="PSUM"))
        ps_s = ctx.enter_context(tc.tile_pool(name="ps_s", bufs=3, space="PSUM"))
        ps_c = ctx.enter_context(tc.tile_pool(name="ps_c", bufs=2, space="PSUM"))

        # ---- constants ----
        eps_t = consts.tile([128, 1], F32, tag="eps")
        nc.vector.memset(eps_t[:], 1e-12)
        ones8 = consts.tile([128, H], BF16, tag="ones8")
        nc.vector.memset(ones8[:], 1.0)
        ids_sb = consts.tile([128, NTT], I32, tag="ids")
        nc.sync.dma_start(out=ids_sb[:], in_=ids_d.rearrange("t p -> p t"))
        vm_sb = consts.tile([128, NTT], F32, tag="vm")
        nc.sync.dma_start(out=vm_sb[:], in_=vm_d.rearrange("t p -> p t"))
        cm_sb = consts.tile([128, CWP], BF16, tag="cm")
        nc.sync.dma_start(out=cm_sb[:],
                          in_=cm_d[None, :].to_broadcast([128, CWP]))
        fcw_sb = consts.tile([128, 6, 3 * NCLS], F32, tag="fcw")
        nc.sync.dma_start(out=fcw_sb[:], in_=fcw_d.rearrange("c p n -> p c n"))
        bcast = lambda ap, n: ap[None, :].to_broadcast([128, n])
        if not fl["ln"]:
            elns = consts.tile([128, D], F32, tag="elns")
            nc.sync.dma_start(out=elns[:], in_=bcast(elns_d, D))
            elnb = consts.tile([128, D], F32, tag="elnb")
            nc.sync.dma_start(out=elnb[:], in_=bcast(elnb_d, D))
        if not fl["cb"]:
            cb = consts.tile([128, 3, 2], F32, tag="cb")
            nc.sync.dma_start(out=cb[:], in_=cb_d.rearrange("k t p -> p k t"))

        cw = {}
        for ki, k in enumerate((1, 2, 3)):
            for j in range(k):
                t = wts.tile([128, ND, NF], BF16, tag=f"cw{ki}_{j}")
                nc.sync.dma_start(out=t[:],
                                  in_=cw_d[ki][j].rearrange("t p f -> p t f"))
                cw[(k, j)] = t

        # ---- persistent state ----
        x_tm = state.tile([128, NTT, D], BF16, tag="x_tm")
        xTa = state.tile([128, ND, TP], BF16, tag="xTa")
        ctxT = state.tile([128, ND, TP], BF16, tag="ctxT")
        vA = state.tile([128, NTT, H, DH1], BF16, tag="vA")
        rep = state.tile([128, 6, NSL], F32, tag="rep")

        def load_layer_weights(l, w=None, part="all"):
            if w is None:
                w = {}
            names = {"qk": ("wq", "wk"), "rest": ("wv", "wo", "wi"),
                     "all": ("wq", "wk", "wv", "wo", "wi")}[part]
            for nm, dd, nfree in (("wq", wq_d, D), ("wk", wk_d, D),
                                  ("wv", wv_d, D), ("wo", wo_d, D),
                                  ("wi", wi_d, FF)):
                if nm not in names:
                    continue
                w[nm] = wts.tile([128, ND, nfree], BF16, tag=nm, name=f"{nm}_{l}",
                                 bufs=2 if nm in ("wq", "wk") else 1)
                for dt in range(ND):
                    nc.sync.dma_start(out=w[nm][:, dt, :], in_=dd[l][dt])
            if part == "qk":
                return w
            w["wo2"] = wts.tile([128, NFT, D], BF16, tag="wo2", name=f"wo2_{l}")
            for ft in range(0, NFT, 4):
                nc.sync.dma_start(
                    out=w["wo2"][:, ft:ft + 4, :],
                    in_=wo2_d[l][ft:ft + 4].rearrange("t p o -> p t o"))
            if not fl["bqk"]:
                w["bq"] = consts.tile([128, ND], F32, tag="bq", name=f"bq_{l}")
                nc.sync.dma_start(out=w["bq"][:], in_=bq_d[l].rearrange("t p -> p t"))
                w["bk"] = consts.tile([128, ND], F32, tag="bk", name=f"bk_{l}")
                nc.sync.dma_start(out=w["bk"][:], in_=bk_d[l].rearrange("t p -> p t"))
            if not fl["bv"]:
                w["bv"] = consts.tile([128, D], F32, tag="bv", name=f"bv_{l}")
                nc.sync.dma_start(out=w["bv"][:], in_=bcast(bv_d[l], D))
            if not fl["bo"]:
                w["bo"] = consts.tile([128, D], F32, tag="bo", name=f"bo_{l}")
                nc.sync.dma_start(out=w["bo"][:], in_=bcast(bo_d[l], D))
            if not fl["bi"]:
                w["bi"] = consts.tile([128, NFT], F32, tag="bi", name=f"bi_{l}")
                nc.sync.dma_start(out=w["bi"][:], in_=bi_d[l].rearrange("t p -> p t"))
            if not fl["bo2"]:
                w["bo2"] = consts.tile([128, D], F32, tag="bo2", name=f"bo2_{l}")
                nc.sync.dma_start(out=w["bo2"][:], in_=bcast(bo2_d[l], D))
            if not fl["ln"]:
                for nm, dd in (("ln1s", ln1s_d), ("ln1b", ln1b_d),
                               ("ln2s", ln2s_d), ("ln2b", ln2b_d)):
                    w[nm] = consts.tile([128, D], F32, tag=nm, name=f"{nm}_{l}")
                    nc.sync.dma_start(out=w[nm][:], in_=bcast(dd[l], D))
            return w

        def ln_start(src_ap, i=0):
            st = small.tile([128, 6], F32, tag="st", name=f"st{i}")
            mv = small.tile([128, 2], F32, tag="mv", name=f"mv{i}")
            nc.vector.bn_stats(out=st[:], in_=src_ap)
            nc.vector.bn_aggr(out=mv[:], in_=st[:])
            sd = small.tile([128, 1], F32, tag="sd", name=f"sd{i}")
            nc.scalar.activation(out=sd[:], in_=mv[:, 1:2], func=AF.Sqrt,
                                 bias=eps_t[:], scale=1.0)
            nmr = small.tile([128, 2], F32, tag="nmr", name=f"nmr{i}")
            nc.vector.reciprocal(out=nmr[:, 1:2], in_=sd[:])
            nc.vector.tensor_scalar(out=nmr[:, 0:1], in0=mv[:, 0:1],
                                    scalar1=nmr[:, 1:2], scalar2=-1.0,
                                    op0=AL.mult, op1=AL.mult)
            return nmr

        def ln_apply(src_ap, dst_ap, nmr, s_tile, b_tile):
            nc.vector.tensor_scalar(out=dst_ap, in0=src_ap,
                                    scalar1=nmr[:, 1:2], scalar2=nmr[:, 0:1],
                                    op0=AL.mult, op1=AL.add)
            if s_tile is not None:
                nc.vector.tensor_tensor(out=dst_ap, in0=dst_ap, in1=s_tile[:],
                                        op=AL.mult)
            if b_tile is not None:
                nc.vector.tensor_tensor(out=dst_ap, in0=dst_ap, in1=b_tile[:],
                                        op=AL.add)

        def embed(nt):
            # ACT-free: pre-LN values land in x_tm (bf16), stats collect
            # into the batch buffer; one Sqrt region finalizes all tiles
            g = work.tile([128, D], F32, tag="g", name=f"g{nt}")
            nc.gpsimd.indirect_dma_start(
                out=g[:], out_offset=None, in_=word[:],
                in_offset=bass.IndirectOffsetOnAxis(
                    ap=ids_sb[:, nt:nt + 1], axis=0))
            pt = work.tile([128, D], F32, tag="pt", name=f"pt{nt}")
            nc.sync.dma_start(out=pt[:], in_=posty_d[nt])
            nc.vector.tensor_tensor(out=x_tm[:, nt, :], in0=g[:], in1=pt[:],
                                    op=AL.add)
            st = small.tile([128, 6], F32, tag="st", name=f"est{nt % 4}")
            nc.vector.bn_stats(out=st[:], in_=x_tm[:, nt, :])
            nc.vector.bn_aggr(out=mvb["emb"][:, :, nt], in_=st[:])

        def qk_group(gi, grp, w):
            g0, g1 = off[grp[0]], off[grp[-1] + 1]
            W = g1 - g0
            qkT = qkp.tile([128, 2, ND, 512], BF16, tag="qkT", name=f"qkT{gi}")
            for qi, wt, bt in ((0, w["wq"], "bq"), (1, w["wk"], "bk")):
                for ot in range(ND):
                    ps = ps_mm.tile([128, 512], F32, tag="mm")
                    for dt in range(ND):
                        nc.tensor.matmul(
                            ps[:, 0:W], wt[:, dt, ot * 128:(ot + 1) * 128],
                            xTa[:, dt, g0:g1], start=dt == 0, stop=dt == ND - 1)
                    if fl["bqk"]:
                        nc.vector.tensor_copy(out=qkT[:, qi, ot, 0:W],
                                              in_=ps[:, 0:W])
                    else:
                        nc.scalar.activation(
                            out=qkT[:, qi, ot, 0:W], in_=ps[:, 0:W],
                            func=AF.Identity, bias=w[bt][:, ot:ot + 1], scale=1.0)
            return qkT

        def v_tile(nt, w):
            ps = ps_mm.tile([128, 512], F32, tag="mm")
            for dt in range(ND):
                nc.tensor.matmul(ps[:], xTa[:, dt, nt * 128:(nt + 1) * 128],
                                 w["wv"][:, dt, :], start=dt == 0,
                                 stop=dt == ND - 1)
            if fl["bv"]:
                nc.vector.tensor_scalar_mul(
                    out=vA[:, nt, :, 0:DH],
                    in0=ps.rearrange("p (h d) -> p h d", h=H),
                    scalar1=vm_sb[:, nt:nt + 1])
            else:
                nc.vector.tensor_tensor(
                    out=vA[:, nt, :, 0:DH],
                    in0=ps.rearrange("p (h d) -> p h d", h=H),
                    in1=w["bv"].rearrange("p (h d) -> p h d", h=H), op=AL.add)
                nc.vector.tensor_scalar_mul(
                    out=vA[:, nt, :, 0:DH], in0=vA[:, nt, :, 0:DH],
                    scalar1=vm_sb[:, nt:nt + 1])
            nc.vector.tensor_scalar_mul(
                out=vA[:, nt, :, DH], in0=ones8[:],
                scalar1=vm_sb[:, nt:nt + 1])

        est = {}

        def attn_scores(l, j, qkT, g0):
            L = lam[j]
            ch = chunks[j]
            q0 = off[j] - g0
            eT = etp.tile([128, H, 512], BF16, tag="eT", name=f"eT{l}_{j}")
            est[j] = eT
            for h in range(H):
                hh, dtH = (h % 2) * DH, h // 2
                pss = ps_s.tile([128, 512], F32, tag="s")
                for ci, (nt, b, kw, rel) in enumerate(ch):
                    nc.tensor.matmul(
                        pss[b:b + kw, ci * L:ci * L + L],
                        qkT[hh:hh + DH, 1, dtH, q0 + rel:q0 + rel + kw],
                        qkT[hh:hh + DH, 0, dtH, q0:q0 + L],
                        start=True, stop=True)
                nc.scalar.activation(out=eT[:, h, 0:len(ch) * L],
                                     in_=pss[:, 0:len(ch) * L], func=AF.Exp,
                                     bias=0.0, scale=1.0)

        def attn_ctx(l, j, ctxb_map, last_slot_of):
            L = lam[j]
            ch = chunks[j]
            eT = est.pop(j)
            # ctx per query chunk
            for (qnt, qb, qw, qrel) in ch:
                for hg in range(2):
                    cps = ps_c.tile([128, 4 * DH1], F32, tag="c")
                    for hi in range(4):
                        h = hg * 4 + hi
                        sl = slice(hi * DH1, hi * DH1 + DH1)
                        for ci, (nt, b, kw, rel) in enumerate(ch):
                            nc.tensor.matmul(
                                cps[qb:qb + qw, sl],
                                eT[b:b + kw, h, ci * L + qrel:ci * L + qrel + qw],
                                vA[b:b + kw, nt, h, :],
                                start=ci == 0, stop=ci == len(ch) - 1)
                    if qnt not in ctxb_map:
                        ctxb_map[qnt] = cxp.tile([128, D], BF16, tag="ctxb",
                                                 name=f"cb{l}_{qnt}")
                    ctxb = ctxb_map[qnt]
                    rcp = small.tile([128, 4], F32, tag="rcp")
                    nc.vector.reciprocal(
                        out=rcp[qb:qb + qw, :],
                        in_=cps.rearrange("p (h c) -> p h c", c=DH1)[qb:qb + qw, :, DH])
                    nc.vector.tensor_tensor(
                        out=ctxb.rearrange("p (h d) -> p h d", d=DH)[
                            qb:qb + qw, hg * 4:hg * 4 + 4, :],
                        in0=cps.rearrange("p (h c) -> p h c", c=DH1)[qb:qb + qw, :, 0:DH],
                        in1=rcp[qb:qb + qw, :, None].to_broadcast([qw, 4, DH]),
                        op=AL.mult)
            # flush finished ctxb tiles
            for (qnt, qb, qw, qrel) in ch:
                if last_slot_of.get(qnt) == j:
                    nc.sync.dma_start_transpose(
                        ctxT[:, :, qnt * 128:(qnt + 1) * 128], ctxb_map[qnt][:])

        # deferred-LN machinery: residual-add lands pre-LN values in x_tm
        # (bf16, in place); per-tile bn stats collect into a batch buffer;
        # one Sqrt region per phase finalizes all tiles (ACT table stays
        # resident for Exp/Gelu — each table swap costs ~1.5us)
        mvb = {}
        for ph in ("ln1", "ln2", "emb"):
            mvb[ph] = state.tile([128, 2, NTT], F32, tag=f"mvb_{ph}",
                                 name=f"mvb_{ph}")

        def res_stats(nt, ps, bias_t, ph, i=0):
            nc.vector.tensor_tensor(out=x_tm[:, nt, :], in0=ps[:],
                                    in1=x_tm[:, nt, :], op=AL.add)
            if bias_t is not None:
                nc.vector.tensor_tensor(out=x_tm[:, nt, :], in0=x_tm[:, nt, :],
                                        in1=bias_t[:], op=AL.add)
            st = small.tile([128, 6], F32, tag="st", name=f"st{i}")
            nc.vector.bn_stats(out=st[:], in_=x_tm[:, nt, :])
            nc.vector.bn_aggr(out=mvb[ph][:, :, nt], in_=st[:])

        def ln_finalize(ph, tiles, s_tile, b_tile, lbl):
            """Batched LN finalize for `tiles`: one Sqrt region, DVE applies,
            transposes into xTa. Safe to call per-chunk (mini batches)."""
            tiles = [t for t in tiles]
            if not tiles:
                return
            mv = mvb[ph]
            t0, t1 = min(tiles), max(tiles) + 1
            sdb = small.tile([128, NTT], F32, tag="sdb", name=f"sdb{lbl}")
            nc.scalar.activation(out=sdb[:, t0:t1], in_=mv[:, 1, t0:t1],
                                 func=AF.Sqrt, bias=eps_t[:], scale=1.0)
            rsb = small.tile([128, NTT], F32, tag="rsb", name=f"rsb{lbl}")
            nc.vector.reciprocal(out=rsb[:, t0:t1], in_=sdb[:, t0:t1])
            nmb = small.tile([128, NTT], F32, tag="nmb", name=f"nmb{lbl}")
            nc.vector.tensor_tensor(out=nmb[:, t0:t1], in0=mv[:, 0, t0:t1],
                                    in1=rsb[:, t0:t1], op=AL.mult)
            nc.vector.tensor_scalar_mul(out=nmb[:, t0:t1], in0=nmb[:, t0:t1],
                                        scalar1=-1.0)
            for nt in tiles:
                nc.vector.tensor_scalar(
                    out=x_tm[:, nt, :], in0=x_tm[:, nt, :],
                    scalar1=rsb[:, nt:nt + 1], scalar2=nmb[:, nt:nt + 1],
                    op0=AL.mult, op1=AL.add)
                if s_tile is not None:
                    nc.vector.tensor_tensor(out=x_tm[:, nt, :],
                                            in0=x_tm[:, nt, :],
                                            in1=s_tile[:], op=AL.mult)
                if b_tile is not None:
                    nc.vector.tensor_tensor(out=x_tm[:, nt, :],
                                            in0=x_tm[:, nt, :],
                                            in1=b_tile[:], op=AL.add)
                nc.sync.dma_start_transpose(
                    xTa[:, :, nt * 128:(nt + 1) * 128], x_tm[:, nt, :])

        def wo_stats(nt, w):
            ps = ps_mm.tile([128, 512], F32, tag="mm")
            for dt in range(ND):
                nc.tensor.matmul(ps[:], ctxT[:, dt, nt * 128:(nt + 1) * 128],
                                 w["wo"][:, dt, :], start=dt == 0,
                                 stop=dt == ND - 1)
            res_stats(nt, ps, None if fl["bo"] else w["bo"], "ln1", i=nt % 4)

        def ffn_chunk(l, c0, c1, w):
            Wc = c1 - c0
            hT = htp.tile([128, NFT, 512], BF16, tag="hT", name=f"hT{l}_{c0}")
            for ft in range(NFT):
                ps = ps_mm.tile([128, 512], F32, tag="mm")
                for dt in range(ND):
                    nc.tensor.matmul(
                        ps[:, 0:Wc], w["wi"][:, dt, ft * 128:(ft + 1) * 128],
                        xTa[:, dt, c0:c1], start=dt == 0, stop=dt == ND - 1)
                nc.scalar.activation(
                    out=hT[:, ft, 0:Wc], in_=ps[:, 0:Wc], func=AF.Gelu,
                    bias=0.0 if fl["bi"] else w["bi"][:, ft:ft + 1], scale=1.0)
            for nt in range(c0 // 128, c1 // 128):
                toff = nt * 128 - c0
                ps = ps_mm.tile([128, 512], F32, tag="mm")
                for ft in range(NFT):
                    nc.tensor.matmul(ps[:], hT[:, ft, toff:toff + 128],
                                     w["wo2"][:, ft, :], start=ft == 0,
                                     stop=ft == NFT - 1)
                res_stats(nt, ps, None if fl["bo2"] else w["bo2"], "ln2",
                          i=nt % 4)

        # last slot writing each ctx tile (for flush scheduling)
        last_slot_of = {}
        for j in range(NSL):
            for (nt, b, kw, rel) in chunks[j]:
                last_slot_of[nt] = j

        # ---- conv head, interleaved with layer-3 FFN ----
        # xcv reuses ctxT's slot (attention is done by then); slot assembly
        # runs on GPSIMD (idle) as its tiles finish LN2; a conv group's
        # matmuls+maxpools run once all its slots are assembled.
        conv_st = {"xcv": None, "asm": set(), "grp": set()}

        def conv_ready(max_tile):
            if conv_st["xcv"] is None:
                conv_st["xcv"] = state.tile([128, ND, CWP], BF16, tag="ctxT",
                                            name="xcv")
                nc.gpsimd.memset(conv_st["xcv"][:], 0.0)
            xcv = conv_st["xcv"]
            for j in range(NSL):
                if j in conv_st["asm"]:
                    continue
                if max(nt for (nt, b, kw, rel) in chunks[j]) > max_tile:
                    continue
                conv_st["asm"].add(j)
                o0, c0 = off[j], coff[j]
                for dt in range(ND):
                    nc.gpsimd.tensor_tensor(
                        out=xcv[:, dt, c0:c0 + lam[j]],
                        in0=xTa[:, dt, o0:o0 + lam[j]],
                        in1=cm_sb[:, c0:c0 + lam[j]], op=AL.mult)
            for cgi, cg in enumerate(lay["cv_groups"]):
                if cgi in conv_st["grp"]:
                    continue
                if not all(j in conv_st["asm"] for j in cg):
                    continue
                conv_st["grp"].add(cgi)
                cs, ce = coff[cg[0]], coff[cg[-1] + 1]
                Wg = ce - cs
                for ki, k in enumerate((1, 2, 3)):
                    pen = work.tile([128, 512], F32, tag="g",
                                    name=f"pn{cgi}_{ki}")
                    nc.sync.dma_start(
                        out=pen[:, 0:Wg],
                        in_=cpen_d[ki, cs:ce][None, :].to_broadcast([128, Wg]))
                    for ft in range(2):
                        ps = ps_mm.tile([128, 512], F32, tag="mm")
                        idx = 0
                        for dt in range(ND):
                            for jj in range(k):
                                nc.tensor.matmul(
                                    ps[:, 0:Wg],
                                    cw[(k, jj)][:, dt, ft * 128:(ft + 1) * 128],
                                    xcv[:, dt, cs + jj:cs + jj + Wg],
                                    start=idx == 0, stop=idx == ND * k - 1)
                                idx += 1
                        cvt = work.tile([128, 512], F32, tag="pt",
                                        name=f"cv{cgi}_{ki}_{ft}")
                        nc.vector.tensor_tensor(out=cvt[:, 0:Wg],
                                                in0=ps[:, 0:Wg],
                                                in1=pen[:, 0:Wg], op=AL.add)
                        for j in cg:
                            rs = coff[j] - cs
                            re = rs + lam[j] - k + 2
                            nc.vector.tensor_reduce(
                                out=rep[:, ki * 2 + ft, j:j + 1],
                                in_=cvt[:, rs:re],
                                axis=mybir.AxisListType.X, op=AL.max)

        NCH = TP // 512 + (1 if TP % 512 else 0)
        chunk_rng = [(ci * 512, min((ci + 1) * 512, TP)) for ci in range(NCH)]

        # tiles first touched by each qk group (for embed/V scheduling)
        emb_done = set()

        def new_tiles(grp):
            g0, g1 = off[grp[0]], off[grp[-1] + 1]
            ts = [t for t in range(g0 // 128, -(-g1 // 128)) if t not in emb_done]
            emb_done.update(ts)
            return ts

        # ---- program ----
        # All embeds first (gathers + posty DMAs ahead of the big weight
        # DMAs in the queues), in group order so group 0 finishes first.
        for grp in lay["qk_groups"]:
            for nt in new_tiles(grp):
                embed(nt)
        for nt in range(TP // 128):
            if nt not in emb_done:
                emb_done.add(nt)
                embed(nt)
        w_cur = load_layer_weights(0, part="qk")
        load_layer_weights(0, w=w_cur, part="rest")
        ln_finalize("emb", range(NTT),
                    None if fl["ln"] else elns,
                    None if fl["ln"] else elnb, "emb")
        nc.vector.memset(ctxT[:], 0.0)
        for l in range(NL):
            ctxb_map = {}
            v_done = set()
            wo_done = set()
            fin1_done = set()
            ln1s = None if fl["ln"] else w_cur["ln1s"]
            ln1b = None if fl["ln"] else w_cur["ln1b"]
            ln2s = None if fl["ln"] else w_cur["ln2s"]
            ln2b = None if fl["ln"] else w_cur["ln2b"]

            def flush(j):
                # Wo matmuls + residual/stats (no ACT) for tiles whose
                # attention is complete — dense PE filler between the
                # ACT-paced softmax chains of consecutive slots. Chunk 0's
                # LN1 finalize runs mid-attention so FFN can start with
                # zero bubble at the phase boundary.
                for nt in range(NTT):
                    if nt in wo_done or last_slot_of.get(nt, -1) > j:
                        continue
                    if nt not in ctxb_map and nt in last_slot_of:
                        continue  # not yet computed this pass
                    wo_done.add(nt)
                    wo_stats(nt, w_cur)
                if 0 not in fin1_done and all(
                        t in wo_done for t in range(4)):
                    fin1_done.add(0)
                    ln_finalize("ln1", range(4), ln1s, ln1b, f"a{l}c0")

            pend = None
            for gi, grp in enumerate(lay["qk_groups"]):
                qkT = qk_group(gi, grp, w_cur)
                for j in grp:
                    for (nt, b, kw, rel) in chunks[j]:
                        if nt not in v_done:
                            v_done.add(nt)
                            v_tile(nt, w_cur)
                    # software pipeline: slot j's scores (and Wo filler)
                    # are emitted before slot j-1's ctx matmuls so the PE
                    # has work while the ACT exp chain for j-1 completes
                    attn_scores(l, j, qkT, off[grp[0]])
                    if pend is not None:
                        if pend >= 1:
                            flush(pend - 1)
                        attn_ctx(l, pend, ctxb_map, last_slot_of)
                    pend = j
                if gi == 0 and l + 1 < NL:
                    w_nxt = load_layer_weights(l + 1, part="qk")
            attn_ctx(l, pend, ctxb_map, last_slot_of)
            if l + 1 < NL:
                load_layer_weights(l + 1, w=w_nxt, part="rest")
            flush(NSL)
            if 0 not in fin1_done:
                ln_finalize("ln1", range(4), ln1s, ln1b, f"a{l}c0")
            ln_finalize("ln1", range(4, NTT), ln1s, ln1b, f"a{l}r")
            for ci, (c0, c1) in enumerate(chunk_rng):
                ffn_chunk(l, c0, c1, w_cur)
                if ci == 0:
                    # next layer's first QK group depends only on these
                    ln_finalize("ln2", range(4), ln2s, ln2b, f"f{l}c0")
                    if l == NL - 1:
                        conv_ready(3)
                if l == NL - 1 and ci > 0:
                    ln_finalize("ln2", range(ci * 4, min(ci * 4 + 4, NTT)),
                                ln2s, ln2b, f"f{l}c{ci}")
                    conv_ready(ci * 4 + 3)
            if l < NL - 1:
                ln_finalize("ln2", range(4, NTT), ln2s, ln2b, f"f{l}r")
                w_cur = w_nxt
            else:
                conv_ready(NTT)

        if debug:
            for nt in range(NTT):
                dx = work.tile([128, D], F32, tag="dbg", name=f"dbg{nt}")
                nc.vector.tensor_copy(out=dx[:], in_=x_tm[:, nt, :])
                nc.sync.dma_start(out=dbgx_d[nt], in_=dx[:])

        if not fl["cb"]:
            for ki in range(3):
                for ft in range(2):
                    nc.vector.tensor_scalar_add(
                        out=rep[:, ki * 2 + ft, :], in0=rep[:, ki * 2 + ft, :],
                        scalar1=cb[:, ki, ft:ft + 1])
        nc.scalar.activation(out=rep[:], in_=rep[:], func=AF.Relu)

        fps = ps_c.tile([128, 3 * NCLS], F32, tag="c")
        for c in range(6):
            nc.tensor.matmul(fps[:NSL, :], rep[:, c, :], fcw_sb[:, c, :],
                             start=c == 0, stop=c == 5)
        ob = small.tile([NSL, 3 * NCLS], F32, tag="ob")
        nc.scalar.copy(out=ob[:], in_=fps[:NSL, :])
        nc.sync.dma_start(out=out_d[:], in_=ob[:])

    nc.compile()
    return nc


def _core_inputs(inputs, fl, lay):
    """Build the 8 per-core input maps from the full problem inputs."""
    f32 = lambda a: np.ascontiguousarray(np.asarray(a, dtype=np.float32))
    tile_w = lambda w: np.ascontiguousarray(
        f32(w).reshape(w.shape[0] // 128, 128, w.shape[1]).astype(BF))

    NTT, TP, CWP = lay["NTT"], lay["TP"], lay["CWP"]
    lam, lam32, off, coff = lay["lam"], lay["lam32"], lay["off"], lay["coff"]
    assign = lay["assign"]

    shared = {}
    # packed position+type embedding
    posv = np.zeros((TP, D), np.float32)
    pe = f32(inputs["pos_emb"])
    for j in range(NSL):
        posv[off[j]:off[j] + lam32[j]] = pe[:lam32[j]]
    posv += f32(inputs["type_emb"][0])[None, :]
    shared["posty"] = np.ascontiguousarray(posv.reshape(NTT, 128, D))
    for l in range(NL):
        shared[f"wq{l}"] = tile_w(f32(inputs["Wq"][l]) / 8.0)
        shared[f"wk{l}"] = tile_w(inputs["Wk"][l])
        shared[f"wv{l}"] = tile_w(inputs["Wv"][l])
        shared[f"wo{l}"] = tile_w(inputs["Wo"][l])
        shared[f"wi{l}"] = tile_w(inputs["Wi"][l])
        shared[f"wo2{l}"] = tile_w(inputs["Wo2"][l])
        if not fl["bqk"]:
            shared[f"bq{l}"] = f32(inputs["bq"][l]).reshape(ND, 128) / 8.0
            shared[f"bk{l}"] = f32(inputs["bk"][l]).reshape(ND, 128)
        if not fl["bv"]:
            shared[f"bv{l}"] = f32(inputs["bv"][l])
        if not fl["bo"]:
            shared[f"bo{l}"] = f32(inputs["bo"][l])
        if not fl["bi"]:
            shared[f"bi{l}"] = f32(inputs["bi"][l]).reshape(NFT, 128)
        if not fl["bo2"]:
            shared[f"bo2{l}"] = f32(inputs["bo2"][l])
        if not fl["ln"]:
            shared[f"ln1s{l}"] = f32(inputs["ln1_s"][l])
            shared[f"ln1b{l}"] = f32(inputs["ln1_b"][l])
            shared[f"ln2s{l}"] = f32(inputs["ln2_s"][l])
            shared[f"ln2b{l}"] = f32(inputs["ln2_b"][l])
    if not fl["ln"]:
        shared["lnes"] = f32(inputs["emb_ln_s"])
        shared["lneb"] = f32(inputs["emb_ln_b"])
    for ki, k in enumerate((1, 2, 3)):
        w = f32(inputs[f"conv_w{k}"])                    # [NF, k, D]
        wt = np.ascontiguousarray(w.transpose(1, 2, 0))  # [k, D, NF]
        shared[f"cw{k}"] = np.ascontiguousarray(
            wt.reshape(k, ND, 128, NF).astype(BF))
    if not fl["cb"]:
        shared["convb"] = np.stack(
            [f32(inputs[f"conv_b{k}"]).reshape(2, 128) for k in (1, 2, 3)])
    # fc weights for all 3 branch-block hypotheses: [6, 128, 3*NCLS]
    fcw = f32(inputs["fc_w"])                            # [2304, NCLS]
    fcw3 = np.zeros((6, 128, 3 * NCLS), np.float32)
    for bb in range(3):
        for ki in range(3):
            for ft in range(2):
                c = ki * 2 + ft
                rows = 768 * bb + 256 * ki + 128 * ft
                fcw3[c, :, bb * NCLS:(bb + 1) * NCLS] = fcw[rows:rows + 128]
    shared["fcw"] = fcw3
    shared["word_emb"] = f32(inputs["word_emb"])

    ids_all = np.stack([np.asarray(inputs[p + "_input_ids"])
                        for p in ("q", "a", "b")]).reshape(96, S)
    mask_all = np.stack([np.asarray(inputs[p + "_attention_mask"])
                         for p in ("q", "a", "b")]).reshape(96, S)
    lens_all = mask_all.sum(1).astype(int)

    in_maps = []
    for c in range(NCORES):
        m = dict(shared)
        idv = np.zeros(TP, np.int32)
        vmv = np.zeros(TP, np.float32)
        cmv = np.zeros(CWP, np.float32)
        pen = np.full((3, CWP), -1e30, np.float32)
        for j in range(NSL):
            sq = int(assign[j, c])
            l = int(lens_all[sq])
            idv[off[j]:off[j] + lam32[j]] = ids_all[sq][:lam32[j]]
            vmv[off[j]:off[j] + l] = 1.0
            cmv[coff[j]:coff[j] + l] = 1.0
            for ki, k in enumerate((1, 2, 3)):
                nw = l - k + 2
                pen[ki, coff[j]:coff[j] + nw] = 0.0
        m["ids"] = np.ascontiguousarray(idv.reshape(NTT, 128))
        m["vmask"] = np.ascontiguousarray(vmv.reshape(NTT, 128))
        m["convmask"] = np.ascontiguousarray(cmv.astype(BF))
        m["convpen"] = np.ascontiguousarray(pen)
        in_maps.append(m)
    return in_maps


def _get_program(fl, lay, debug=False):
    key = (tuple(sorted(fl.items())), lay["key"], debug)
    if key not in _CACHE:
        _CACHE[key] = _build_program(fl, lay, debug=debug)
    return _CACHE[key]


def run_sharded(inputs, debug=False, **run_kwargs):
    """Shard, run on 8 cores, gather. Returns (output, BassKernelResults)."""
    from concourse.bass_utils import run_bass_kernel_spmd
    fl = _flags(inputs)
    lens96 = np.concatenate([
        np.asarray(inputs[p + "_attention_mask"]).sum(1) for p in ("q", "a", "b")])
    lay = _layout(lens96)
    nc = _get_program(fl, lay, debug=debug)
    in_maps = _core_inputs(inputs, fl, lay)
    res = run_bass_kernel_spmd(nc, in_maps, core_ids=list(range(NCORES)),
                               **run_kwargs)
    border = {0: 0, 1: 2, 2: 1}   # branch q/a/b -> fc block q,b,a
    out = np.zeros((B, NCLS), np.float32)
    for c in range(NCORES):
        o3 = np.asarray(res.results[c]["out"], np.float32)   # [NSL, 12]
        for j in range(NSL):
            sq = int(lay["assign"][j, c])
            br, sample = sq // B, sq % B
            out[sample] += o3[j, border[br] * NCLS:(border[br] + 1) * NCLS]
    out += np.asarray(inputs["fc_b"], np.float32)[None, :]
    return out, res


def kernel(**inputs):
    out, _ = run_sharded(inputs)
    return out


# revision 40
# speedup vs baseline: 1.8696x; 1.0010x over previous
"""Trainium2 Bass kernel for nn_BertCNN (3x BERT-small encoder + CNN maxpool head).

Ragged-packed data-parallel strategy. The 96 sequences (3 branches x 32
samples) are sorted by actual length (from the attention mask), dealt
round-robin into 8 cores x 12 slots, and each core packs its 12 sequences
into one ~1900-token stream (slot budgets = max length in each rank group,
32-aligned starts). All encoder linear ops (QKV/O/FFN/conv) run over the
packed stream; attention runs per-slot with exact budget widths; key
validity is folded multiplicatively into V (invalid keys get zero V rows
and a zero softmax-denominator contribution), so no attention bias is
needed. The conv head runs over a separately packed layout with 2-token
gaps; per-slot maxpool ranges and window-validity penalties come from the
host. The fc output is computed for all 3 branch hypotheses per slot
([12, 12] per core) and the host scatter-adds the right 4 columns into the
final [32, 4].

The Bass program depends only on the slot-budget layout (not on per-core
data); it is built once per layout signature and cached.
"""

import numpy as np
import ml_dtypes

V, D, H, DH, NL, FF = 30522, 512, 8, 64, 4, 2048
NF, NCLS, B, S = 256, 4, 32, 256
NCORES = 8
NSL = 12                 # slots (sequences) per core
NSEQ = NSL               # test.py compat
SPC = 4                  # test.py compat
ND = D // 128
NFT = FF // 128
DH1 = DH + 1

BF = ml_dtypes.bfloat16
_CACHE = {}


def _flags(inputs):
    z = lambda a: bool(np.all(np.asarray(a) == 0))
    o = lambda a: bool(np.all(np.asarray(a) == 1))
    return {
        "bqk": z(inputs["bq"]) and z(inputs["bk"]),
        "bv": z(inputs["bv"]),
        "bo": z(inputs["bo"]),
        "bi": z(inputs["bi"]),
        "bo2": z(inputs["bo2"]),
        "ln": all(o(inputs[k]) for k in ("emb_ln_s", "ln1_s", "ln2_s"))
        and all(z(inputs[k]) for k in ("emb_ln_b", "ln1_b", "ln2_b")),
        "cb": z(inputs["conv_b1"]) and z(inputs["conv_b2"]) and z(inputs["conv_b3"]),
    }


def _layout(lens96):
    """Pack layout shared by all cores (program-shaping constants)."""
    lens96 = np.asarray(lens96, dtype=np.int64)
    order = np.argsort(-lens96, kind="stable")
    assign = order.reshape(NSL, NCORES)           # [slot, core] -> seq idx
    lam = lens96[assign].max(1).astype(int)       # slot budgets (max len)
    # 64-aligned slot starts (PE col/row tiling only supports base 0/64 for
    # >32-wide tiles); bump a start to the next 128 boundary when the slot
    # would otherwise span 3 token tiles (score/eT tiles hold 2 chunks)
    lam32 = ((lam + 63) // 64) * 64
    off = np.zeros(NSL + 1, np.int64)
    for j in range(NSL):
        o = off[j]
        if (o % 128) + lam[j] > 256:
            o = ((o + 127) // 128) * 128
            off[j] = o
        off[j + 1] = o + lam32[j]
    T32 = int(off[-1])
    NTT = -(-T32 // 128)
    TP = NTT * 128
    coff = np.zeros(NSL + 1, np.int64)
    coff[1:] = np.cumsum(lam + 2)
    CW = int(coff[-1])

    def greedy(offs, cap):
        groups, cur = [], [0]
        for j in range(1, NSL):
            if offs[j + 1] - offs[cur[0]] <= cap:
                cur.append(j)
            else:
                groups.append(cur)
                cur = [j]
        groups.append(cur)
        return groups

    qk_groups = greedy(off, 512)
    cv_groups = greedy(coff, 512)

    # per-slot key/query chunks: intersections with the global 128 grid
    chunks = []
    for j in range(NSL):
        lo, hi = int(off[j]), int(off[j] + lam[j])
        ch = []
        p = lo
        while p < hi:
            nt = p // 128
            e = min(hi, (nt + 1) * 128)
            ch.append((nt, p - nt * 128, e - p, p - lo))  # (tile, base, width, rel)
            p = e
        chunks.append(ch)
    return dict(
        assign=assign, lam=[int(x) for x in lam], lam32=[int(x) for x in lam32],
        off=[int(x) for x in off], coff=[int(x) for x in coff],
        T32=T32, NTT=NTT, TP=TP, CW=CW, CWP=CW + 2,
        qk_groups=qk_groups, cv_groups=cv_groups, chunks=chunks,
        key=(tuple(int(x) for x in lam), tuple(int(x) for x in lam32)),
    )


def _build_program(fl, lay, debug=False):
    import contextlib
    import concourse.bass as bass
    import concourse.mybir as mybir
    import concourse.tile as tile
    from concourse import bacc

    F32, BF16, I32 = mybir.dt.float32, mybir.dt.bfloat16, mybir.dt.int32
    AL, AF = mybir.AluOpType, mybir.ActivationFunctionType

    NTT, TP, CWP = lay["NTT"], lay["TP"], lay["CWP"]
    lam, lam32, off, coff = lay["lam"], lay["lam32"], lay["off"], lay["coff"]
    chunks = lay["chunks"]

    nc = bacc.Bacc("TRN2", target_bir_lowering=False, debug=False,
                   num_devices=NCORES)

    di = lambda n, s, d: nc.dram_tensor(n, s, d, kind="ExternalInput").ap()
    word = di("word_emb", [V, D], F32)
    ids_d = di("ids", [NTT, 128], I32)
    vm_d = di("vmask", [NTT, 128], F32)
    posty_d = di("posty", [NTT, 128, D], F32)
    cm_d = di("convmask", [CWP], BF16)
    cpen_d = di("convpen", [3, CWP], F32)
    wq_d = [di(f"wq{l}", [ND, 128, D], BF16) for l in range(NL)]
    wk_d = [di(f"wk{l}", [ND, 128, D], BF16) for l in range(NL)]
    wv_d = [di(f"wv{l}", [ND, 128, D], BF16) for l in range(NL)]
    wo_d = [di(f"wo{l}", [ND, 128, D], BF16) for l in range(NL)]
    wi_d = [di(f"wi{l}", [ND, 128, FF], BF16) for l in range(NL)]
    wo2_d = [di(f"wo2{l}", [NFT, 128, D], BF16) for l in range(NL)]
    cw_d = [di(f"cw{k}", [k, ND, 128, NF], BF16) for k in (1, 2, 3)]
    fcw_d = di("fcw", [6, 128, 3 * NCLS], F32)
    if not fl["bqk"]:
        bq_d = [di(f"bq{l}", [ND, 128], F32) for l in range(NL)]
        bk_d = [di(f"bk{l}", [ND, 128], F32) for l in range(NL)]
    if not fl["bv"]:
        bv_d = [di(f"bv{l}", [D], F32) for l in range(NL)]
    if not fl["bo"]:
        bo_d = [di(f"bo{l}", [D], F32) for l in range(NL)]
    if not fl["bi"]:
        bi_d = [di(f"bi{l}", [NFT, 128], F32) for l in range(NL)]
    if not fl["bo2"]:
        bo2_d = [di(f"bo2{l}", [D], F32) for l in range(NL)]
    if not fl["ln"]:
        elns_d = di("lnes", [D], F32)
        elnb_d = di("lneb", [D], F32)
        ln1s_d = [di(f"ln1s{l}", [D], F32) for l in range(NL)]
        ln1b_d = [di(f"ln1b{l}", [D], F32) for l in range(NL)]
        ln2s_d = [di(f"ln2s{l}", [D], F32) for l in range(NL)]
        ln2b_d = [di(f"ln2b{l}", [D], F32) for l in range(NL)]
    if not fl["cb"]:
        cb_d = di("convb", [3, 2, 128], F32)

    out_d = nc.dram_tensor("out", [NSL, 3 * NCLS], F32, kind="ExternalOutput").ap()
    if debug:
        dbgx_d = nc.dram_tensor("dbgx", [NTT, 128, D], F32,
                                kind="ExternalOutput").ap()

    with tile.TileContext(nc) as tc, contextlib.ExitStack() as ctx:
        consts = ctx.enter_context(tc.tile_pool(name="consts", bufs=1))
        state = ctx.enter_context(tc.tile_pool(name="state", bufs=1))
        wts = ctx.enter_context(tc.tile_pool(name="wts", bufs=1))
        qkp = ctx.enter_context(tc.tile_pool(name="qkp", bufs=2))
        etp = ctx.enter_context(tc.tile_pool(name="etp", bufs=2))
        htp = ctx.enter_context(tc.tile_pool(name="htp", bufs=1))
        work = ctx.enter_context(tc.tile_pool(name="work", bufs=2))
        cxp = ctx.enter_context(tc.tile_pool(name="cxp", bufs=3))
        small = ctx.enter_context(tc.tile_pool(name="small", bufs=4))
        ps_mm = ctx.enter_context(tc.tile_pool(name="ps_mm", bufs=3, space="PSUM"))
        ps_s = ctx.enter_context(tc.tile_pool(name="ps_s", bufs=3, space="PSUM"))
        ps_c = ctx.enter_context(tc.tile_pool(name="ps_c", bufs=2, space="PSUM"))

        # ---- constants ----
        eps_t = consts.tile([128, 1], F32, tag="eps")
        nc.vector.memset(eps_t[:], 1e-12)
        ones8 = consts.tile([128, H], BF16, tag="ones8")
        nc.vector.memset(ones8[:], 1.0)
        ids_sb = consts.tile([128, NTT], I32, tag="ids")
        nc.sync.dma_start(out=ids_sb[:], in_=ids_d.rearrange("t p -> p t"))
        vm_sb = consts.tile([128, NTT], F32, tag="vm")
        nc.sync.dma_start(out=vm_sb[:], in_=vm_d.rearrange("t p -> p t"))
        cm_sb = consts.tile([128, CWP], BF16, tag="cm")
        nc.sync.dma_start(out=cm_sb[:],
                          in_=cm_d[None, :].to_broadcast([128, CWP]))
        fcw_sb = consts.tile([128, 6, 3 * NCLS], F32, tag="fcw")
        nc.sync.dma_start(out=fcw_sb[:], in_=fcw_d.rearrange("c p n -> p c n"))
        bcast = lambda ap, n: ap[None, :].to_broadcast([128, n])
        if not fl["ln"]:
            elns = consts.tile([128, D], F32, tag="elns")
            nc.sync.dma_start(out=elns[:], in_=bcast(elns_d, D))
            elnb = consts.tile([128, D], F32, tag="elnb")
            nc.sync.dma_start(out=elnb[:], in_=bcast(elnb_d, D))
        if not fl["cb"]:
            cb = consts.tile([128, 3, 2], F32, tag="cb")
            nc.sync.dma_start(out=cb[:], in_=cb_d.rearrange("k t p -> p k t"))

        cw = {}
        for ki, k in enumerate((1, 2, 3)):
            for j in range(k):
                t = wts.tile([128, ND, NF], BF16, tag=f"cw{ki}_{j}")
                nc.sync.dma_start(out=t[:],
                                  in_=cw_d[ki][j].rearrange("t p f -> p t f"))
                cw[(k, j)] = t

        # ---- persistent state ----
        x_tm = state.tile([128, NTT, D], BF16, tag="x_tm")
        xTa = state.tile([128, ND, TP], BF16, tag="xTa")
        ctxT = state.tile([128, ND, TP], BF16, tag="ctxT")
        vA = state.tile([128, NTT, H, DH1], BF16, tag="vA")
        rep = state.tile([128, 6, NSL], F32, tag="rep")

        def load_layer_weights(l, w=None, part="all"):
            if w is None:
                w = {}
            names = {"qk": ("wq", "wk"), "rest": ("wv", "wo", "wi"),
                     "all": ("wq", "wk", "wv", "wo", "wi")}[part]
            for nm, dd, nfree in (("wq", wq_d, D), ("wk", wk_d, D),
                                  ("wv", wv_d, D), ("wo", wo_d, D),
                                  ("wi", wi_d, FF)):
                if nm not in names:
                    continue
                w[nm] = wts.tile([128, ND, nfree], BF16, tag=nm, name=f"{nm}_{l}",
                                 bufs=2 if nm in ("wq", "wk") else 1)
                for dt in range(ND):
                    nc.sync.dma_start(out=w[nm][:, dt, :], in_=dd[l][dt])
            if part == "qk":
                return w
            w["wo2"] = wts.tile([128, NFT, D], BF16, tag="wo2", name=f"wo2_{l}")
            for ft in range(0, NFT, 4):
                nc.sync.dma_start(
                    out=w["wo2"][:, ft:ft + 4, :],
                    in_=wo2_d[l][ft:ft + 4].rearrange("t p o -> p t o"))
            if not fl["bqk"]:
                w["bq"] = consts.tile([128, ND], F32, tag="bq", name=f"bq_{l}")
                nc.sync.dma_start(out=w["bq"][:], in_=bq_d[l].rearrange("t p -> p t"))
                w["bk"] = consts.tile([128, ND], F32, tag="bk", name=f"bk_{l}")
                nc.sync.dma_start(out=w["bk"][:], in_=bk_d[l].rearrange("t p -> p t"))
            if not fl["bv"]:
                w["bv"] = consts.tile([128, D], F32, tag="bv", name=f"bv_{l}")
                nc.sync.dma_start(out=w["bv"][:], in_=bcast(bv_d[l], D))
            if not fl["bo"]:
                w["bo"] = consts.tile([128, D], F32, tag="bo", name=f"bo_{l}")
                nc.sync.dma_start(out=w["bo"][:], in_=bcast(bo_d[l], D))
            if not fl["bi"]:
                w["bi"] = consts.tile([128, NFT], F32, tag="bi", name=f"bi_{l}")
                nc.sync.dma_start(out=w["bi"][:], in_=bi_d[l].rearrange("t p -> p t"))
            if not fl["bo2"]:
                w["bo2"] = consts.tile([128, D], F32, tag="bo2", name=f"bo2_{l}")
                nc.sync.dma_start(out=w["bo2"][:], in_=bcast(bo2_d[l], D))
            if not fl["ln"]:
                for nm, dd in (("ln1s", ln1s_d), ("ln1b", ln1b_d),
                               ("ln2s", ln2s_d), ("ln2b", ln2b_d)):
                    w[nm] = consts.tile([128, D], F32, tag=nm, name=f"{nm}_{l}")
                    nc.sync.dma_start(out=w[nm][:], in_=bcast(dd[l], D))
            return w

        def ln_start(src_ap, i=0):
            st = small.tile([128, 6], F32, tag="st", name=f"st{i}")
            mv = small.tile([128, 2], F32, tag="mv", name=f"mv{i}")
            nc.vector.bn_stats(out=st[:], in_=src_ap)
            nc.vector.bn_aggr(out=mv[:], in_=st[:])
            sd = small.tile([128, 1], F32, tag="sd", name=f"sd{i}")
            nc.scalar.activation(out=sd[:], in_=mv[:, 1:2], func=AF.Sqrt,
                                 bias=eps_t[:], scale=1.0)
            nmr = small.tile([128, 2], F32, tag="nmr", name=f"nmr{i}")
            nc.vector.reciprocal(out=nmr[:, 1:2], in_=sd[:])
            nc.vector.tensor_scalar(out=nmr[:, 0:1], in0=mv[:, 0:1],
                                    scalar1=nmr[:, 1:2], scalar2=-1.0,
                                    op0=AL.mult, op1=AL.mult)
            return nmr

        def ln_apply(src_ap, dst_ap, nmr, s_tile, b_tile):
            nc.vector.tensor_scalar(out=dst_ap, in0=src_ap,
                                    scalar1=nmr[:, 1:2], scalar2=nmr[:, 0:1],
                                    op0=AL.mult, op1=AL.add)
            if s_tile is not None:
                nc.vector.tensor_tensor(out=dst_ap, in0=dst_ap, in1=s_tile[:],
                                        op=AL.mult)
            if b_tile is not None:
                nc.vector.tensor_tensor(out=dst_ap, in0=dst_ap, in1=b_tile[:],
                                        op=AL.add)

        def embed(nt):
            # ACT-free: pre-LN values land in x_tm (bf16), stats collect
            # into the batch buffer; one Sqrt region finalizes all tiles
            g = work.tile([128, D], F32, tag="g", name=f"g{nt}")
            nc.gpsimd.indirect_dma_start(
                out=g[:], out_offset=None, in_=word[:],
                in_offset=bass.IndirectOffsetOnAxis(
                    ap=ids_sb[:, nt:nt + 1], axis=0))
            pt = work.tile([128, D], F32, tag="pt", name=f"pt{nt}")
            nc.sync.dma_start(out=pt[:], in_=posty_d[nt])
            nc.vector.tensor_tensor(out=x_tm[:, nt, :], in0=g[:], in1=pt[:],
                                    op=AL.add)
            st = small.tile([128, 6], F32, tag="st", name=f"est{nt % 4}")
            nc.vector.bn_stats(out=st[:], in_=x_tm[:, nt, :])
            nc.vector.bn_aggr(out=mvb["emb"][:, :, nt], in_=st[:])

        def qk_group(gi, grp, w):
            g0, g1 = off[grp[0]], off[grp[-1] + 1]
            W = g1 - g0
            qkT = qkp.tile([128, 2, ND, 512], BF16, tag="qkT", name=f"qkT{gi}")
            for qi, wt, bt in ((0, w["wq"], "bq"), (1, w["wk"], "bk")):
                for ot in range(ND):
                    ps = ps_mm.tile([128, 512], F32, tag="mm")
                    for dt in range(ND):
                        nc.tensor.matmul(
                            ps[:, 0:W], wt[:, dt, ot * 128:(ot + 1) * 128],
                            xTa[:, dt, g0:g1], start=dt == 0, stop=dt == ND - 1)
                    if fl["bqk"]:
                        nc.vector.tensor_copy(out=qkT[:, qi, ot, 0:W],
                                              in_=ps[:, 0:W])
                    else:
                        nc.scalar.activation(
                            out=qkT[:, qi, ot, 0:W], in_=ps[:, 0:W],
                            func=AF.Identity, bias=w[bt][:, ot:ot + 1], scale=1.0)
            return qkT

        def v_tile(nt, w):
            ps = ps_mm.tile([128, 512], F32, tag="mm")
            for dt in range(ND):
                nc.tensor.matmul(ps[:], xTa[:, dt, nt * 128:(nt + 1) * 128],
                                 w["wv"][:, dt, :], start=dt == 0,
                                 stop=dt == ND - 1)
            if fl["bv"]:
                nc.vector.tensor_scalar_mul(
                    out=vA[:, nt, :, 0:DH],
                    in0=ps.rearrange("p (h d) -> p h d", h=H),
                    scalar1=vm_sb[:, nt:nt + 1])
            else:
                nc.vector.tensor_tensor(
                    out=vA[:, nt, :, 0:DH],
                    in0=ps.rearrange("p (h d) -> p h d", h=H),
                    in1=w["bv"].rearrange("p (h d) -> p h d", h=H), op=AL.add)
                nc.vector.tensor_scalar_mul(
                    out=vA[:, nt, :, 0:DH], in0=vA[:, nt, :, 0:DH],
                    scalar1=vm_sb[:, nt:nt + 1])
            nc.vector.tensor_scalar_mul(
                out=vA[:, nt, :, DH], in0=ones8[:],
                scalar1=vm_sb[:, nt:nt + 1])

        est = {}

        def attn_scores(l, j, qkT, g0):
            L = lam[j]
            ch = chunks[j]
            q0 = off[j] - g0
            eT = etp.tile([128, H, 512], BF16, tag="eT", name=f"eT{l}_{j}")
            est[j] = eT
            for h in range(H):
                hh, dtH = (h % 2) * DH, h // 2
                pss = ps_s.tile([128, 512], F32, tag="s")
                for ci, (nt, b, kw, rel) in enumerate(ch):
                    nc.tensor.matmul(
                        pss[b:b + kw, ci * L:ci * L + L],
                        qkT[hh:hh + DH, 1, dtH, q0 + rel:q0 + rel + kw],
                        qkT[hh:hh + DH, 0, dtH, q0:q0 + L],
                        start=True, stop=True)
                nc.scalar.activation(out=eT[:, h, 0:len(ch) * L],
                                     in_=pss[:, 0:len(ch) * L], func=AF.Exp,
                                     bias=0.0, scale=1.0)

        def attn_ctx(l, j, ctxb_map, last_slot_of):
            L = lam[j]
            ch = chunks[j]
            eT = est.pop(j)
            # ctx per query chunk
            for (qnt, qb, qw, qrel) in ch:
                for hg in range(2):
                    cps = ps_c.tile([128, 4 * DH1], F32, tag="c")
                    for hi in range(4):
                        h = hg * 4 + hi
                        sl = slice(hi * DH1, hi * DH1 + DH1)
                        for ci, (nt, b, kw, rel) in enumerate(ch):
                            nc.tensor.matmul(
                                cps[qb:qb + qw, sl],
                                eT[b:b + kw, h, ci * L + qrel:ci * L + qrel + qw],
                                vA[b:b + kw, nt, h, :],
                                start=ci == 0, stop=ci == len(ch) - 1)
                    if qnt not in ctxb_map:
                        ctxb_map[qnt] = cxp.tile([128, D], BF16, tag="ctxb",
                                                 name=f"cb{l}_{qnt}")
                    ctxb = ctxb_map[qnt]
                    rcp = small.tile([128, 4], F32, tag="rcp")
                    nc.vector.reciprocal(
                        out=rcp[qb:qb + qw, :],
                        in_=cps.rearrange("p (h c) -> p h c", c=DH1)[qb:qb + qw, :, DH])
                    nc.vector.tensor_tensor(
                        out=ctxb.rearrange("p (h d) -> p h d", d=DH)[
                            qb:qb + qw, hg * 4:hg * 4 + 4, :],
                        in0=cps.rearrange("p (h c) -> p h c", c=DH1)[qb:qb + qw, :, 0:DH],
                        in1=rcp[qb:qb + qw, :, None].to_broadcast([qw, 4, DH]),
                        op=AL.mult)
            # flush finished ctxb tiles
            for (qnt, qb, qw, qrel) in ch:
                if last_slot_of.get(qnt) == j:
                    nc.sync.dma_start_transpose(
                        ctxT[:, :, qnt * 128:(qnt + 1) * 128], ctxb_map[qnt][:])

        # deferred-LN machinery: residual-add lands pre-LN values in x_tm
        # (bf16, in place); per-tile bn stats collect into a batch buffer;
        # one Sqrt region per phase finalizes all tiles (ACT table stays
        # resident for Exp/Gelu — each table swap costs ~1.5us)
        mvb = {}
        for ph in ("ln1", "ln2", "emb"):
            mvb[ph] = state.tile([128, 2, NTT], F32, tag=f"mvb_{ph}",
                                 name=f"mvb_{ph}")

        def res_stats(nt, ps, bias_t, ph, i=0):
            nc.vector.tensor_tensor(out=x_tm[:, nt, :], in0=ps[:],
                                    in1=x_tm[:, nt, :], op=AL.add)
            if bias_t is not None:
                nc.vector.tensor_tensor(out=x_tm[:, nt, :], in0=x_tm[:, nt, :],
                                        in1=bias_t[:], op=AL.add)
            st = small.tile([128, 6], F32, tag="st", name=f"st{i}")
            nc.vector.bn_stats(out=st[:], in_=x_tm[:, nt, :])
            nc.vector.bn_aggr(out=mvb[ph][:, :, nt], in_=st[:])

        def ln_finalize(ph, tiles, s_tile, b_tile, lbl):
            """Batched LN finalize for `tiles`: one Sqrt region, DVE applies,
            transposes into xTa. Safe to call per-chunk (mini batches)."""
            tiles = [t for t in tiles]
            if not tiles:
                return
            mv = mvb[ph]
            t0, t1 = min(tiles), max(tiles) + 1
            sdb = small.tile([128, NTT], F32, tag="sdb", name=f"sdb{lbl}")
            nc.scalar.activation(out=sdb[:, t0:t1], in_=mv[:, 1, t0:t1],
                                 func=AF.Sqrt, bias=eps_t[:], scale=1.0)
            rsb = small.tile([128, NTT], F32, tag="rsb", name=f"rsb{lbl}")
            nc.vector.reciprocal(out=rsb[:, t0:t1], in_=sdb[:, t0:t1])
            nmb = small.tile([128, NTT], F32, tag="nmb", name=f"nmb{lbl}")
            nc.vector.tensor_tensor(out=nmb[:, t0:t1], in0=mv[:, 0, t0:t1],
                                    in1=rsb[:, t0:t1], op=AL.mult)
            nc.vector.tensor_scalar_mul(out=nmb[:, t0:t1], in0=nmb[:, t0:t1],
                                        scalar1=-1.0)
            for nt in tiles:
                nc.vector.tensor_scalar(
                    out=x_tm[:, nt, :], in0=x_tm[:, nt, :],
                    scalar1=rsb[:, nt:nt + 1], scalar2=nmb[:, nt:nt + 1],
                    op0=AL.mult, op1=AL.add)
                if s_tile is not None:
                    nc.vector.tensor_tensor(out=x_tm[:, nt, :],
                                            in0=x_tm[:, nt, :],
                                            in1=s_tile[:], op=AL.mult)
                if b_tile is not None:
                    nc.vector.tensor_tensor(out=x_tm[:, nt, :],
                                            in0=x_tm[:, nt, :],
                                            in1=b_tile[:], op=AL.add)
                nc.sync.dma_start_transpose(
                    xTa[:, :, nt * 128:(nt + 1) * 128], x_tm[:, nt, :])

        def wo_stats(nt, w):
            ps = ps_mm.tile([128, 512], F32, tag="mm")
            for dt in range(ND):
                nc.tensor.matmul(ps[:], ctxT[:, dt, nt * 128:(nt + 1) * 128],
                                 w["wo"][:, dt, :], start=dt == 0,
                                 stop=dt == ND - 1)
            res_stats(nt, ps, None if fl["bo"] else w["bo"], "ln1", i=nt % 4)

        def ffn_chunk(l, c0, c1, w):
            Wc = c1 - c0
            hT = htp.tile([128, NFT, 512], BF16, tag="hT", name=f"hT{l}_{c0}")
            for ft in range(NFT):
                ps = ps_mm.tile([128, 512], F32, tag="mm")
                for dt in range(ND):
                    nc.tensor.matmul(
                        ps[:, 0:Wc], w["wi"][:, dt, ft * 128:(ft + 1) * 128],
                        xTa[:, dt, c0:c1], start=dt == 0, stop=dt == ND - 1)
                nc.scalar.activation(
                    out=hT[:, ft, 0:Wc], in_=ps[:, 0:Wc], func=AF.Gelu,
                    bias=0.0 if fl["bi"] else w["bi"][:, ft:ft + 1], scale=1.0)
            for nt in range(c0 // 128, c1 // 128):
                toff = nt * 128 - c0
                ps = ps_mm.tile([128, 512], F32, tag="mm")
                for ft in range(NFT):
                    nc.tensor.matmul(ps[:], hT[:, ft, toff:toff + 128],
                                     w["wo2"][:, ft, :], start=ft == 0,
                                     stop=ft == NFT - 1)
                res_stats(nt, ps, None if fl["bo2"] else w["bo2"], "ln2",
                          i=nt % 4)

        # last slot writing each ctx tile (for flush scheduling)
        last_slot_of = {}
        for j in range(NSL):
            for (nt, b, kw, rel) in chunks[j]:
                last_slot_of[nt] = j

        # ---- conv head, interleaved with layer-3 FFN ----
        # xcv reuses ctxT's slot (attention is done by then); slot assembly
        # runs on GPSIMD (idle) as its tiles finish LN2; a conv group's
        # matmuls+maxpools run once all its slots are assembled.
        conv_st = {"xcv": None, "asm": set(), "grp": set()}

        def conv_ready(max_tile):
            if conv_st["xcv"] is None:
                conv_st["xcv"] = state.tile([128, ND, CWP], BF16, tag="ctxT",
                                            name="xcv")
                nc.gpsimd.memset(conv_st["xcv"][:], 0.0)
            xcv = conv_st["xcv"]
            for j in range(NSL):
                if j in conv_st["asm"]:
                    continue
                if max(nt for (nt, b, kw, rel) in chunks[j]) > max_tile:
                    continue
                conv_st["asm"].add(j)
                o0, c0 = off[j], coff[j]
                for dt in range(ND):
                    eng = nc.vector if dt < 2 else nc.gpsimd
                    eng.tensor_tensor(
                        out=xcv[:, dt, c0:c0 + lam[j]],
                        in0=xTa[:, dt, o0:o0 + lam[j]],
                        in1=cm_sb[:, c0:c0 + lam[j]], op=AL.mult)
            for cgi, cg in enumerate(lay["cv_groups"]):
                if cgi in conv_st["grp"]:
                    continue
                if not all(j in conv_st["asm"] for j in cg):
                    continue
                conv_st["grp"].add(cgi)
                cs, ce = coff[cg[0]], coff[cg[-1] + 1]
                Wg = ce - cs
                for ki, k in enumerate((1, 2, 3)):
                    pen = work.tile([128, 512], F32, tag="g",
                                    name=f"pn{cgi}_{ki}")
                    nc.sync.dma_start(
                        out=pen[:, 0:Wg],
                        in_=cpen_d[ki, cs:ce][None, :].to_broadcast([128, Wg]))
                    for ft in range(2):
                        ps = ps_mm.tile([128, 512], F32, tag="mm")
                        idx = 0
                        for dt in range(ND):
                            for jj in range(k):
                                nc.tensor.matmul(
                                    ps[:, 0:Wg],
                                    cw[(k, jj)][:, dt, ft * 128:(ft + 1) * 128],
                                    xcv[:, dt, cs + jj:cs + jj + Wg],
                                    start=idx == 0, stop=idx == ND * k - 1)
                                idx += 1
                        cvt = work.tile([128, 512], F32, tag="pt",
                                        name=f"cv{cgi}_{ki}_{ft}")
                        nc.vector.tensor_tensor(out=cvt[:, 0:Wg],
                                                in0=ps[:, 0:Wg],
                                                in1=pen[:, 0:Wg], op=AL.add)
                        for j in cg:
                            rs = coff[j] - cs
                            re = rs + lam[j] - k + 2
                            nc.vector.tensor_reduce(
                                out=rep[:, ki * 2 + ft, j:j + 1],
                                in_=cvt[:, rs:re],
                                axis=mybir.AxisListType.X, op=AL.max)

        NCH = TP // 512 + (1 if TP % 512 else 0)
        chunk_rng = [(ci * 512, min((ci + 1) * 512, TP)) for ci in range(NCH)]

        # tiles first touched by each qk group (for embed/V scheduling)
        emb_done = set()

        def new_tiles(grp):
            g0, g1 = off[grp[0]], off[grp[-1] + 1]
            ts = [t for t in range(g0 // 128, -(-g1 // 128)) if t not in emb_done]
            emb_done.update(ts)
            return ts

        # ---- program ----
        # All embeds first (gathers + posty DMAs ahead of the big weight
        # DMAs in the queues), in group order so group 0 finishes first.
        for grp in lay["qk_groups"]:
            for nt in new_tiles(grp):
                embed(nt)
        for nt in range(TP // 128):
            if nt not in emb_done:
                emb_done.add(nt)
                embed(nt)
        w_cur = load_layer_weights(0, part="qk")
        load_layer_weights(0, w=w_cur, part="rest")
        ln_finalize("emb", range(NTT),
                    None if fl["ln"] else elns,
                    None if fl["ln"] else elnb, "emb")
        nc.vector.memset(ctxT[:], 0.0)
        for l in range(NL):
            ctxb_map = {}
            v_done = set()
            wo_done = set()
            fin1_done = set()
            ln1s = None if fl["ln"] else w_cur["ln1s"]
            ln1b = None if fl["ln"] else w_cur["ln1b"]
            ln2s = None if fl["ln"] else w_cur["ln2s"]
            ln2b = None if fl["ln"] else w_cur["ln2b"]

            def flush(j):
                # Wo matmuls + residual/stats (no ACT) for tiles whose
                # attention is complete — dense PE filler between the
                # ACT-paced softmax chains of consecutive slots. Chunk 0's
                # LN1 finalize runs mid-attention so FFN can start with
                # zero bubble at the phase boundary.
                for nt in range(NTT):
                    if nt in wo_done or last_slot_of.get(nt, -1) > j:
                        continue
                    if nt not in ctxb_map and nt in last_slot_of:
                        continue  # not yet computed this pass
                    wo_done.add(nt)
                    wo_stats(nt, w_cur)
                if 0 not in fin1_done and all(
                        t in wo_done for t in range(4)):
                    fin1_done.add(0)
                    ln_finalize("ln1", range(4), ln1s, ln1b, f"a{l}c0")

            pend = None
            for gi, grp in enumerate(lay["qk_groups"]):
                qkT = qk_group(gi, grp, w_cur)
                for j in grp:
                    for (nt, b, kw, rel) in chunks[j]:
                        if nt not in v_done:
                            v_done.add(nt)
                            v_tile(nt, w_cur)
                    # software pipeline: slot j's scores (and Wo filler)
                    # are emitted before slot j-1's ctx matmuls so the PE
                    # has work while the ACT exp chain for j-1 completes
                    attn_scores(l, j, qkT, off[grp[0]])
                    if pend is not None:
                        if pend >= 1:
                            flush(pend - 1)
                        attn_ctx(l, pend, ctxb_map, last_slot_of)
                    pend = j
                if gi == 0 and l + 1 < NL:
                    w_nxt = load_layer_weights(l + 1, part="qk")
            attn_ctx(l, pend, ctxb_map, last_slot_of)
            if l + 1 < NL:
                load_layer_weights(l + 1, w=w_nxt, part="rest")
            flush(NSL)
            if 0 not in fin1_done:
                ln_finalize("ln1", range(4), ln1s, ln1b, f"a{l}c0")
            ln_finalize("ln1", range(4, NTT), ln1s, ln1b, f"a{l}r")
            for ci, (c0, c1) in enumerate(chunk_rng):
                ffn_chunk(l, c0, c1, w_cur)
                if ci == 0:
                    # next layer's first QK group depends only on these
                    ln_finalize("ln2", range(4), ln2s, ln2b, f"f{l}c0")
                    if l == NL - 1:
                        conv_ready(3)
                if l == NL - 1 and ci > 0:
                    ln_finalize("ln2", range(ci * 4, min(ci * 4 + 4, NTT)),
                                ln2s, ln2b, f"f{l}c{ci}")
                    conv_ready(ci * 4 + 3)
            if l < NL - 1:
                ln_finalize("ln2", range(4, NTT), ln2s, ln2b, f"f{l}r")
                w_cur = w_nxt
            else:
                conv_ready(NTT)

        if debug:
            for nt in range(NTT):
                dx = work.tile([128, D], F32, tag="dbg", name=f"dbg{nt}")
                nc.vector.tensor_copy(out=dx[:], in_=x_tm[:, nt, :])
                nc.sync.dma_start(out=dbgx_d[nt], in_=dx[:])

        if not fl["cb"]:
            for ki in range(3):
                for ft in range(2):
                    nc.vector.tensor_scalar_add(
                        out=rep[:, ki * 2 + ft, :], in0=rep[:, ki * 2 + ft, :],
                        scalar1=cb[:, ki, ft:ft + 1])
        nc.scalar.activation(out=rep[:], in_=rep[:], func=AF.Relu)

        fps = ps_c.tile([128, 3 * NCLS], F32, tag="c")
        for c in range(6):
            nc.tensor.matmul(fps[:NSL, :], rep[:, c, :], fcw_sb[:, c, :],
                             start=c == 0, stop=c == 5)
        ob = small.tile([NSL, 3 * NCLS], F32, tag="ob")
        nc.scalar.copy(out=ob[:], in_=fps[:NSL, :])
        nc.sync.dma_start(out=out_d[:], in_=ob[:])

    nc.compile()
    return nc


def _core_inputs(inputs, fl, lay):
    """Build the 8 per-core input maps from the full problem inputs."""
    f32 = lambda a: np.ascontiguousarray(np.asarray(a, dtype=np.float32))
    tile_w = lambda w: np.ascontiguousarray(
        f32(w).reshape(w.shape[0] // 128, 128, w.shape[1]).astype(BF))

    NTT, TP, CWP = lay["NTT"], lay["TP"], lay["CWP"]
    lam, lam32, off, coff = lay["lam"], lay["lam32"], lay["off"], lay["coff"]
    assign = lay["assign"]

    shared = {}
    # packed position+type embedding
    posv = np.zeros((TP, D), np.float32)
    pe = f32(inputs["pos_emb"])
    for j in range(NSL):
        posv[off[j]:off[j] + lam32[j]] = pe[:lam32[j]]
    posv += f32(inputs["type_emb"][0])[None, :]
    shared["posty"] = np.ascontiguousarray(posv.reshape(NTT, 128, D))
    for l in range(NL):
        shared[f"wq{l}"] = tile_w(f32(inputs["Wq"][l]) / 8.0)
        shared[f"wk{l}"] = tile_w(inputs["Wk"][l])
        shared[f"wv{l}"] = tile_w(inputs["Wv"][l])
        shared[f"wo{l}"] = tile_w(inputs["Wo"][l])
        shared[f"wi{l}"] = tile_w(inputs["Wi"][l])
        shared[f"wo2{l}"] = tile_w(inputs["Wo2"][l])
        if not fl["bqk"]:
            shared[f"bq{l}"] = f32(inputs["bq"][l]).reshape(ND, 128) / 8.0
            shared[f"bk{l}"] = f32(inputs["bk"][l]).reshape(ND, 128)
        if not fl["bv"]:
            shared[f"bv{l}"] = f32(inputs["bv"][l])
        if not fl["bo"]:
            shared[f"bo{l}"] = f32(inputs["bo"][l])
        if not fl["bi"]:
            shared[f"bi{l}"] = f32(inputs["bi"][l]).reshape(NFT, 128)
        if not fl["bo2"]:
            shared[f"bo2{l}"] = f32(inputs["bo2"][l])
        if not fl["ln"]:
            shared[f"ln1s{l}"] = f32(inputs["ln1_s"][l])
            shared[f"ln1b{l}"] = f32(inputs["ln1_b"][l])
            shared[f"ln2s{l}"] = f32(inputs["ln2_s"][l])
            shared[f"ln2b{l}"] = f32(inputs["ln2_b"][l])
    if not fl["ln"]:
        shared["lnes"] = f32(inputs["emb_ln_s"])
        shared["lneb"] = f32(inputs["emb_ln_b"])
    for ki, k in enumerate((1, 2, 3)):
        w = f32(inputs[f"conv_w{k}"])                    # [NF, k, D]
        wt = np.ascontiguousarray(w.transpose(1, 2, 0))  # [k, D, NF]
        shared[f"cw{k}"] = np.ascontiguousarray(
            wt.reshape(k, ND, 128, NF).astype(BF))
    if not fl["cb"]:
        shared["convb"] = np.stack(
            [f32(inputs[f"conv_b{k}"]).reshape(2, 128) for k in (1, 2, 3)])
    # fc weights for all 3 branch-block hypotheses: [6, 128, 3*NCLS]
    fcw = f32(inputs["fc_w"])                            # [2304, NCLS]
    fcw3 = np.zeros((6, 128, 3 * NCLS), np.float32)
    for bb in range(3):
        for ki in range(3):
            for ft in range(2):
                c = ki * 2 + ft
                rows = 768 * bb + 256 * ki + 128 * ft
                fcw3[c, :, bb * NCLS:(bb + 1) * NCLS] = fcw[rows:rows + 128]
    shared["fcw"] = fcw3
    shared["word_emb"] = f32(inputs["word_emb"])

    ids_all = np.stack([np.asarray(inputs[p + "_input_ids"])
                        for p in ("q", "a", "b")]).reshape(96, S)
    mask_all = np.stack([np.asarray(inputs[p + "_attention_mask"])
                         for p in ("q", "a", "b")]).reshape(96, S)
    lens_all = mask_all.sum(1).astype(int)

    in_maps = []
    for c in range(NCORES):
        m = dict(shared)
        idv = np.zeros(TP, np.int32)
        vmv = np.zeros(TP, np.float32)
        cmv = np.zeros(CWP, np.float32)
        pen = np.full((3, CWP), -1e30, np.float32)
        for j in range(NSL):
            sq = int(assign[j, c])
            l = int(lens_all[sq])
            idv[off[j]:off[j] + lam32[j]] = ids_all[sq][:lam32[j]]
            vmv[off[j]:off[j] + l] = 1.0
            cmv[coff[j]:coff[j] + l] = 1.0
            for ki, k in enumerate((1, 2, 3)):
                nw = l - k + 2
                pen[ki, coff[j]:coff[j] + nw] = 0.0
        m["ids"] = np.ascontiguousarray(idv.reshape(NTT, 128))
        m["vmask"] = np.ascontiguousarray(vmv.reshape(NTT, 128))
        m["convmask"] = np.ascontiguousarray(cmv.astype(BF))
        m["convpen"] = np.ascontiguousarray(pen)
        in_maps.append(m)
    return in_maps


def _get_program(fl, lay, debug=False):
    key = (tuple(sorted(fl.items())), lay["key"], debug)
    if key not in _CACHE:
        _CACHE[key] = _build_program(fl, lay, debug=debug)
    return _CACHE[key]


def run_sharded(inputs, debug=False, **run_kwargs):
    """Shard, run on 8 cores, gather. Returns (output, BassKernelResults)."""
    from concourse.bass_utils import run_bass_kernel_spmd
    fl = _flags(inputs)
    lens96 = np.concatenate([
        np.asarray(inputs[p + "_attention_mask"]).sum(1) for p in ("q", "a", "b")])
    lay = _layout(lens96)
    nc = _get_program(fl, lay, debug=debug)
    in_maps = _core_inputs(inputs, fl, lay)
    res = run_bass_kernel_spmd(nc, in_maps, core_ids=list(range(NCORES)),
                               **run_kwargs)
    border = {0: 0, 1: 2, 2: 1}   # branch q/a/b -> fc block q,b,a
    out = np.zeros((B, NCLS), np.float32)
    for c in range(NCORES):
        o3 = np.asarray(res.results[c]["out"], np.float32)   # [NSL, 12]
        for j in range(NSL):
            sq = int(lay["assign"][j, c])
            br, sample = sq // B, sq % B
            out[sample] += o3[j, border[br] * NCLS:(border[br] + 1) * NCLS]
    out += np.asarray(inputs["fc_b"], np.float32)[None, :]
    return out, res


def kernel(**inputs):
    out, _ = run_sharded(inputs)
    return out


# revision 41
# speedup vs baseline: 1.9205x; 1.0272x over previous
"""Trainium2 Bass kernel for nn_BertCNN (3x BERT-small encoder + CNN maxpool head).

Ragged-packed data-parallel strategy. The 96 sequences (3 branches x 32
samples) are sorted by actual length (from the attention mask), dealt
round-robin into 8 cores x 12 slots, and each core packs its 12 sequences
into one ~1900-token stream (slot budgets = max length in each rank group,
32-aligned starts). All encoder linear ops (QKV/O/FFN/conv) run over the
packed stream; attention runs per-slot with exact budget widths; key
validity is folded multiplicatively into V (invalid keys get zero V rows
and a zero softmax-denominator contribution), so no attention bias is
needed. The conv head runs over a separately packed layout with 2-token
gaps; per-slot maxpool ranges and window-validity penalties come from the
host. The fc output is computed for all 3 branch hypotheses per slot
([12, 12] per core) and the host scatter-adds the right 4 columns into the
final [32, 4].

The Bass program depends only on the slot-budget layout (not on per-core
data); it is built once per layout signature and cached.
"""

import numpy as np
import ml_dtypes

V, D, H, DH, NL, FF = 30522, 512, 8, 64, 4, 2048
NF, NCLS, B, S = 256, 4, 32, 256
NCORES = 8
NSL = 12                 # slots (sequences) per core
NSEQ = NSL               # test.py compat
SPC = 4                  # test.py compat
ND = D // 128
NFT = FF // 128
DH1 = DH + 1

BF = ml_dtypes.bfloat16
_CACHE = {}


def _flags(inputs):
    z = lambda a: bool(np.all(np.asarray(a) == 0))
    o = lambda a: bool(np.all(np.asarray(a) == 1))
    return {
        "bqk": z(inputs["bq"]) and z(inputs["bk"]),
        "bv": z(inputs["bv"]),
        "bo": z(inputs["bo"]),
        "bi": z(inputs["bi"]),
        "bo2": z(inputs["bo2"]),
        "ln": all(o(inputs[k]) for k in ("emb_ln_s", "ln1_s", "ln2_s"))
        and all(z(inputs[k]) for k in ("emb_ln_b", "ln1_b", "ln2_b")),
        "cb": z(inputs["conv_b1"]) and z(inputs["conv_b2"]) and z(inputs["conv_b3"]),
    }


def _layout(lens96):
    """Pack layout shared by all cores (program-shaping constants)."""
    lens96 = np.asarray(lens96, dtype=np.int64)
    order = np.argsort(-lens96, kind="stable")
    assign = order.reshape(NSL, NCORES)           # [slot, core] -> seq idx
    lam = lens96[assign].max(1).astype(int)       # slot budgets (max len)
    # 64-aligned slot starts (PE col/row tiling only supports base 0/64 for
    # >32-wide tiles); bump a start to the next 128 boundary when the slot
    # would otherwise span 3 token tiles (score/eT tiles hold 2 chunks)
    lam32 = ((lam + 63) // 64) * 64
    off = np.zeros(NSL + 1, np.int64)
    for j in range(NSL):
        o = off[j]
        if (o % 128) + lam[j] > 256:
            o = ((o + 127) // 128) * 128
            off[j] = o
        off[j + 1] = o + lam32[j]
    T32 = int(off[-1])
    NTT = -(-T32 // 128)
    TP = NTT * 128
    coff = np.zeros(NSL + 1, np.int64)
    coff[1:] = np.cumsum(lam + 2)
    CW = int(coff[-1])

    def greedy(offs, cap):
        groups, cur = [], [0]
        for j in range(1, NSL):
            if offs[j + 1] - offs[cur[0]] <= cap:
                cur.append(j)
            else:
                groups.append(cur)
                cur = [j]
        groups.append(cur)
        return groups

    qk_groups = greedy(off, 512)
    cv_groups = greedy(coff, 512)

    # per-slot key/query chunks: intersections with the global 128 grid
    chunks = []
    for j in range(NSL):
        lo, hi = int(off[j]), int(off[j] + lam[j])
        ch = []
        p = lo
        while p < hi:
            nt = p // 128
            e = min(hi, (nt + 1) * 128)
            ch.append((nt, p - nt * 128, e - p, p - lo))  # (tile, base, width, rel)
            p = e
        chunks.append(ch)
    return dict(
        assign=assign, lam=[int(x) for x in lam], lam32=[int(x) for x in lam32],
        off=[int(x) for x in off], coff=[int(x) for x in coff],
        T32=T32, NTT=NTT, TP=TP, CW=CW, CWP=CW + 2,
        qk_groups=qk_groups, cv_groups=cv_groups, chunks=chunks,
        key=(tuple(int(x) for x in lam), tuple(int(x) for x in lam32)),
    )


def _build_program(fl, lay, debug=False):
    import contextlib
    import concourse.bass as bass
    import concourse.mybir as mybir
    import concourse.tile as tile
    from concourse import bacc

    F32, BF16, I32 = mybir.dt.float32, mybir.dt.bfloat16, mybir.dt.int32
    AL, AF = mybir.AluOpType, mybir.ActivationFunctionType

    NTT, TP, CWP = lay["NTT"], lay["TP"], lay["CWP"]
    lam, lam32, off, coff = lay["lam"], lay["lam32"], lay["off"], lay["coff"]
    chunks = lay["chunks"]

    nc = bacc.Bacc("TRN2", target_bir_lowering=False, debug=False,
                   num_devices=NCORES)

    di = lambda n, s, d: nc.dram_tensor(n, s, d, kind="ExternalInput").ap()
    word = di("word_emb", [V, D], F32)
    ids_d = di("ids", [NTT, 128], I32)
    vm_d = di("vmask", [NTT, 128], F32)
    posty_d = di("posty", [NTT, 128, D], F32)
    cm_d = di("convmask", [CWP], BF16)
    cpen_d = di("convpen", [3, CWP], F32)
    wq_d = [di(f"wq{l}", [ND, 128, D], BF16) for l in range(NL)]
    wk_d = [di(f"wk{l}", [ND, 128, D], BF16) for l in range(NL)]
    wv_d = [di(f"wv{l}", [ND, 128, D], BF16) for l in range(NL)]
    wo_d = [di(f"wo{l}", [ND, 128, D], BF16) for l in range(NL)]
    wi_d = [di(f"wi{l}", [ND, 128, FF], BF16) for l in range(NL)]
    wo2_d = [di(f"wo2{l}", [NFT, 128, D], BF16) for l in range(NL)]
    cw_d = [di(f"cw{k}", [k, ND, 128, NF], BF16) for k in (1, 2, 3)]
    fcw_d = di("fcw", [6, 128, 3 * NCLS], F32)
    if not fl["bqk"]:
        bq_d = [di(f"bq{l}", [ND, 128], F32) for l in range(NL)]
        bk_d = [di(f"bk{l}", [ND, 128], F32) for l in range(NL)]
    if not fl["bv"]:
        bv_d = [di(f"bv{l}", [D], F32) for l in range(NL)]
    if not fl["bo"]:
        bo_d = [di(f"bo{l}", [D], F32) for l in range(NL)]
    if not fl["bi"]:
        bi_d = [di(f"bi{l}", [NFT, 128], F32) for l in range(NL)]
    if not fl["bo2"]:
        bo2_d = [di(f"bo2{l}", [D], F32) for l in range(NL)]
    if not fl["ln"]:
        elns_d = di("lnes", [D], F32)
        elnb_d = di("lneb", [D], F32)
        ln1s_d = [di(f"ln1s{l}", [D], F32) for l in range(NL)]
        ln1b_d = [di(f"ln1b{l}", [D], F32) for l in range(NL)]
        ln2s_d = [di(f"ln2s{l}", [D], F32) for l in range(NL)]
        ln2b_d = [di(f"ln2b{l}", [D], F32) for l in range(NL)]
    if not fl["cb"]:
        cb_d = di("convb", [3, 2, 128], F32)

    out_d = nc.dram_tensor("out", [NSL, 3 * NCLS], F32, kind="ExternalOutput").ap()
    if debug:
        dbgx_d = nc.dram_tensor("dbgx", [NTT, 128, D], F32,
                                kind="ExternalOutput").ap()

    with tile.TileContext(nc) as tc, contextlib.ExitStack() as ctx:
        consts = ctx.enter_context(tc.tile_pool(name="consts", bufs=1))
        state = ctx.enter_context(tc.tile_pool(name="state", bufs=1))
        wts = ctx.enter_context(tc.tile_pool(name="wts", bufs=1))
        qkp = ctx.enter_context(tc.tile_pool(name="qkp", bufs=2))
        etp = ctx.enter_context(tc.tile_pool(name="etp", bufs=2))
        htp = ctx.enter_context(tc.tile_pool(name="htp", bufs=1))
        work = ctx.enter_context(tc.tile_pool(name="work", bufs=2))
        cxp = ctx.enter_context(tc.tile_pool(name="cxp", bufs=3))
        small = ctx.enter_context(tc.tile_pool(name="small", bufs=4))
        ps_mm = ctx.enter_context(tc.tile_pool(name="ps_mm", bufs=3, space="PSUM"))
        ps_s = ctx.enter_context(tc.tile_pool(name="ps_s", bufs=3, space="PSUM"))
        ps_c = ctx.enter_context(tc.tile_pool(name="ps_c", bufs=2, space="PSUM"))

        # ---- constants ----
        eps_t = consts.tile([128, 1], F32, tag="eps")
        nc.vector.memset(eps_t[:], 1e-12)
        ones8 = consts.tile([128, H], BF16, tag="ones8")
        nc.vector.memset(ones8[:], 1.0)
        ids_sb = consts.tile([128, NTT], I32, tag="ids")
        nc.sync.dma_start(out=ids_sb[:], in_=ids_d.rearrange("t p -> p t"))
        vm_sb = consts.tile([128, NTT], F32, tag="vm")
        nc.sync.dma_start(out=vm_sb[:], in_=vm_d.rearrange("t p -> p t"))
        cm_sb = consts.tile([128, CWP], BF16, tag="cm")
        nc.sync.dma_start(out=cm_sb[:],
                          in_=cm_d[None, :].to_broadcast([128, CWP]))
        fcw_sb = consts.tile([128, 6, 3 * NCLS], F32, tag="fcw")
        nc.sync.dma_start(out=fcw_sb[:], in_=fcw_d.rearrange("c p n -> p c n"))
        bcast = lambda ap, n: ap[None, :].to_broadcast([128, n])
        if not fl["ln"]:
            elns = consts.tile([128, D], F32, tag="elns")
            nc.sync.dma_start(out=elns[:], in_=bcast(elns_d, D))
            elnb = consts.tile([128, D], F32, tag="elnb")
            nc.sync.dma_start(out=elnb[:], in_=bcast(elnb_d, D))
        if not fl["cb"]:
            cb = consts.tile([128, 3, 2], F32, tag="cb")
            nc.sync.dma_start(out=cb[:], in_=cb_d.rearrange("k t p -> p k t"))

        cw = {}
        for ki, k in enumerate((1, 2, 3)):
            for j in range(k):
                t = wts.tile([128, ND, NF], BF16, tag=f"cw{ki}_{j}")
                nc.sync.dma_start(out=t[:],
                                  in_=cw_d[ki][j].rearrange("t p f -> p t f"))
                cw[(k, j)] = t

        # ---- persistent state ----
        x_tm = state.tile([128, NTT, D], BF16, tag="x_tm")
        xTa = state.tile([128, ND, TP], BF16, tag="xTa")
        ctxT = state.tile([128, ND, TP], BF16, tag="ctxT")
        vA = state.tile([128, NTT, H, DH1], BF16, tag="vA")
        rep = state.tile([128, 6, NSL], F32, tag="rep")

        def load_layer_weights(l, w=None, part="all"):
            if w is None:
                w = {}
            names = {"qk": ("wq", "wk"), "rest": ("wv", "wo", "wi"),
                     "all": ("wq", "wk", "wv", "wo", "wi")}[part]
            for nm, dd, nfree in (("wq", wq_d, D), ("wk", wk_d, D),
                                  ("wv", wv_d, D), ("wo", wo_d, D),
                                  ("wi", wi_d, FF)):
                if nm not in names:
                    continue
                w[nm] = wts.tile([128, ND, nfree], BF16, tag=nm, name=f"{nm}_{l}",
                                 bufs=2 if nm in ("wq", "wk") else 1)
                for dt in range(ND):
                    nc.sync.dma_start(out=w[nm][:, dt, :], in_=dd[l][dt])
            if part == "qk":
                return w
            w["wo2"] = wts.tile([128, NFT, D], BF16, tag="wo2", name=f"wo2_{l}")
            for ft in range(0, NFT, 4):
                nc.sync.dma_start(
                    out=w["wo2"][:, ft:ft + 4, :],
                    in_=wo2_d[l][ft:ft + 4].rearrange("t p o -> p t o"))
            if not fl["bqk"]:
                w["bq"] = consts.tile([128, ND], F32, tag="bq", name=f"bq_{l}")
                nc.sync.dma_start(out=w["bq"][:], in_=bq_d[l].rearrange("t p -> p t"))
                w["bk"] = consts.tile([128, ND], F32, tag="bk", name=f"bk_{l}")
                nc.sync.dma_start(out=w["bk"][:], in_=bk_d[l].rearrange("t p -> p t"))
            if not fl["bv"]:
                w["bv"] = consts.tile([128, D], F32, tag="bv", name=f"bv_{l}")
                nc.sync.dma_start(out=w["bv"][:], in_=bcast(bv_d[l], D))
            if not fl["bo"]:
                w["bo"] = consts.tile([128, D], F32, tag="bo", name=f"bo_{l}")
                nc.sync.dma_start(out=w["bo"][:], in_=bcast(bo_d[l], D))
            if not fl["bi"]:
                w["bi"] = consts.tile([128, NFT], F32, tag="bi", name=f"bi_{l}")
                nc.sync.dma_start(out=w["bi"][:], in_=bi_d[l].rearrange("t p -> p t"))
            if not fl["bo2"]:
                w["bo2"] = consts.tile([128, D], F32, tag="bo2", name=f"bo2_{l}")
                nc.sync.dma_start(out=w["bo2"][:], in_=bcast(bo2_d[l], D))
            if not fl["ln"]:
                for nm, dd in (("ln1s", ln1s_d), ("ln1b", ln1b_d),
                               ("ln2s", ln2s_d), ("ln2b", ln2b_d)):
                    w[nm] = consts.tile([128, D], F32, tag=nm, name=f"{nm}_{l}")
                    nc.sync.dma_start(out=w[nm][:], in_=bcast(dd[l], D))
            return w

        def ln_start(src_ap, i=0):
            st = small.tile([128, 6], F32, tag="st", name=f"st{i}")
            mv = small.tile([128, 2], F32, tag="mv", name=f"mv{i}")
            nc.vector.bn_stats(out=st[:], in_=src_ap)
            nc.vector.bn_aggr(out=mv[:], in_=st[:])
            sd = small.tile([128, 1], F32, tag="sd", name=f"sd{i}")
            nc.scalar.activation(out=sd[:], in_=mv[:, 1:2], func=AF.Sqrt,
                                 bias=eps_t[:], scale=1.0)
            nmr = small.tile([128, 2], F32, tag="nmr", name=f"nmr{i}")
            nc.vector.reciprocal(out=nmr[:, 1:2], in_=sd[:])
            nc.vector.tensor_scalar(out=nmr[:, 0:1], in0=mv[:, 0:1],
                                    scalar1=nmr[:, 1:2], scalar2=-1.0,
                                    op0=AL.mult, op1=AL.mult)
            return nmr

        def ln_apply(src_ap, dst_ap, nmr, s_tile, b_tile):
            nc.vector.tensor_scalar(out=dst_ap, in0=src_ap,
                                    scalar1=nmr[:, 1:2], scalar2=nmr[:, 0:1],
                                    op0=AL.mult, op1=AL.add)
            if s_tile is not None:
                nc.vector.tensor_tensor(out=dst_ap, in0=dst_ap, in1=s_tile[:],
                                        op=AL.mult)
            if b_tile is not None:
                nc.vector.tensor_tensor(out=dst_ap, in0=dst_ap, in1=b_tile[:],
                                        op=AL.add)

        def embed(nt):
            # ACT-free: pre-LN values land in x_tm (bf16), stats collect
            # into the batch buffer; one Sqrt region finalizes all tiles
            g = work.tile([128, D], F32, tag="g", name=f"g{nt}")
            nc.gpsimd.indirect_dma_start(
                out=g[:], out_offset=None, in_=word[:],
                in_offset=bass.IndirectOffsetOnAxis(
                    ap=ids_sb[:, nt:nt + 1], axis=0))
            pt = work.tile([128, D], F32, tag="pt", name=f"pt{nt}")
            nc.sync.dma_start(out=pt[:], in_=posty_d[nt])
            nc.vector.tensor_tensor(out=x_tm[:, nt, :], in0=g[:], in1=pt[:],
                                    op=AL.add)
            st = small.tile([128, 6], F32, tag="st", name=f"est{nt % 4}")
            nc.vector.bn_stats(out=st[:], in_=x_tm[:, nt, :])
            nc.vector.bn_aggr(out=mvb["emb"][:, :, nt], in_=st[:])

        def qk_group(gi, grp, w):
            g0, g1 = off[grp[0]], off[grp[-1] + 1]
            W = g1 - g0
            qkT = qkp.tile([128, 2, ND, 512], BF16, tag="qkT", name=f"qkT{gi}")
            for qi, wt, bt in ((0, w["wq"], "bq"), (1, w["wk"], "bk")):
                for ot in range(ND):
                    ps = ps_mm.tile([128, 512], F32, tag="mm")
                    for dt in range(ND):
                        nc.tensor.matmul(
                            ps[:, 0:W], wt[:, dt, ot * 128:(ot + 1) * 128],
                            xTa[:, dt, g0:g1], start=dt == 0, stop=dt == ND - 1)
                    if fl["bqk"]:
                        nc.vector.tensor_copy(out=qkT[:, qi, ot, 0:W],
                                              in_=ps[:, 0:W])
                    else:
                        nc.scalar.activation(
                            out=qkT[:, qi, ot, 0:W], in_=ps[:, 0:W],
                            func=AF.Identity, bias=w[bt][:, ot:ot + 1], scale=1.0)
            return qkT

        def v_tile(nt, w):
            ps = ps_mm.tile([128, 512], F32, tag="mm")
            for dt in range(ND):
                nc.tensor.matmul(ps[:], xTa[:, dt, nt * 128:(nt + 1) * 128],
                                 w["wv"][:, dt, :], start=dt == 0,
                                 stop=dt == ND - 1)
            if fl["bv"]:
                nc.vector.tensor_scalar_mul(
                    out=vA[:, nt, :, 0:DH],
                    in0=ps.rearrange("p (h d) -> p h d", h=H),
                    scalar1=vm_sb[:, nt:nt + 1])
            else:
                nc.vector.tensor_tensor(
                    out=vA[:, nt, :, 0:DH],
                    in0=ps.rearrange("p (h d) -> p h d", h=H),
                    in1=w["bv"].rearrange("p (h d) -> p h d", h=H), op=AL.add)
                nc.vector.tensor_scalar_mul(
                    out=vA[:, nt, :, 0:DH], in0=vA[:, nt, :, 0:DH],
                    scalar1=vm_sb[:, nt:nt + 1])
            nc.vector.tensor_scalar_mul(
                out=vA[:, nt, :, DH], in0=ones8[:],
                scalar1=vm_sb[:, nt:nt + 1])

        est = {}

        def attn_scores(l, j, qkT, g0):
            L = lam[j]
            ch = chunks[j]
            q0 = off[j] - g0
            eT = etp.tile([128, H, 512], BF16, tag="eT", name=f"eT{l}_{j}")
            est[j] = eT
            for h in range(H):
                hh, dtH = (h % 2) * DH, h // 2
                pss = ps_s.tile([128, 512], F32, tag="s")
                for ci, (nt, b, kw, rel) in enumerate(ch):
                    nc.tensor.matmul(
                        pss[b:b + kw, ci * L:ci * L + L],
                        qkT[hh:hh + DH, 1, dtH, q0 + rel:q0 + rel + kw],
                        qkT[hh:hh + DH, 0, dtH, q0:q0 + L],
                        start=True, stop=True)
                nc.scalar.activation(out=eT[:, h, 0:len(ch) * L],
                                     in_=pss[:, 0:len(ch) * L], func=AF.Exp,
                                     bias=0.0, scale=1.0)

        def attn_ctx(l, j, ctxb_map, last_slot_of):
            L = lam[j]
            ch = chunks[j]
            eT = est.pop(j)
            # ctx per query chunk
            for (qnt, qb, qw, qrel) in ch:
                for hg in range(2):
                    cps = ps_c.tile([128, 4 * DH1], F32, tag="c")
                    for hi in range(4):
                        h = hg * 4 + hi
                        sl = slice(hi * DH1, hi * DH1 + DH1)
                        for ci, (nt, b, kw, rel) in enumerate(ch):
                            nc.tensor.matmul(
                                cps[qb:qb + qw, sl],
                                eT[b:b + kw, h, ci * L + qrel:ci * L + qrel + qw],
                                vA[b:b + kw, nt, h, :],
                                start=ci == 0, stop=ci == len(ch) - 1)
                    if qnt not in ctxb_map:
                        ctxb_map[qnt] = cxp.tile([128, D], BF16, tag="ctxb",
                                                 name=f"cb{l}_{qnt}")
                    ctxb = ctxb_map[qnt]
                    rcp = small.tile([128, 4], F32, tag="rcp")
                    nc.vector.reciprocal(
                        out=rcp[qb:qb + qw, :],
                        in_=cps.rearrange("p (h c) -> p h c", c=DH1)[qb:qb + qw, :, DH])
                    nc.vector.tensor_tensor(
                        out=ctxb.rearrange("p (h d) -> p h d", d=DH)[
                            qb:qb + qw, hg * 4:hg * 4 + 4, :],
                        in0=cps.rearrange("p (h c) -> p h c", c=DH1)[qb:qb + qw, :, 0:DH],
                        in1=rcp[qb:qb + qw, :, None].to_broadcast([qw, 4, DH]),
                        op=AL.mult)
            # flush finished ctxb tiles
            for (qnt, qb, qw, qrel) in ch:
                if last_slot_of.get(qnt) == j:
                    nc.sync.dma_start_transpose(
                        ctxT[:, :, qnt * 128:(qnt + 1) * 128], ctxb_map[qnt][:])

        # deferred-LN machinery: residual-add lands pre-LN values in x_tm
        # (bf16, in place); per-tile bn stats collect into a batch buffer;
        # one Sqrt region per phase finalizes all tiles (ACT table stays
        # resident for Exp/Gelu — each table swap costs ~1.5us)
        mvb = {}
        for ph in ("ln1", "ln2", "emb"):
            mvb[ph] = state.tile([128, 2, NTT], F32, tag=f"mvb_{ph}",
                                 name=f"mvb_{ph}")

        def res_stats(nt, ps, bias_t, ph, i=0):
            nc.vector.tensor_tensor(out=x_tm[:, nt, :], in0=ps[:],
                                    in1=x_tm[:, nt, :], op=AL.add)
            if bias_t is not None:
                nc.vector.tensor_tensor(out=x_tm[:, nt, :], in0=x_tm[:, nt, :],
                                        in1=bias_t[:], op=AL.add)
            st = small.tile([128, 6], F32, tag="st", name=f"st{i}")
            nc.vector.bn_stats(out=st[:], in_=x_tm[:, nt, :])
            nc.vector.bn_aggr(out=mvb[ph][:, :, nt], in_=st[:])

        def ln_finalize(ph, tiles, s_tile, b_tile, lbl):
            """Batched LN finalize for `tiles`: one Sqrt region, DVE applies,
            transposes into xTa. Safe to call per-chunk (mini batches)."""
            tiles = [t for t in tiles]
            if not tiles:
                return
            mv = mvb[ph]
            t0, t1 = min(tiles), max(tiles) + 1
            sdb = small.tile([128, NTT], F32, tag="sdb", name=f"sdb{lbl}")
            nc.scalar.activation(out=sdb[:, t0:t1], in_=mv[:, 1, t0:t1],
                                 func=AF.Sqrt, bias=eps_t[:], scale=1.0)
            rsb = small.tile([128, NTT], F32, tag="rsb", name=f"rsb{lbl}")
            nc.vector.reciprocal(out=rsb[:, t0:t1], in_=sdb[:, t0:t1])
            nmb = small.tile([128, NTT], F32, tag="nmb", name=f"nmb{lbl}")
            nc.vector.tensor_tensor(out=nmb[:, t0:t1], in0=mv[:, 0, t0:t1],
                                    in1=rsb[:, t0:t1], op=AL.mult)
            nc.vector.tensor_scalar_mul(out=nmb[:, t0:t1], in0=nmb[:, t0:t1],
                                        scalar1=-1.0)
            for nt in tiles:
                nc.vector.tensor_scalar(
                    out=x_tm[:, nt, :], in0=x_tm[:, nt, :],
                    scalar1=rsb[:, nt:nt + 1], scalar2=nmb[:, nt:nt + 1],
                    op0=AL.mult, op1=AL.add)
                if s_tile is not None:
                    nc.vector.tensor_tensor(out=x_tm[:, nt, :],
                                            in0=x_tm[:, nt, :],
                                            in1=s_tile[:], op=AL.mult)
                if b_tile is not None:
                    nc.vector.tensor_tensor(out=x_tm[:, nt, :],
                                            in0=x_tm[:, nt, :],
                                            in1=b_tile[:], op=AL.add)
                nc.sync.dma_start_transpose(
                    xTa[:, :, nt * 128:(nt + 1) * 128], x_tm[:, nt, :])

        def wo_stats(nt, w):
            ps = ps_mm.tile([128, 512], F32, tag="mm")
            for dt in range(ND):
                nc.tensor.matmul(ps[:], ctxT[:, dt, nt * 128:(nt + 1) * 128],
                                 w["wo"][:, dt, :], start=dt == 0,
                                 stop=dt == ND - 1)
            res_stats(nt, ps, None if fl["bo"] else w["bo"], "ln1", i=nt % 4)

        def ffn_chunk(l, c0, c1, w):
            Wc = c1 - c0
            hT = htp.tile([128, NFT, 512], BF16, tag="hT", name=f"hT{l}_{c0}")
            for ft in range(NFT):
                ps = ps_mm.tile([128, 512], F32, tag="mm")
                for dt in range(ND):
                    nc.tensor.matmul(
                        ps[:, 0:Wc], w["wi"][:, dt, ft * 128:(ft + 1) * 128],
                        xTa[:, dt, c0:c1], start=dt == 0, stop=dt == ND - 1)
                nc.scalar.activation(
                    out=hT[:, ft, 0:Wc], in_=ps[:, 0:Wc], func=AF.Gelu,
                    bias=0.0 if fl["bi"] else w["bi"][:, ft:ft + 1], scale=1.0)
            for nt in range(c0 // 128, c1 // 128):
                toff = nt * 128 - c0
                ps = ps_mm.tile([128, 512], F32, tag="mm")
                for ft in range(NFT):
                    nc.tensor.matmul(ps[:], hT[:, ft, toff:toff + 128],
                                     w["wo2"][:, ft, :], start=ft == 0,
                                     stop=ft == NFT - 1)
                res_stats(nt, ps, None if fl["bo2"] else w["bo2"], "ln2",
                          i=nt % 4)

        # last slot writing each ctx tile (for flush scheduling)
        last_slot_of = {}
        for j in range(NSL):
            for (nt, b, kw, rel) in chunks[j]:
                last_slot_of[nt] = j

        # ---- conv head, interleaved with layer-3 FFN ----
        # xcv reuses ctxT's slot (attention is done by then); slot assembly
        # runs on GPSIMD (idle) as its tiles finish LN2; a conv group's
        # matmuls+maxpools run once all its slots are assembled.
        conv_st = {"xcv": None, "asm": set(), "grp": set()}

        def conv_ready(max_tile):
            if conv_st["xcv"] is None:
                conv_st["xcv"] = state.tile([128, ND, CWP], BF16, tag="ctxT",
                                            name="xcv")
                nc.gpsimd.memset(conv_st["xcv"][:], 0.0)
            xcv = conv_st["xcv"]
            for j in range(NSL):
                if j in conv_st["asm"]:
                    continue
                if max(nt for (nt, b, kw, rel) in chunks[j]) > max_tile:
                    continue
                conv_st["asm"].add(j)
                o0, c0 = off[j], coff[j]
                for dt in range(ND):
                    eng = nc.vector if dt < 2 else nc.gpsimd
                    eng.tensor_tensor(
                        out=xcv[:, dt, c0:c0 + lam[j]],
                        in0=xTa[:, dt, o0:o0 + lam[j]],
                        in1=cm_sb[:, c0:c0 + lam[j]], op=AL.mult)
            for cgi, cg in enumerate(lay["cv_groups"]):
                if cgi in conv_st["grp"]:
                    continue
                if not all(j in conv_st["asm"] for j in cg):
                    continue
                conv_st["grp"].add(cgi)
                cs, ce = coff[cg[0]], coff[cg[-1] + 1]
                Wg = ce - cs
                for ki, k in enumerate((1, 2, 3)):
                    pen = work.tile([128, 512], F32, tag="g",
                                    name=f"pn{cgi}_{ki}")
                    nc.sync.dma_start(
                        out=pen[:, 0:Wg],
                        in_=cpen_d[ki, cs:ce][None, :].to_broadcast([128, Wg]))
                    for ft in range(2):
                        ps = ps_mm.tile([128, 512], F32, tag="mm")
                        idx = 0
                        for dt in range(ND):
                            for jj in range(k):
                                nc.tensor.matmul(
                                    ps[:, 0:Wg],
                                    cw[(k, jj)][:, dt, ft * 128:(ft + 1) * 128],
                                    xcv[:, dt, cs + jj:cs + jj + Wg],
                                    start=idx == 0, stop=idx == ND * k - 1)
                                idx += 1
                        cvt = work.tile([128, 512], F32, tag="pt",
                                        name=f"cv{cgi}_{ki}_{ft}")
                        nc.vector.tensor_tensor(out=cvt[:, 0:Wg],
                                                in0=ps[:, 0:Wg],
                                                in1=pen[:, 0:Wg], op=AL.add)
                        for j in cg:
                            rs = coff[j] - cs
                            re = rs + lam[j] - k + 2
                            nc.vector.tensor_reduce(
                                out=rep[:, ki * 2 + ft, j:j + 1],
                                in_=cvt[:, rs:re],
                                axis=mybir.AxisListType.X, op=AL.max)

        NCH = TP // 512 + (1 if TP % 512 else 0)
        chunk_rng = [(ci * 512, min((ci + 1) * 512, TP)) for ci in range(NCH)]

        # tiles first touched by each qk group (for embed/V scheduling)
        emb_done = set()

        def new_tiles(grp):
            g0, g1 = off[grp[0]], off[grp[-1] + 1]
            ts = [t for t in range(g0 // 128, -(-g1 // 128)) if t not in emb_done]
            emb_done.update(ts)
            return ts

        # ---- program ----
        # All embeds first (gathers + posty DMAs ahead of the big weight
        # DMAs in the queues), in group order so group 0 finishes first.
        for grp in lay["qk_groups"]:
            for nt in new_tiles(grp):
                embed(nt)
        for nt in range(TP // 128):
            if nt not in emb_done:
                emb_done.add(nt)
                embed(nt)
        w_cur = load_layer_weights(0, part="qk")
        load_layer_weights(0, w=w_cur, part="rest")
        ln_finalize("emb", range(NTT),
                    None if fl["ln"] else elns,
                    None if fl["ln"] else elnb, "emb")
        nc.vector.memset(ctxT[:], 0.0)
        for l in range(NL):
            ctxb_map = {}
            v_done = set()
            wo_done = set()
            fin1_done = set()
            ln1s = None if fl["ln"] else w_cur["ln1s"]
            ln1b = None if fl["ln"] else w_cur["ln1b"]
            ln2s = None if fl["ln"] else w_cur["ln2s"]
            ln2b = None if fl["ln"] else w_cur["ln2b"]

            def flush(j):
                # Wo matmuls + residual/stats (no ACT) for tiles whose
                # attention is complete — dense PE filler between the
                # ACT-paced softmax chains of consecutive slots. Chunk 0's
                # LN1 finalize runs mid-attention so FFN can start with
                # zero bubble at the phase boundary.
                for nt in range(NTT):
                    if nt in wo_done or last_slot_of.get(nt, -1) > j:
                        continue
                    if nt not in ctxb_map and nt in last_slot_of:
                        continue  # not yet computed this pass
                    wo_done.add(nt)
                    wo_stats(nt, w_cur)
                if 0 not in fin1_done and all(
                        t in wo_done for t in range(4)):
                    fin1_done.add(0)
                    ln_finalize("ln1", range(4), ln1s, ln1b, f"a{l}c0")

            pend = None
            for gi, grp in enumerate(lay["qk_groups"]):
                qkT = qk_group(gi, grp, w_cur)
                for j in grp:
                    for (nt, b, kw, rel) in chunks[j]:
                        if nt not in v_done:
                            v_done.add(nt)
                            v_tile(nt, w_cur)
                    # software pipeline: slot j's scores (and Wo filler)
                    # are emitted before slot j-1's ctx matmuls so the PE
                    # has work while the ACT exp chain for j-1 completes
                    attn_scores(l, j, qkT, off[grp[0]])
                    if pend is not None:
                        if pend >= 1:
                            flush(pend - 1)
                        attn_ctx(l, pend, ctxb_map, last_slot_of)
                    pend = j
                if gi == 0 and l + 1 < NL:
                    w_nxt = load_layer_weights(l + 1, part="qk")
            attn_ctx(l, pend, ctxb_map, last_slot_of)
            if l + 1 < NL:
                load_layer_weights(l + 1, w=w_nxt, part="rest")
            flush(NSL)
            if 0 not in fin1_done:
                ln_finalize("ln1", range(4), ln1s, ln1b, f"a{l}c0")
            ln_finalize("ln1", range(4, NTT), ln1s, ln1b, f"a{l}r")
            for ci, (c0, c1) in enumerate(chunk_rng):
                ffn_chunk(l, c0, c1, w_cur)
                # per-chunk LN2 finalize: chunk 0 feeds the next layer's
                # first QK group; at l3 it feeds the conv head, whose
                # groups trail one chunk so FFN matmuls (already ready)
                # aren't stuck behind conv's DVE assembly in the PE FIFO
                ln_finalize("ln2", range(ci * 4, min(ci * 4 + 4, NTT)),
                            ln2s, ln2b, f"f{l}c{ci}")
                if l == NL - 1 and ci > 0:
                    conv_ready(ci * 4 - 1)
            if l < NL - 1:
                w_cur = w_nxt
            else:
                conv_ready(NTT)

        if debug:
            for nt in range(NTT):
                dx = work.tile([128, D], F32, tag="dbg", name=f"dbg{nt}")
                nc.vector.tensor_copy(out=dx[:], in_=x_tm[:, nt, :])
                nc.sync.dma_start(out=dbgx_d[nt], in_=dx[:])

        if not fl["cb"]:
            for ki in range(3):
                for ft in range(2):
                    nc.vector.tensor_scalar_add(
                        out=rep[:, ki * 2 + ft, :], in0=rep[:, ki * 2 + ft, :],
                        scalar1=cb[:, ki, ft:ft + 1])
        nc.scalar.activation(out=rep[:], in_=rep[:], func=AF.Relu)

        fps = ps_c.tile([128, 3 * NCLS], F32, tag="c")
        for c in range(6):
            nc.tensor.matmul(fps[:NSL, :], rep[:, c, :], fcw_sb[:, c, :],
                             start=c == 0, stop=c == 5)
        ob = small.tile([NSL, 3 * NCLS], F32, tag="ob")
        nc.scalar.copy(out=ob[:], in_=fps[:NSL, :])
        nc.sync.dma_start(out=out_d[:], in_=ob[:])

    nc.compile()
    return nc


def _core_inputs(inputs, fl, lay):
    """Build the 8 per-core input maps from the full problem inputs."""
    f32 = lambda a: np.ascontiguousarray(np.asarray(a, dtype=np.float32))
    tile_w = lambda w: np.ascontiguousarray(
        f32(w).reshape(w.shape[0] // 128, 128, w.shape[1]).astype(BF))

    NTT, TP, CWP = lay["NTT"], lay["TP"], lay["CWP"]
    lam, lam32, off, coff = lay["lam"], lay["lam32"], lay["off"], lay["coff"]
    assign = lay["assign"]

    shared = {}
    # packed position+type embedding
    posv = np.zeros((TP, D), np.float32)
    pe = f32(inputs["pos_emb"])
    for j in range(NSL):
        posv[off[j]:off[j] + lam32[j]] = pe[:lam32[j]]
    posv += f32(inputs["type_emb"][0])[None, :]
    shared["posty"] = np.ascontiguousarray(posv.reshape(NTT, 128, D))
    for l in range(NL):
        shared[f"wq{l}"] = tile_w(f32(inputs["Wq"][l]) / 8.0)
        shared[f"wk{l}"] = tile_w(inputs["Wk"][l])
        shared[f"wv{l}"] = tile_w(inputs["Wv"][l])
        shared[f"wo{l}"] = tile_w(inputs["Wo"][l])
        shared[f"wi{l}"] = tile_w(inputs["Wi"][l])
        shared[f"wo2{l}"] = tile_w(inputs["Wo2"][l])
        if not fl["bqk"]:
            shared[f"bq{l}"] = f32(inputs["bq"][l]).reshape(ND, 128) / 8.0
            shared[f"bk{l}"] = f32(inputs["bk"][l]).reshape(ND, 128)
        if not fl["bv"]:
            shared[f"bv{l}"] = f32(inputs["bv"][l])
        if not fl["bo"]:
            shared[f"bo{l}"] = f32(inputs["bo"][l])
        if not fl["bi"]:
            shared[f"bi{l}"] = f32(inputs["bi"][l]).reshape(NFT, 128)
        if not fl["bo2"]:
            shared[f"bo2{l}"] = f32(inputs["bo2"][l])
        if not fl["ln"]:
            shared[f"ln1s{l}"] = f32(inputs["ln1_s"][l])
            shared[f"ln1b{l}"] = f32(inputs["ln1_b"][l])
            shared[f"ln2s{l}"] = f32(inputs["ln2_s"][l])
            shared[f"ln2b{l}"] = f32(inputs["ln2_b"][l])
    if not fl["ln"]:
        shared["lnes"] = f32(inputs["emb_ln_s"])
        shared["lneb"] = f32(inputs["emb_ln_b"])
    for ki, k in enumerate((1, 2, 3)):
        w = f32(inputs[f"conv_w{k}"])                    # [NF, k, D]
        wt = np.ascontiguousarray(w.transpose(1, 2, 0))  # [k, D, NF]
        shared[f"cw{k}"] = np.ascontiguousarray(
            wt.reshape(k, ND, 128, NF).astype(BF))
    if not fl["cb"]:
        shared["convb"] = np.stack(
            [f32(inputs[f"conv_b{k}"]).reshape(2, 128) for k in (1, 2, 3)])
    # fc weights for all 3 branch-block hypotheses: [6, 128, 3*NCLS]
    fcw = f32(inputs["fc_w"])                            # [2304, NCLS]
    fcw3 = np.zeros((6, 128, 3 * NCLS), np.float32)
    for bb in range(3):
        for ki in range(3):
            for ft in range(2):
                c = ki * 2 + ft
                rows = 768 * bb + 256 * ki + 128 * ft
                fcw3[c, :, bb * NCLS:(bb + 1) * NCLS] = fcw[rows:rows + 128]
    shared["fcw"] = fcw3
    shared["word_emb"] = f32(inputs["word_emb"])

    ids_all = np.stack([np.asarray(inputs[p + "_input_ids"])
                        for p in ("q", "a", "b")]).reshape(96, S)
    mask_all = np.stack([np.asarray(inputs[p + "_attention_mask"])
                         for p in ("q", "a", "b")]).reshape(96, S)
    lens_all = mask_all.sum(1).astype(int)

    in_maps = []
    for c in range(NCORES):
        m = dict(shared)
        idv = np.zeros(TP, np.int32)
        vmv = np.zeros(TP, np.float32)
        cmv = np.zeros(CWP, np.float32)
        pen = np.full((3, CWP), -1e30, np.float32)
        for j in range(NSL):
            sq = int(assign[j, c])
            l = int(lens_all[sq])
            idv[off[j]:off[j] + lam32[j]] = ids_all[sq][:lam32[j]]
            vmv[off[j]:off[j] + l] = 1.0
            cmv[coff[j]:coff[j] + l] = 1.0
            for ki, k in enumerate((1, 2, 3)):
                nw = l - k + 2
                pen[ki, coff[j]:coff[j] + nw] = 0.0
        m["ids"] = np.ascontiguousarray(idv.reshape(NTT, 128))
        m["vmask"] = np.ascontiguousarray(vmv.reshape(NTT, 128))
        m["convmask"] = np.ascontiguousarray(cmv.astype(BF))
        m["convpen"] = np.ascontiguousarray(pen)
        in_maps.append(m)
    return in_maps


def _get_program(fl, lay, debug=False):
    key = (tuple(sorted(fl.items())), lay["key"], debug)
    if key not in _CACHE:
        _CACHE[key] = _build_program(fl, lay, debug=debug)
    return _CACHE[key]


def run_sharded(inputs, debug=False, **run_kwargs):
    """Shard, run on 8 cores, gather. Returns (output, BassKernelResults)."""
    from concourse.bass_utils import run_bass_kernel_spmd
    fl = _flags(inputs)
    lens96 = np.concatenate([
        np.asarray(inputs[p + "_attention_mask"]).sum(1) for p in ("q", "a", "b")])
    lay = _layout(lens96)
    nc = _get_program(fl, lay, debug=debug)
    in_maps = _core_inputs(inputs, fl, lay)
    res = run_bass_kernel_spmd(nc, in_maps, core_ids=list(range(NCORES)),
                               **run_kwargs)
    border = {0: 0, 1: 2, 2: 1}   # branch q/a/b -> fc block q,b,a
    out = np.zeros((B, NCLS), np.float32)
    for c in range(NCORES):
        o3 = np.asarray(res.results[c]["out"], np.float32)   # [NSL, 12]
        for j in range(NSL):
            sq = int(lay["assign"][j, c])
            br, sample = sq // B, sq % B
            out[sample] += o3[j, border[br] * NCLS:(border[br] + 1) * NCLS]
    out += np.asarray(inputs["fc_b"], np.float32)[None, :]
    return out, res


def kernel(**inputs):
    out, _ = run_sharded(inputs)
    return out
